# revision 1
# baseline (speedup 1.0000x reference)
"""Trainium2 Bass kernel for nn_Attention_43301860278871.

Full attention layer: fused QK projection + V projection, interleaved RoPE,
causal SDPA, output projection.  B=2, S=2048, D=2048, H=16, HD=128.

Sharding: 8 cores = 2 batches x 4 head-groups (tensor parallel over heads,
data parallel over batch).  Each core computes 4 heads for one batch and a
partial [S, D] output-projection contribution; the host sums the 4 partials
per batch (the wo contraction distributed over head-group slices), so no
on-device collectives are needed.

Per-core dataflow (all matmuls in float32r - measured bit-identical to fp32
on this HW, but 1 cycle/row at free-dim 512 instead of 4):
  1. Merged projection pass, x^T streamed once in 4 token chunks with all
     three weight sets resident: Q^T / K^T produced head-major ([channel, t])
     so heads feed scores directly; V token-major ([t, e]) for the PV
     contraction.  Results staged to per-chunk DRAM scratch tiles (per-chunk
     so SDPA prefetch dependencies resolve early).
  2. Interleaved RoPE fused into the projection epilogue, in channel-major
     layout: pair-swap via a permutation matmul (PE) + cos/sin multiply-add
     (DVE) against host-precomputed [128, S] factor tiles (the sin tile
     carries the +/- interleave signs).
  3. Causal SDPA, i-chunk outer / head inner: scores computed transposed
     (S^T[j, i] = K-tile^T Q-chunk, contraction = head dim), exp on ScalarE
     over j-tile PAIRS (scale folded in; no max-subtraction - scores are
     ~N(0,1) bounded so exp cannot overflow), causal masking as fp16 0/1
     multiplies on diagonal pairs only (processed first to hide their longer
     dependency chain), row-sums via ones-column matmuls accumulated in
     PSUM, PV accumulated in PSUM over j-tiles, and normalization deferred:
     1/sums broadcast to 128 partitions with a K=1 matmul and applied while
     copying PV out of PSUM.
  4. Output projection (out^T tiles stationary x wo^T moving) interleaved
     after each i-chunk so its PE work fills SDPA scheduling gaps.

Timeline-simulator exec time: ~404 us/core; rel err vs fp32 reference 4.2e-4.
"""
import sys
sys.path.insert(0, '/opt/trn_rl_repo')

import numpy as np

import concourse.bass as bass
import concourse.mybir as mybir
from concourse.bass_utils import run_bass_kernel_spmd
from concourse.tile import TileContext

B, S, D, H = 2, 2048, 2048, 16
HD = D // H            # 128
G = 4                  # head-groups (cores per batch)
HPG = H // G           # heads per core = 4
E = HPG * HD           # per-core projection width = 512
ROPE_BASE = 10000.0
SCALE = float(HD) ** -0.5

f32 = mybir.dt.float32
f32r = mybir.dt.float32r

KT = D // 128          # 16 contraction tiles
TT = S // 128          # 16 token tiles
TC = S // 512          # 4 token chunks
ET = E // 128          # 4 e-tiles (= heads per core)


# ---------------------------------------------------------------------------
# Workarounds for this walrus build: at most ONE sem wait per instruction.
# Tile's scheduler attaches several; hoist the excess onto NoOps injected on
# the same engine immediately before (sequencer executes waits in order, so
# semantics are identical).
# ---------------------------------------------------------------------------

def _patched_drain_and_barrier(self, tick_clock, wait_clock):
    from concourse.vector_clock import ScopedClock
    drain_inst = self.nc.sync.drain()
    wait_clock.add_sem_waits(
        drain_inst.ins, ScopedClock({None: tick_clock.global_clock})
    )
    si = drain_inst.ins.sync_info
    if si is not None and si.on_wait and len(si.on_wait) > 1:
        waits = list(si.on_wait)
        si.on_wait = waits[:1]
        for w in waits[1:]:
            extra = self.nc.sync.drain()
            esi = extra.ins.sync_info
            if esi is None:
                extra.ins.sync_info = mybir.SyncInfo(on_wait=[w], on_update=[])
            else:
                esi.on_wait = [w]

    self.nc.all_engine_barrier()
    assert self.sems is not None
    popped = self.nc._tile_sem_poison_stack.pop()
    assert popped is self._sem_poison
    self.nc.clear_and_free_semaphores(list(self.sems.allocated().values()))
    self.nc.all_engine_barrier()


def _install_tile_patch():
    import concourse.tile as tile_mod
    tile_mod.TileContext._drain_and_barrier = _patched_drain_and_barrier


def _split_waits(nc, max_waits: int = 1):
    for fn in nc.m.functions:
        for bb in fn.blocks:
            out = []
            changed = False
            for inst in list(bb.instructions):
                si = inst.sync_info
                if si is not None and si.on_wait and len(si.on_wait) > max_waits:
                    waits = list(si.on_wait)
                    for w in waits[:-max_waits]:
                        out.append(mybir.InstNoOp(
                            name=nc.get_next_instruction_name(),
                            engine=inst.engine,
                            sync_info=mybir.SyncInfo(on_wait=[w], on_update=[]),
                        ))
                    si.on_wait = waits[-max_waits:]
                    changed = True
                out.append(inst)
            if changed:
                bb.instructions = out


# ---------------------------------------------------------------------------
# Kernel build (one Bass module, SPMD across the 8 cores via input slices)
# ---------------------------------------------------------------------------

def _build_nc():
    _install_tile_patch()
    nc = bass.Bass()

    xT = nc.dram_tensor("xT", [128, KT, S], f32r, kind="ExternalInput")
    wqT = nc.dram_tensor("wqT", [128, KT, ET, 128], f32r, kind="ExternalInput")
    wkT = nc.dram_tensor("wkT", [128, KT, ET, 128], f32r, kind="ExternalInput")
    wvT = nc.dram_tensor("wvT", [128, KT, E], f32r, kind="ExternalInput")
    woT = nc.dram_tensor("woT", [128, ET, D], f32r, kind="ExternalInput")
    cosF = nc.dram_tensor("cosF", [128, S], f32, kind="ExternalInput")
    sinF = nc.dram_tensor("sinF", [128, S], f32, kind="ExternalInput")
    rperm = nc.dram_tensor("rperm", [128, 128], f32r, kind="ExternalInput")
    masks = nc.dram_tensor("masks", [128, ET, 512], mybir.dt.float16, kind="ExternalInput")
    onesc = nc.dram_tensor("onesc", [128, 1], f32r, kind="ExternalInput")
    onesr = nc.dram_tensor("onesr", [1, 128], f32r, kind="ExternalInput")
    out = nc.dram_tensor("out", [S, D], f32, kind="ExternalOutput")

    Exp = mybir.ActivationFunctionType.Exp
    mult = mybir.AluOpType.mult
    add = mybir.AluOpType.add

    with TileContext(nc) as tc:
        with (
            nc.allow_low_precision(reason="float32r is 4-byte fp32 for PE"),
            tc.tile_pool(name="consts", bufs=1) as consts,
            tc.tile_pool(name="dram", bufs=1, space="DRAM") as dram,
        ):
            m_sb = consts.tile([128, ET, 512], mybir.dt.float16)
            rp_sb = consts.tile([128, 128], f32r)
            oc_sb = consts.tile([128, 1], f32r)
            or_sb = consts.tile([1, 128], f32r)
            nc.sync.dma_start(rp_sb[:], rperm[:])
            nc.sync.dma_start(oc_sb[:], onesc[:])
            nc.sync.dma_start(or_sb[:], onesr[:])

            qt_s = [dram.tile([128, ET, 512], f32r, tag=f"qt{t}", name=f"qt{t}")
                    for t in range(TC)]
            kt_s = [dram.tile([128, ET, 512], f32r, tag=f"kt{t}", name=f"kt{t}")
                    for t in range(TC)]
            v_s = [dram.tile([128, 4, E], f32r, tag=f"v{t}", name=f"v{t}")
                   for t in range(TC)]

            # ---- Phase 1+2: merged Q/K/V projection, x read once ----
            with (
                tc.tile_pool(name="wpool", bufs=1) as wpool,
                tc.tile_pool(name="xpool", bufs=2) as xpool,
                tc.tile_pool(name="trig", bufs=2) as trig,
                tc.tile_pool(name="stage", bufs=3) as stage,
                tc.tile_pool(name="psA", bufs=4, space="PSUM") as psA,
                tc.tile_pool(name="psB", bufs=2, space="PSUM") as psB,
            ):
                wq_sb = wpool.tile([128, KT, ET, 128], f32r, tag="wq")
                wk_sb = wpool.tile([128, KT, ET, 128], f32r, tag="wk")
                wv_sb = wpool.tile([128, KT, E], f32r, tag="wv")

                for tcb in range(TC):
                    ts = slice(tcb * 512, (tcb + 1) * 512)
                    xc = xpool.tile([128, KT, 512], f32r, tag="xc")
                    if tcb == 0:
                        # interleave first-chunk x and wq loads, singles first,
                        # so the k=0 matmul starts after ~0.5MB
                        for k in range(4):
                            nc.sync.dma_start(wq_sb[:, k:k + 1], wqT[:, k:k + 1])
                            nc.sync.dma_start(xc[:, k:k + 1], xT[:, k:k + 1, ts])
                        for kc in range(1, 4):
                            ks = slice(kc * 4, (kc + 1) * 4)
                            nc.sync.dma_start(wq_sb[:, ks], wqT[:, ks])
                            nc.sync.dma_start(xc[:, ks], xT[:, ks, ts])
                    else:
                        for kc in range(4):
                            ks = slice(kc * 4, (kc + 1) * 4)
                            nc.sync.dma_start(xc[:, ks], xT[:, ks, ts])
                    c_sb = trig.tile([128, 512], f32, tag="cos")
                    s_sb = trig.tile([128, 512], f32, tag="sin")
                    nc.sync.dma_start(c_sb[:], cosF[:, ts])
                    nc.sync.dma_start(s_sb[:], sinF[:, ts])
                    if tcb == 0:
                        for kc in range(4):
                            ks = slice(kc * 4, (kc + 1) * 4)
                            nc.sync.dma_start(wk_sb[:, ks], wkT[:, ks])
                        for kc in range(4):
                            ks = slice(kc * 4, (kc + 1) * 4)
                            nc.sync.dma_start(wv_sb[:, ks], wvT[:, ks])
                    # Q^T and K^T e-major + RoPE
                    for (w_sb, dst) in ((wq_sb, qt_s[tcb]), (wk_sb, kt_s[tcb])):
                        for et in range(ET):
                            pq = psA.tile([128, 512], f32, tag="acc")
                            for k in range(KT):
                                nc.tensor.matmul(
                                    pq[:], w_sb[:, k, et, :], xc[:, k, :],
                                    start=(k == 0), stop=(k == KT - 1),
                                )
                            qsb = stage.tile([128, 512], f32r, tag="qsb")
                            nc.scalar.copy(qsb[:], pq[:])
                            ps2 = psB.tile([128, 512], f32, tag="aux")
                            nc.tensor.matmul(ps2[:], rp_sb[:], qsb[:],
                                             start=True, stop=True)
                            t1 = stage.tile([128, 512], f32r, tag="t1")
                            nc.vector.tensor_tensor(
                                t1.bitcast(f32), qsb.bitcast(f32), c_sb[:], mult)
                            t2 = stage.tile([128, 512], f32, tag="t2")
                            nc.vector.tensor_tensor(t2[:], ps2[:], s_sb[:], mult)
                            nc.vector.tensor_tensor(t1[:], t1.bitcast(f32), t2[:], add)
                            nc.sync.dma_start(dst[:, et, :], t1[:])
                    # V t-major
                    for tt in range(4):
                        pv = psA.tile([128, 512], f32, tag="acc")
                        for k in range(KT):
                            nc.tensor.matmul(
                                pv[:], xc[:, k, tt * 128:(tt + 1) * 128], wv_sb[:, k, :],
                                start=(k == 0), stop=(k == KT - 1),
                            )
                        vsb = stage.tile([128, 512], f32r, tag="qsb")
                        nc.scalar.copy(vsb[:], pv[:])
                        nc.sync.dma_start(v_s[tcb][:, tt, :], vsb[:])

            # ---- Phase 3+4: SDPA (ic-outer, all heads resident) with the
            # output projection interleaved per i-chunk so its PE work fills
            # SDPA scheduling gaps ----
            with (
                tc.tile_pool(name="hpool", bufs=1) as hpool,
                tc.tile_pool(name="outT", bufs=1) as outTp,
                tc.tile_pool(name="wpool2", bufs=1) as wpool2,
                tc.tile_pool(name="ptpool", bufs=3) as ptpool,
                tc.tile_pool(name="qpool", bufs=2) as qpool,
                tc.tile_pool(name="stage2", bufs=2) as stage2,
                tc.tile_pool(name="ost", bufs=3) as ostp,
                tc.tile_pool(name="psC", bufs=2, space="PSUM") as psC,
                tc.tile_pool(name="psD", bufs=1, space="PSUM") as psD,
                tc.tile_pool(name="psE", bufs=1, space="PSUM") as psE,
                tc.tile_pool(name="psF", bufs=2, space="PSUM") as psF,
            ):
                wo_sb = wpool2.tile([128, ET, D], f32r, tag="wo")
                # per-chunk combined tiles: one DMA each (SWDGE fixed cost
                # per dma_start is ~2us on the single gpsimd queue)
                kth_c = [hpool.tile([128, ET, 512], f32r, tag=f"kc{t}", name=f"kc{t}")
                         for t in range(TC)]
                vth_c = [hpool.tile([128, 4, E], f32r, tag=f"vc{t}", name=f"vc{t}")
                         for t in range(TC)]
                qtb_c = {}

                def _load_block_inputs(tcb, eng):
                    eng.dma_start(kth_c[tcb][:], kt_s[tcb][:])
                    q = qpool.tile([128, ET, 512], f32r, tag="qc", name=f"qc{tcb}")
                    eng.dma_start(q[:], qt_s[tcb][:])
                    qtb_c[tcb] = q
                    eng.dma_start(vth_c[tcb][:], v_s[tcb][:])

                nc.gpsimd.dma_start(m_sb[:], masks[:])
                _load_block_inputs(0, nc.sync)
                for dcc in range(4):
                    dsl = slice(dcc * 512, (dcc + 1) * 512)
                    nc.sync.dma_start(wo_sb[:, :, dsl], woT[:, :, dsl])

                for ic in range(TC):
                    nj = 4 * (ic + 1)
                    npair = nj // 2
                    isl = slice(ic * 512, (ic + 1) * 512)
                    if ic + 1 < TC:
                        _load_block_inputs(ic + 1, nc.gpsimd)
                    oT_ic = outTp.tile([128, ET, 512], f32r, tag=f"oT{ic}")
                    for h in range(ET):
                        qtb = qtb_c[ic][:, h, :]
                        ps_out = psD.tile([128, 512], f32, tag="pv")
                        ps_sums = psE.tile([128, 512], f32, tag="sums")
                        # diagonal pairs first: their exp->mask chain is the
                        # longest; lower tiles then keep the PE fed
                        pairs = list(range(2 * ic, npair)) + list(range(0, 2 * ic))
                        first, last = pairs[0], pairs[-1]
                        for p in pairs:
                            ps_sc = psC.tile([128, 2, 512], f32, tag="sc")
                            for half in range(2):
                                jt = 2 * p + half
                                nc.tensor.matmul(
                                    ps_sc[:, half, :],
                                    kth_c[jt // 4][:, h, (jt % 4) * 128:(jt % 4 + 1) * 128],
                                    qtb,
                                    start=True, stop=True,
                                )
                            pt = ptpool.tile([128, 2, 512], f32r, tag="pt")
                            nc.scalar.activation(pt[:], ps_sc[:], Exp, scale=SCALE)
                            m = 2 * p - 4 * ic
                            if m >= 0:
                                nc.vector.tensor_tensor(
                                    pt[:], pt.bitcast(f32), m_sb[:, m:m + 2, :], mult)
                            for half in range(2):
                                jt = 2 * p + half
                                st = (p == first and half == 0)
                                sp = (p == last and half == 1)
                                nc.tensor.matmul(ps_sums[0:1, :], oc_sb[:],
                                                 pt[:, half, :], start=st, stop=sp)
                                nc.tensor.matmul(
                                    ps_out[:],
                                    vth_c[jt // 4][:, jt % 4, h * 128:(h + 1) * 128],
                                    pt[:, half, :], start=st, stop=sp)
                        od_raw = stage2.tile([128, 512], f32, tag="odraw")
                        nc.scalar.copy(od_raw[:], ps_out[:])
                        rc = stage2.tile([1, 512], f32r, tag="rc")
                        nc.vector.reciprocal(rc[:], ps_sums[0:1, :])
                        pb = psE.tile([128, 512], f32, tag="sums")
                        nc.tensor.matmul(pb[:], or_sb[:], rc[:],
                                         start=True, stop=True)
                        pbs = stage2.tile([128, 512], f32, tag="pbs")
                        nc.vector.tensor_copy(pbs[:], pb[:])
                        nc.vector.tensor_tensor(oT_ic[:, h, :], od_raw[:], pbs[:], mult)

                    # output projection for the 4 token tiles of this i-chunk
                    for tl in range(4):
                        tt = 4 * ic + tl
                        ost = ostp.tile([128, D], f32, tag="ost")
                        for dc in range(4):
                            po = psF.tile([128, 512], f32, tag="acc")
                            for eh in range(ET):
                                nc.tensor.matmul(
                                    po[:],
                                    oT_ic[:, eh, tl * 128:(tl + 1) * 128],
                                    wo_sb[:, eh, dc * 512:(dc + 1) * 512],
                                    start=(eh == 0), stop=(eh == ET - 1),
                                )
                            nc.vector.tensor_copy(ost[:, dc * 512:(dc + 1) * 512], po[:])
                        nc.sync.dma_start(out[tt * 128:(tt + 1) * 128, :], ost[:])

    _split_waits(nc)
    return nc


_NC = None


def _get_nc():
    global _NC
    if _NC is None:
        _NC = _build_nc()
    return _NC


# ---------------------------------------------------------------------------
# Host-side prep + gather
# ---------------------------------------------------------------------------

def _rope_tables():
    j = np.arange(0, HD, 2, dtype=np.float32) / HD
    inv_freq = (1.0 / (ROPE_BASE ** j)).astype(np.float32)          # [64]
    t = np.arange(S, dtype=np.float32)
    freqs = np.outer(t, inv_freq)                                    # [S, 64]
    cos = np.cos(freqs).astype(np.float32)                           # [S, 64]
    sin = np.sin(freqs).astype(np.float32)
    cosF = np.empty((128, S), dtype=np.float32)
    sinF = np.empty((128, S), dtype=np.float32)
    cosF[0::2, :] = cos.T
    cosF[1::2, :] = cos.T
    sinF[0::2, :] = -sin.T
    sinF[1::2, :] = sin.T
    return cosF, sinF


def _static_inputs():
    cosF, sinF = _rope_tables()
    rperm = np.zeros((128, 128), dtype=np.float32)
    idx = np.arange(128)
    rperm[idx ^ 1, idx] = 1.0
    masks = np.zeros((128, ET, 512), dtype=np.float16)
    il = np.arange(512)
    for m in range(ET):
        for p in range(128):
            masks[p, m, :] = (il >= 128 * m + p).astype(np.float16)
    onesc = np.ones((128, 1), dtype=np.float32)
    onesr = np.ones((1, 128), dtype=np.float32)
    return {
        "cosF": cosF, "sinF": sinF, "rperm": rperm,
        "masks": masks, "onesc": onesc, "onesr": onesr,
    }


def _core_inputs(x, wqk, wv, wo, static, b, g):
    xb = np.ascontiguousarray(x[b].T)                                # [D, S]
    xT = np.ascontiguousarray(
        xb.reshape(KT, 128, S).transpose(1, 0, 2))                   # [128, KT, S]

    wq_g = wqk[E * g:E * (g + 1), :]                                 # [E, D]
    wk_g = wqk[D + E * g:D + E * (g + 1), :]
    wv_g = wv[E * g:E * (g + 1), :]
    wqT = np.ascontiguousarray(
        wq_g.T.reshape(KT, 128, ET, 128).transpose(1, 0, 2, 3))
    wkT = np.ascontiguousarray(
        wk_g.T.reshape(KT, 128, ET, 128).transpose(1, 0, 2, 3))
    wvT = np.ascontiguousarray(
        wv_g.T.reshape(KT, 128, E).transpose(1, 0, 2))
    woT = np.ascontiguousarray(
        wo[:, E * g:E * (g + 1)].T.reshape(ET, 128, D).transpose(1, 0, 2))

    m = dict(static)
    m.update({"xT": xT, "wqT": wqT, "wkT": wkT, "wvT": wvT, "woT": woT})
    return m


def kernel(x, wqk, wv, wo):
    x = np.asarray(x, dtype=np.float32)
    wqk = np.asarray(wqk, dtype=np.float32)
    wv = np.asarray(wv, dtype=np.float32)
    wo = np.asarray(wo, dtype=np.float32)

    nc = _get_nc()
    static = _static_inputs()
    in_maps = [
        _core_inputs(x, wqk, wv, wo, static, c // G, c % G) for c in range(8)
    ]
    res = run_bass_kernel_spmd(nc, in_maps, core_ids=list(range(8)))
    out = np.zeros((B, S, D), dtype=np.float32)
    for c in range(8):
        out[c // G] += res.results[c]["out"]
    return out



# revision 23
# speedup vs baseline: 1.1562x; 1.1562x over previous
"""Trainium2 Bass kernel for nn_Attention_43301860278871.

Full attention layer: fused QK projection + V projection, interleaved RoPE,
causal SDPA, output projection.  B=2, S=2048, D=2048, H=16, HD=128.

Sharding: 8 cores = 2 batches x 4 head-groups (tensor parallel over heads,
data parallel over batch).  Each core computes 4 heads for one batch and a
partial [S, D] output-projection contribution; the host sums the 4 partials
per batch, so no on-device collectives are needed.

v2 design (vs the f32r/DRAM-staging baseline):
  - All matmul operands in bf16 (host-converted); PSUM accumulation stays
    f32.  Q/K/V live in SBUF for the whole kernel - no DRAM round-trip and
    no phase barrier.
  - RoPE pair-swap via a bf16 permutation matmul (DVE lanes are
    partition-locked, so the swap cannot run there), then cos/sin
    multiply-add on DVE with bf16 2x modes where operands allow.
  - Causal masking: scores are computed per 128-wide j-tile with true
    (128-granular) causality; the boundary-diagonal 128x128 piece gets a
    second matmul accumulating -1e9 * max(0, j-i) into the scores PSUM
    (lower-tri x upper-tri constant operands), so exp() produces exact
    zeros and no mask multiply exists on the DVE critical path.
  - Softmax row sums: exp tiles (bf16) are accumulated over j-tiles on DVE
    (scalar_tensor_tensor, 4x mode) and reduced with ONE ones-column
    matmul per (head, i-chunk) instead of one per j-tile.
  - 1/sums broadcast via gpsimd partition_broadcast (Pool engine).
  - Output projection interleaved one i-chunk behind SDPA, one t-tile per
    head, in 4-matmul quanta between score tiles, so the Tensor engine
    stays fed while ACT streams exp().
"""
import sys
sys.path.insert(0, '/opt/trn_rl_repo')

import numpy as np
import ml_dtypes

import concourse.bass as bass
import concourse.mybir as mybir
from concourse.bass_utils import run_bass_kernel_spmd
from concourse.tile import TileContext

B, S, D, H = 2, 2048, 2048, 16
HD = D // H            # 128
G = 4                  # head-groups (cores per batch)
HPG = H // G           # heads per core = 4
E = HPG * HD           # per-core projection width = 512
ROPE_BASE = 10000.0
SCALE = float(HD) ** -0.5

f32 = mybir.dt.float32
bf16 = mybir.dt.bfloat16
npbf16 = ml_dtypes.bfloat16

KT = D // 128          # 16 contraction tiles
TT = S // 128          # 16 token tiles
TC = S // 512          # 4 token chunks
ET = E // 128          # 4 e-tiles (= heads per core)

Exp = mybir.ActivationFunctionType.Exp
mult = mybir.AluOpType.mult
add = mybir.AluOpType.add


# ---------------------------------------------------------------------------
# Workarounds for this walrus build: at most ONE sem wait per instruction.
# Tile's scheduler attaches several; hoist the excess onto NoOps injected on
# the same engine immediately before (sequencer executes waits in order, so
# semantics are identical).
# ---------------------------------------------------------------------------

def _patched_drain_and_barrier(self, tick_clock, wait_clock):
    from concourse.vector_clock import ScopedClock
    drain_inst = self.nc.sync.drain()
    wait_clock.add_sem_waits(
        drain_inst.ins, ScopedClock({None: tick_clock.global_clock})
    )
    si = drain_inst.ins.sync_info
    if si is not None and si.on_wait and len(si.on_wait) > 1:
        waits = list(si.on_wait)
        si.on_wait = waits[:1]
        for w in waits[1:]:
            extra = self.nc.sync.drain()
            esi = extra.ins.sync_info
            if esi is None:
                extra.ins.sync_info = mybir.SyncInfo(on_wait=[w], on_update=[])
            else:
                esi.on_wait = [w]

    self.nc.all_engine_barrier()
    assert self.sems is not None
    popped = self.nc._tile_sem_poison_stack.pop()
    assert popped is self._sem_poison
    self.nc.clear_and_free_semaphores(list(self.sems.allocated().values()))
    self.nc.all_engine_barrier()


def _install_tile_patch():
    import concourse.tile as tile_mod
    tile_mod.TileContext._drain_and_barrier = _patched_drain_and_barrier


def _split_waits(nc, max_waits: int = 1):
    for fn in nc.m.functions:
        for bb in fn.blocks:
            out = []
            changed = False
            for inst in list(bb.instructions):
                si = inst.sync_info
                if si is not None and si.on_wait and len(si.on_wait) > max_waits:
                    waits = list(si.on_wait)
                    for w in waits[:-max_waits]:
                        out.append(mybir.InstNoOp(
                            name=nc.get_next_instruction_name(),
                            engine=inst.engine,
                            sync_info=mybir.SyncInfo(on_wait=[w], on_update=[]),
                        ))
                    si.on_wait = waits[-max_waits:]
                    changed = True
                out.append(inst)
            if changed:
                bb.instructions = out


# ---------------------------------------------------------------------------
# Kernel build (one Bass module, SPMD across the 8 cores via input slices)
# ---------------------------------------------------------------------------

def _build_nc():
    _install_tile_patch()
    nc = bass.Bass()

    xT = nc.dram_tensor("xT", [128, KT, S], bf16, kind="ExternalInput")
    wqT = nc.dram_tensor("wqT", [128, KT, ET, 128], bf16, kind="ExternalInput")
    wkT = nc.dram_tensor("wkT", [128, KT, ET, 128], bf16, kind="ExternalInput")
    wvT = nc.dram_tensor("wvT", [128, KT, E], bf16, kind="ExternalInput")
    woT = nc.dram_tensor("woT", [128, ET, D], bf16, kind="ExternalInput")
    cosF = nc.dram_tensor("cosF", [128, S], bf16, kind="ExternalInput")
    sinF = nc.dram_tensor("sinF", [128, S], bf16, kind="ExternalInput")
    triL = nc.dram_tensor("triL", [128, 128], bf16, kind="ExternalInput")
    triU = nc.dram_tensor("triU", [128, 128], bf16, kind="ExternalInput")
    rperm = nc.dram_tensor("rperm", [128, 128], bf16, kind="ExternalInput")
    onesb = nc.dram_tensor("onesb", [128, 1], bf16, kind="ExternalInput")
    onesr = nc.dram_tensor("onesr", [1, 128], bf16, kind="ExternalInput")
    out = nc.dram_tensor("out", [S, D], bf16, kind="ExternalOutput")

    with TileContext(nc) as tc:
        with (
            nc.allow_low_precision(reason="bf16 datapath, fp32 accumulation"),
            tc.tile_pool(name="consts", bufs=1) as consts,
            tc.tile_pool(name="resid", bufs=1) as resid,
        ):
            lt_sb = consts.tile([128, 128], bf16, tag="lt")
            ut_sb = consts.tile([128, 128], bf16, tag="ut")
            ob_sb = consts.tile([128, 1], bf16, tag="ob")
            or_sb = consts.tile([1, 128], bf16, tag="or")
            rp_sb = consts.tile([128, 128], bf16, tag="rp")
            c_sb = consts.tile([128, S], bf16, tag="cos")
            s_sb = consts.tile([128, S], bf16, tag="sin")
            nc.sync.dma_start(lt_sb[:], triL[:])
            nc.sync.dma_start(ut_sb[:], triU[:])
            nc.sync.dma_start(ob_sb[:], onesb[:])
            nc.sync.dma_start(or_sb[:], onesr[:])
            nc.sync.dma_start(rp_sb[:], rperm[:])

            # SBUF-resident Q^T/K^T (e-major per head) and V (t-major)
            qres = resid.tile([128, ET, S], bf16, tag="qres")
            kres = resid.tile([128, ET, S], bf16, tag="kres")
            vres = resid.tile([128, TT, E], bf16, tag="vres")
            wo_sb = resid.tile([128, ET, D], bf16, tag="wo")

            # ---- Phase 1: merged Q/K/V projection + RoPE, x read once ----
            with (
                tc.tile_pool(name="wpool", bufs=1) as wpool,
                tc.tile_pool(name="xpool", bufs=2) as xpool,
                tc.tile_pool(name="stage", bufs=4) as stage,
                tc.tile_pool(name="psA", bufs=4, space="PSUM") as psA,
                tc.tile_pool(name="psB", bufs=2, space="PSUM") as psB,
            ):
                wq_sb = wpool.tile([128, KT, ET, 128], bf16, tag="wq")
                wk_sb = wpool.tile([128, KT, ET, 128], bf16, tag="wk")
                wv_sb = wpool.tile([128, KT, E], bf16, tag="wv")

                xc_t = {}
                for tcb in range(TC):
                    ts = slice(tcb * 512, (tcb + 1) * 512)
                    if tcb == 0:
                        xc = xpool.tile([128, KT, 512], bf16, tag="xc")
                        xc_t[0] = xc
                        # interleave first-chunk x and wq loads, singles
                        # first, so the k=0 matmul starts early
                        for k in range(4):
                            nc.sync.dma_start(wq_sb[:, k:k + 1], wqT[:, k:k + 1])
                            nc.sync.dma_start(xc[:, k:k + 1], xT[:, k:k + 1, ts])
                        for kc in range(1, 4):
                            ks = slice(kc * 4, (kc + 1) * 4)
                            nc.sync.dma_start(wq_sb[:, ks], wqT[:, ks])
                            nc.sync.dma_start(xc[:, ks], xT[:, ks, ts])
                        for kc in range(4):
                            ks = slice(kc * 4, (kc + 1) * 4)
                            nc.sync.dma_start(wk_sb[:, ks], wkT[:, ks])
                        for kc in range(4):
                            ks = slice(kc * 4, (kc + 1) * 4)
                            nc.sync.dma_start(wv_sb[:, ks], wvT[:, ks])
                        nc.sync.dma_start(c_sb[:], cosF[:])
                        nc.sync.dma_start(s_sb[:], sinF[:])
                    xc = xc_t[tcb]
                    if tcb + 1 < TC:
                        nts = slice((tcb + 1) * 512, (tcb + 2) * 512)
                        nxc = xpool.tile([128, KT, 512], bf16, tag="xc")
                        xc_t[tcb + 1] = nxc
                        for kc in range(4):
                            ks = slice(kc * 4, (kc + 1) * 4)
                            nc.sync.dma_start(nxc[:, ks], xT[:, ks, nts])
                    if tcb == TC - 1:
                        for dcc in range(4):
                            dsl = slice(dcc * 512, (dcc + 1) * 512)
                            nc.sync.dma_start(wo_sb[:, :, dsl], woT[:, :, dsl])

                    # Q^T / K^T e-major with rotate-half RoPE fused
                    for (w_sb, dst) in ((wq_sb, qres), (wk_sb, kres)):
                        for et in range(ET):
                            pq = psA.tile([128, 512], f32, tag="acc")
                            for k in range(KT):
                                nc.tensor.matmul(
                                    pq[:], w_sb[:, k, et, :], xc[:, k, :],
                                    start=(k == 0), stop=(k == KT - 1),
                                )
                            qsb = stage.tile([128, 512], bf16, tag="qsb")
                            nc.scalar.copy(qsb[:], pq[:])
                            ps2 = psB.tile([128, 512], f32, tag="swap")
                            nc.tensor.matmul(ps2[:], rp_sb[:], qsb[:],
                                             start=True, stop=True)
                            t1 = stage.tile([128, 512], bf16, tag="t1")
                            t2 = stage.tile([128, 512], bf16, tag="t2")
                            nc.vector.tensor_tensor(t1[:], qsb[:], c_sb[:, ts], mult)
                            nc.vector.tensor_tensor(t2[:], ps2[:], s_sb[:, ts], mult)
                            nc.vector.tensor_tensor(dst[:, et, ts], t1[:], t2[:], add)
                    # V t-major
                    for tt in range(4):
                        pv = psA.tile([128, 512], f32, tag="acc")
                        for k in range(KT):
                            nc.tensor.matmul(
                                pv[:], xc[:, k, tt * 128:(tt + 1) * 128],
                                wv_sb[:, k, :],
                                start=(k == 0), stop=(k == KT - 1),
                            )
                        nc.scalar.copy(vres[:, 4 * tcb + tt, :], pv[:])

            # ---- Phase 2: SDPA with fine-grained causality; output
            # projection interleaved one i-chunk behind ----
            with (
                tc.tile_pool(name="oTp", bufs=2) as oTp,
                tc.tile_pool(name="ptp", bufs=4) as ptp,
                tc.tile_pool(name="accp", bufs=2) as accp,
                tc.tile_pool(name="rcp", bufs=2) as rcp,
                tc.tile_pool(name="ostp", bufs=2) as ostp,
                tc.tile_pool(name="psC", bufs=2, space="PSUM") as psC,
                tc.tile_pool(name="psD", bufs=2, space="PSUM") as psD,
                tc.tile_pool(name="psE", bufs=1, space="PSUM") as psE,
                tc.tile_pool(name="psG", bufs=1, space="PSUM") as psG,
                tc.tile_pool(name="psF", bufs=2, space="PSUM") as psF,
            ):
                oT_c = {}

                def emit_outproj_tile(tt, oT):
                    # one t-tile of the output projection: 4 dc-quanta of 4
                    # matmuls each, returned as callables to interleave
                    tl = tt % 4
                    quanta = []
                    ost = ostp.tile([128, D], bf16, tag="ost")

                    def mk(dc):
                        def q():
                            po = psF.tile([128, 512], f32, tag="acc")
                            for eh in range(ET):
                                nc.tensor.matmul(
                                    po[:],
                                    oT[:, eh, tl * 128:(tl + 1) * 128],
                                    wo_sb[:, eh, dc * 512:(dc + 1) * 512],
                                    start=(eh == 0), stop=(eh == ET - 1),
                                )
                            dsl = slice(dc * 512, (dc + 1) * 512)
                            if dc < 3:
                                nc.vector.tensor_copy(ost[:, dsl], po[:])
                            else:
                                nc.scalar.copy(ost[:, dsl], po[:])
                        return q
                    for dc in range(4):
                        quanta.append(mk(dc))

                    def fin():
                        nc.sync.dma_start(out[tt * 128:(tt + 1) * 128, :], ost[:])
                    return quanta, fin

                for ic in range(TC):
                    isl = slice(ic * 512, (ic + 1) * 512)
                    oT_ic = oTp.tile([128, ET, 512], bf16, tag="oT")
                    oT_c[ic] = oT_ic
                    for h in range(ET):
                        # deferred output projection work (one ic behind)
                        if ic > 0:
                            quanta, fin = emit_outproj_tile(
                                4 * (ic - 1) + h, oT_c[ic - 1])
                        else:
                            quanta, fin = [], None
                        qtb = qres[:, h, isl]
                        ps_pv = psD.tile([128, 512], f32, tag="pv")
                        acc = accp.tile([128, 512], bf16, tag="acc")
                        # j-tiles: full below the diagonal block, then the
                        # 4 staircase tiles (i-extent shrinks by 128 each)
                        tiles = [(jt, 0) for jt in range(4 * ic)]
                        tiles += [(4 * ic + r, 128 * r) for r in range(4)]
                        n = len(tiles)
                        pts = [None] * n
                        LOOK = 2

                        def emit_pv(idx):
                            jt, ilo = tiles[idx]
                            nc.tensor.matmul(
                                ps_pv[:, ilo:512],
                                vres[:, jt, h * 128:(h + 1) * 128],
                                pts[idx][:, ilo:512],
                                start=(idx == 0), stop=(idx == n - 1),
                                skip_group_check=True,
                            )

                        for idx, (jt, ilo) in enumerate(tiles):
                            ps_sc = psC.tile([128, 512], f32, tag="sc")
                            nc.tensor.matmul(
                                ps_sc[:, ilo:512],
                                kres[:, h, jt * 128:(jt + 1) * 128],
                                qtb[:, ilo:512],
                                start=True, stop=True,
                            )
                            if jt >= 4 * ic:
                                # boundary-diagonal piece: accumulate
                                # -1e5*max(0, j-i) so exp gives exact zeros
                                nc.tensor.matmul(
                                    ps_sc[:, ilo:ilo + 128],
                                    lt_sb[:], ut_sb[:],
                                    start=False, stop=True,
                                    skip_group_check=True,
                                )
                            pt = ptp.tile([128, 512], bf16, tag="pt")
                            pts[idx] = pt
                            nc.scalar.activation(
                                pt[:, ilo:512], ps_sc[:, ilo:512], Exp,
                                scale=SCALE)
                            if idx == 0:
                                nc.vector.tensor_copy(acc[:], pt[:])
                            else:
                                nc.vector.scalar_tensor_tensor(
                                    acc[:, ilo:512], pt[:, ilo:512], 1.0,
                                    acc[:, ilo:512], mult, add)
                            if idx < len(quanta):
                                quanta[idx]()
                            if idx >= LOOK:
                                emit_pv(idx - LOOK)
                        for idx in range(max(0, n - LOOK), n):
                            emit_pv(idx)
                        if fin is not None:
                            fin()
                        # normalization: sums -> 1/sums -> broadcast -> mult
                        ps_sums = psE.tile([1, 512], f32, tag="sums")
                        nc.tensor.matmul(ps_sums[:], ob_sb[:], acc[:],
                                         start=True, stop=True)
                        rc = rcp.tile([1, 512], bf16, tag="rc")
                        rcb = rcp.tile([128, 512], bf16, tag="rcb")
                        nc.vector.reciprocal(rc[:], ps_sums[:])
                        pb = psG.tile([128, 512], f32, tag="pb")
                        nc.tensor.matmul(pb[:], or_sb[:], rc[:],
                                         start=True, stop=True)
                        nc.vector.tensor_copy(rcb[:], pb[:])
                        nc.vector.tensor_tensor(
                            oT_ic[:, h, :], ps_pv[:], rcb[:], mult)

                # tail: output projection for the last i-chunk
                for h in range(ET):
                    quanta, fin = emit_outproj_tile(4 * (TC - 1) + h, oT_c[TC - 1])
                    for q in quanta:
                        q()
                    fin()

    _split_waits(nc)
    return nc


_NC = None


def _get_nc():
    global _NC
    if _NC is None:
        _NC = _build_nc()
    return _NC


# ---------------------------------------------------------------------------
# Host-side prep + gather
# ---------------------------------------------------------------------------

def _rope_tables():
    j = np.arange(0, HD, 2, dtype=np.float32) / HD
    inv_freq = (1.0 / (ROPE_BASE ** j)).astype(np.float32)           # [64]
    t = np.arange(S, dtype=np.float32)
    freqs = np.outer(inv_freq, t)                                    # [64, S]
    cos = np.cos(freqs)
    sin = np.sin(freqs)
    cosF = np.empty((128, S), dtype=np.float32)
    sinF = np.empty((128, S), dtype=np.float32)
    cosF[0::2] = cos
    cosF[1::2] = cos
    sinF[0::2] = -sin
    sinF[1::2] = sin
    return cosF.astype(npbf16), sinF.astype(npbf16)


def _static_inputs():
    cosF, sinF = _rope_tables()
    k = np.arange(128)
    triL = (k[:, None] < k[None, :]).astype(np.float32)      # [k, j] k<j
    triU = np.where(k[:, None] >= k[None, :], -1e5, 0.0)     # [k, i] k>=i
    onesb = np.ones((128, 1), dtype=np.float32)
    onesr = np.ones((1, 128), dtype=np.float32)
    rperm = np.zeros((128, 128), dtype=np.float32)
    idx = np.arange(128)
    rperm[idx ^ 1, idx] = 1.0
    return {
        "cosF": cosF, "sinF": sinF,
        "triL": triL.astype(npbf16), "triU": triU.astype(npbf16),
        "onesb": onesb.astype(npbf16), "onesr": onesr.astype(npbf16),
        "rperm": rperm.astype(npbf16),
    }


def _core_inputs(x, wqk, wv, wo, static, b, g):
    xb = np.ascontiguousarray(x[b].T)                                # [D, S]
    xT = np.ascontiguousarray(
        xb.reshape(KT, 128, S).transpose(1, 0, 2)).astype(npbf16)

    wq_g = wqk[E * g:E * (g + 1), :]                                 # [E, D]
    wk_g = wqk[D + E * g:D + E * (g + 1), :]
    wv_g = wv[E * g:E * (g + 1), :]
    wqT = np.ascontiguousarray(
        wq_g.T.reshape(KT, 128, ET, 128).transpose(1, 0, 2, 3)).astype(npbf16)
    wkT = np.ascontiguousarray(
        wk_g.T.reshape(KT, 128, ET, 128).transpose(1, 0, 2, 3)).astype(npbf16)
    wvT = np.ascontiguousarray(
        wv_g.T.reshape(KT, 128, E).transpose(1, 0, 2)).astype(npbf16)
    woT = np.ascontiguousarray(
        wo[:, E * g:E * (g + 1)].T.reshape(ET, 128, D).transpose(1, 0, 2)
    ).astype(npbf16)

    m = dict(static)
    m.update({"xT": xT, "wqT": wqT, "wkT": wkT, "wvT": wvT, "woT": woT})
    return m


def kernel(x, wqk, wv, wo):
    x = np.asarray(x, dtype=np.float32)
    wqk = np.asarray(wqk, dtype=np.float32)
    wv = np.asarray(wv, dtype=np.float32)
    wo = np.asarray(wo, dtype=np.float32)

    nc = _get_nc()
    static = _static_inputs()
    in_maps = [
        _core_inputs(x, wqk, wv, wo, static, c // G, c % G) for c in range(8)
    ]
    res = run_bass_kernel_spmd(nc, in_maps, core_ids=list(range(8)))
    out = np.zeros((B, S, D), dtype=np.float32)
    for c in range(8):
        out[c // G] += res.results[c]["out"].astype(np.float32)
    return out


# revision 48
# speedup vs baseline: 1.2324x; 1.0659x over previous
"""Trainium2 Bass kernel for nn_Attention_43301860278871.

Full attention layer: fused QK projection + V projection, interleaved RoPE,
causal SDPA, output projection.  B=2, S=2048, D=2048, H=16, HD=128.

Sharding: 8 cores = 2 batches x 4 head-groups (tensor parallel over heads,
data parallel over batch).  Each core computes 4 heads for one batch and a
partial [S, D] output-projection contribution; the host sums the 4 partials
per batch, so no on-device collectives are needed.

v2 design (vs the f32r/DRAM-staging baseline):
  - All matmul operands in bf16 (host-converted); PSUM accumulation stays
    f32.  Q/K/V live in SBUF for the whole kernel - no DRAM round-trip and
    no phase barrier.
  - RoPE pair-swap via a bf16 permutation matmul (DVE lanes are
    partition-locked, so the swap cannot run there), then cos/sin
    multiply-add on DVE with bf16 2x modes where operands allow.
  - Causal masking: scores are computed per 128-wide j-tile with true
    (128-granular) causality; the boundary-diagonal 128x128 piece gets a
    second matmul accumulating -1e9 * max(0, j-i) into the scores PSUM
    (lower-tri x upper-tri constant operands), so exp() produces exact
    zeros and no mask multiply exists on the DVE critical path.
  - Softmax row sums: exp tiles (bf16) are accumulated over j-tiles on DVE
    (scalar_tensor_tensor, 4x mode) and reduced with ONE ones-column
    matmul per (head, i-chunk) instead of one per j-tile.
  - 1/sums broadcast via gpsimd partition_broadcast (Pool engine).
  - Output projection interleaved one i-chunk behind SDPA, one t-tile per
    head, in 4-matmul quanta between score tiles, so the Tensor engine
    stays fed while ACT streams exp().
"""
import sys
sys.path.insert(0, '/opt/trn_rl_repo')

import numpy as np
import ml_dtypes

import concourse.bass as bass
import concourse.mybir as mybir
from concourse.bass_utils import run_bass_kernel_spmd
from concourse.tile import TileContext

B, S, D, H = 2, 2048, 2048, 16
HD = D // H            # 128
G = 4                  # head-groups (cores per batch)
HPG = H // G           # heads per core = 4
E = HPG * HD           # per-core projection width = 512
ROPE_BASE = 10000.0
SCALE = float(HD) ** -0.5

f32 = mybir.dt.float32
bf16 = mybir.dt.bfloat16
npbf16 = ml_dtypes.bfloat16

KT = D // 128          # 16 contraction tiles
TT = S // 128          # 16 token tiles
TC = S // 512          # 4 token chunks
ET = E // 128          # 4 e-tiles (= heads per core)

Exp = mybir.ActivationFunctionType.Exp
mult = mybir.AluOpType.mult
add = mybir.AluOpType.add


# ---------------------------------------------------------------------------
# Workarounds for this walrus build: at most ONE sem wait per instruction.
# Tile's scheduler attaches several; hoist the excess onto NoOps injected on
# the same engine immediately before (sequencer executes waits in order, so
# semantics are identical).
# ---------------------------------------------------------------------------

def _patched_drain_and_barrier(self, tick_clock, wait_clock):
    from concourse.vector_clock import ScopedClock
    drain_inst = self.nc.sync.drain()
    wait_clock.add_sem_waits(
        drain_inst.ins, ScopedClock({None: tick_clock.global_clock})
    )
    si = drain_inst.ins.sync_info
    if si is not None and si.on_wait and len(si.on_wait) > 1:
        waits = list(si.on_wait)
        si.on_wait = waits[:1]
        for w in waits[1:]:
            extra = self.nc.sync.drain()
            esi = extra.ins.sync_info
            if esi is None:
                extra.ins.sync_info = mybir.SyncInfo(on_wait=[w], on_update=[])
            else:
                esi.on_wait = [w]

    self.nc.all_engine_barrier()
    assert self.sems is not None
    popped = self.nc._tile_sem_poison_stack.pop()
    assert popped is self._sem_poison
    self.nc.clear_and_free_semaphores(list(self.sems.allocated().values()))
    self.nc.all_engine_barrier()


def _install_tile_patch():
    import concourse.tile as tile_mod
    tile_mod.TileContext._drain_and_barrier = _patched_drain_and_barrier


def _split_waits(nc, max_waits: int = 1):
    for fn in nc.m.functions:
        for bb in fn.blocks:
            out = []
            changed = False
            for inst in list(bb.instructions):
                si = inst.sync_info
                if si is not None and si.on_wait and len(si.on_wait) > max_waits:
                    waits = list(si.on_wait)
                    for w in waits[:-max_waits]:
                        out.append(mybir.InstNoOp(
                            name=nc.get_next_instruction_name(),
                            engine=inst.engine,
                            sync_info=mybir.SyncInfo(on_wait=[w], on_update=[]),
                        ))
                    si.on_wait = waits[-max_waits:]
                    changed = True
                out.append(inst)
            if changed:
                bb.instructions = out


# ---------------------------------------------------------------------------
# Kernel build (one Bass module, SPMD across the 8 cores via input slices)
# ---------------------------------------------------------------------------

def _build_nc():
    _install_tile_patch()
    nc = bass.Bass()

    xT = nc.dram_tensor("xT", [128, KT, S], bf16, kind="ExternalInput")
    wqT = nc.dram_tensor("wqT", [128, ET, KT, 128], bf16, kind="ExternalInput")
    wkT = nc.dram_tensor("wkT", [128, ET, KT, 128], bf16, kind="ExternalInput")
    wvT = nc.dram_tensor("wvT", [128, KT, E], bf16, kind="ExternalInput")
    woT = nc.dram_tensor("woT", [128, ET, D], bf16, kind="ExternalInput")
    cosF = nc.dram_tensor("cosF", [128, S], bf16, kind="ExternalInput")
    sinF = nc.dram_tensor("sinF", [128, S], bf16, kind="ExternalInput")
    triL = nc.dram_tensor("triL", [128, 128], bf16, kind="ExternalInput")
    triU = nc.dram_tensor("triU", [128, 128], bf16, kind="ExternalInput")
    rperm = nc.dram_tensor("rperm", [128, 128], bf16, kind="ExternalInput")
    onesb = nc.dram_tensor("onesb", [128, 1], bf16, kind="ExternalInput")
    onesr = nc.dram_tensor("onesr", [1, 128], bf16, kind="ExternalInput")
    out = nc.dram_tensor("out", [S, D], bf16, kind="ExternalOutput")

    with TileContext(nc) as tc:
        with (
            nc.allow_low_precision(reason="bf16 datapath, fp32 accumulation"),
            tc.tile_pool(name="consts", bufs=1) as consts,
            tc.tile_pool(name="resid", bufs=1) as resid,
        ):
            lt_sb = consts.tile([128, 128], bf16, tag="lt")
            ut_sb = consts.tile([128, 128], bf16, tag="ut")
            ob_sb = consts.tile([128, 1], bf16, tag="ob")
            or_sb = consts.tile([1, 128], bf16, tag="or")
            rp_sb = consts.tile([128, 128], bf16, tag="rp")
            c_sb = consts.tile([128, S], bf16, tag="cos")
            s_sb = consts.tile([128, S], bf16, tag="sin")
            # consts go on the idle Pool queue so they don't block the
            # critical first-chunk loads on the SP queue
            nc.gpsimd.dma_start(rp_sb[:], rperm[:])
            nc.gpsimd.dma_start(lt_sb[:], triL[:])
            nc.gpsimd.dma_start(ut_sb[:], triU[:])
            nc.gpsimd.dma_start(ob_sb[:], onesb[:])
            nc.gpsimd.dma_start(or_sb[:], onesr[:])

            # SBUF-resident Q^T/K^T (e-major per head) and V (t-major)
            qres = resid.tile([128, ET, S], bf16, tag="qres")
            kres = resid.tile([128, ET, S], bf16, tag="kres")
            vres = resid.tile([128, TT, E], bf16, tag="vres")
            wo_sb = resid.tile([128, ET, D], bf16, tag="wo")

            with (
                tc.tile_pool(name="wpool", bufs=1) as wpool,
                tc.tile_pool(name="xpool", bufs=2) as xpool,
                tc.tile_pool(name="stage", bufs=4) as stage,
                tc.tile_pool(name="oTp", bufs=2) as oTp,
                tc.tile_pool(name="ptp", bufs=5) as ptp,
                tc.tile_pool(name="accp", bufs=2) as accp,
                tc.tile_pool(name="rcp", bufs=2) as rcp,
                tc.tile_pool(name="ostp", bufs=2) as ostp,
                # PSUM: proj-acc/rope-swap/outproj ring (3) + scores (2,
                # ACT-paced so isolated) + PV accumulators (2) + softmax
                # sums/broadcast alternating in one bank = 8 banks
                tc.tile_pool(name="psMM", bufs=3, space="PSUM") as psMM,
                tc.tile_pool(name="psC", bufs=2, space="PSUM") as psC,
                tc.tile_pool(name="psD", bufs=2, space="PSUM") as psD,
                tc.tile_pool(name="psAux", bufs=1, space="PSUM") as psAux,
            ):
                wq_sb = wpool.tile([128, ET, KT, 128], bf16, tag="wq")
                wk_sb = wpool.tile([128, ET, KT, 128], bf16, tag="wk")
                wv_sb = wpool.tile([128, KT, E], bf16, tag="wv")

                xc_t = {}
                oT_c = {}
                # deferred-emission slots: PE-blocking ops postponed until
                # the engine has other queued work covering their input
                # latency (ACT copy for the rope swap, DVE chain for the
                # softmax normalization)
                pending = {"swap": None, "tail1": None, "tail2": None}

                def flush(key):
                    fn = pending[key]
                    if fn is not None:
                        pending[key] = None
                        fn()

                def emit_proj_chunk(tcb):
                    ts = slice(tcb * 512, (tcb + 1) * 512)
                    if tcb == 0:
                        xc = xpool.tile([128, KT, 512], bf16, tag="xc")
                        xc_t[0] = xc
                        # first chunk: spread loads over all four DGE
                        # queues - x k-slices on SP, wq et-slices on ACT
                        # (group et0 needs just x + its et-slice), wk on
                        # DVE, wv/trig on Pool
                        # single queue: issue order == transfer order on
                        # the (serialized) DMA path, so strict priority:
                        # first-group inputs, then just-in-time streaming
                        for k in range(4):
                            nc.sync.dma_start(xc[:, k:k + 1], xT[:, k:k + 1, ts])
                        nc.sync.dma_start(wq_sb[:, 0], wqT[:, 0])
                        for kc in range(1, 4):
                            ks = slice(kc * 4, (kc + 1) * 4)
                            nc.sync.dma_start(xc[:, ks], xT[:, ks, ts])
                        for et in range(1, ET):
                            nc.sync.dma_start(wq_sb[:, et], wqT[:, et])
                        nc.sync.dma_start(c_sb[:], cosF[:])
                        nc.sync.dma_start(s_sb[:], sinF[:])
                        for et in range(ET):
                            nc.sync.dma_start(wk_sb[:, et], wkT[:, et])
                        for kc in range(4):
                            ks = slice(kc * 4, (kc + 1) * 4)
                            nc.sync.dma_start(wv_sb[:, ks], wvT[:, ks])
                    xc = xc_t[tcb]
                    if tcb + 1 < TC:
                        nts = slice((tcb + 1) * 512, (tcb + 2) * 512)
                        nxc = xpool.tile([128, KT, 512], bf16, tag="xc")
                        xc_t[tcb + 1] = nxc
                        for kc in range(4):
                            ks = slice(kc * 4, (kc + 1) * 4)
                            nc.sync.dma_start(nxc[:, ks], xT[:, ks, nts])
                    if tcb == 1:
                        for dcc in range(4):
                            dsl = slice(dcc * 512, (dcc + 1) * 512)
                            nc.sync.dma_start(wo_sb[:, :, dsl], woT[:, :, dsl])

                    # Q^T / K^T e-major with RoPE fused; the pair-swap
                    # matmul of each group is deferred behind the next
                    # group so the PE never waits on the ACT psum copy
                    for (w_sb, dst) in ((wq_sb, qres), (wk_sb, kres)):
                        for et in range(ET):
                            pq = psMM.tile([128, 512], f32, tag="mm")
                            for k in range(KT):
                                nc.tensor.matmul(
                                    pq[:], w_sb[:, et, k, :], xc[:, k, :],
                                    start=(k == 0), stop=(k == KT - 1),
                                )
                            flush("swap")
                            qsb = stage.tile([128, 512], bf16, tag="qsb")
                            nc.scalar.copy(qsb[:], pq[:])

                            def mkswap(qsb=qsb, et=et, dst=dst, ts=ts):
                                def f():
                                    ps2 = psMM.tile([128, 512], f32, tag="mm")
                                    nc.tensor.matmul(ps2[:], rp_sb[:], qsb[:],
                                                     start=True, stop=True)
                                    t1 = stage.tile([128, 512], bf16, tag="t1")
                                    t2 = stage.tile([128, 512], bf16, tag="t2")
                                    nc.vector.tensor_tensor(
                                        t1[:], qsb[:], c_sb[:, ts], mult)
                                    nc.vector.tensor_tensor(
                                        t2[:], ps2[:], s_sb[:, ts], mult)
                                    nc.vector.tensor_tensor(
                                        dst[:, et, ts], t1[:], t2[:], add)
                                return f
                            pending["swap"] = mkswap()
                    # V t-major
                    for tt in range(4):
                        pv = psMM.tile([128, 512], f32, tag="mm")
                        for k in range(KT):
                            nc.tensor.matmul(
                                pv[:], xc[:, k, tt * 128:(tt + 1) * 128],
                                wv_sb[:, k, :],
                                start=(k == 0), stop=(k == KT - 1),
                            )
                        if tt == 1:
                            flush("swap")
                        nc.scalar.copy(vres[:, 4 * tcb + tt, :], pv[:])

                def emit_outproj_tile(tt, oT, eager_dma=False):
                    # one t-tile of the output projection: 4 dc-quanta of 4
                    # matmuls each, returned as callables to interleave
                    tl = tt % 4
                    quanta = []
                    ost = ostp.tile([128, D], bf16, tag="ost")
                    rsl = slice(tt * 128, (tt + 1) * 128)

                    def mk(dc):
                        def q():
                            po = psMM.tile([128, 512], f32, tag="mm")
                            for eh in range(ET):
                                nc.tensor.matmul(
                                    po[:],
                                    oT[:, eh, tl * 128:(tl + 1) * 128],
                                    wo_sb[:, eh, dc * 512:(dc + 1) * 512],
                                    start=(eh == 0), stop=(eh == ET - 1),
                                )
                            dsl = slice(dc * 512, (dc + 1) * 512)
                            if dc < 3:
                                nc.vector.tensor_copy(ost[:, dsl], po[:])
                            else:
                                nc.scalar.copy(ost[:, dsl], po[:])
                            if eager_dma:
                                nc.sync.dma_start(out[rsl, dsl], ost[:, dsl])
                        return q
                    for dc in range(4):
                        quanta.append(mk(dc))

                    def fin():
                        if not eager_dma:
                            nc.sync.dma_start(out[rsl, :], ost[:])
                    return quanta, fin

                def emit_sdpa(ic):
                    isl = slice(ic * 512, (ic + 1) * 512)
                    oT_ic = oTp.tile([128, ET, 512], bf16, tag="oT")
                    oT_c[ic] = oT_ic
                    for h in range(ET):
                        # deferred output projection work (one ic behind)
                        if ic > 0:
                            quanta, fin = emit_outproj_tile(
                                4 * (ic - 1) + h, oT_c[ic - 1])
                        else:
                            quanta, fin = [], None
                        qtb = qres[:, h, isl]
                        ps_pv = psD.tile([128, 512], f32, tag="pv")
                        acc = accp.tile([128, 512], bf16, tag="acc")
                        # j-tiles: full below the diagonal block, then the
                        # 4 staircase tiles (i-extent shrinks by 128 each)
                        tiles = [(jt, 0) for jt in range(4 * ic)]
                        tiles += [(4 * ic + r, 128 * r) for r in range(4)]
                        n = len(tiles)
                        pts = [None] * n
                        LOOK = 3

                        def emit_pv(idx):
                            jt, ilo = tiles[idx]
                            nc.tensor.matmul(
                                ps_pv[:, ilo:512],
                                vres[:, jt, h * 128:(h + 1) * 128],
                                pts[idx][:, ilo:512],
                                start=(idx == 0), stop=(idx == n - 1),
                                skip_group_check=True,
                            )

                        for idx, (jt, ilo) in enumerate(tiles):
                            ps_sc = psC.tile([128, 512], f32, tag="sc")
                            nc.tensor.matmul(
                                ps_sc[:, ilo:512],
                                kres[:, h, jt * 128:(jt + 1) * 128],
                                qtb[:, ilo:512],
                                start=True, stop=True,
                            )
                            if jt >= 4 * ic:
                                # boundary-diagonal piece: accumulate
                                # -1e5*max(0, j-i) so exp gives exact zeros
                                nc.tensor.matmul(
                                    ps_sc[:, ilo:ilo + 128],
                                    lt_sb[:], ut_sb[:],
                                    start=False, stop=True,
                                    skip_group_check=True,
                                )
                            pt = ptp.tile([128, 512], bf16, tag="pt")
                            pts[idx] = pt
                            nc.scalar.activation(
                                pt[:, ilo:512], ps_sc[:, ilo:512], Exp,
                                scale=SCALE)
                            if idx == 0:
                                nc.vector.tensor_copy(acc[:], pt[:])
                            else:
                                nc.vector.scalar_tensor_tensor(
                                    acc[:, ilo:512], pt[:, ilo:512], 1.0,
                                    acc[:, ilo:512], mult, add)
                            if h == 0 and idx == 0:
                                # h0 quanta read every head slice of the
                                # previous oT - the pending tail must land
                                # before the first quantum
                                flush("tail1")
                                flush("tail2")
                            elif idx == 1:
                                flush("tail1")
                            elif idx == 3:
                                flush("tail2")
                            if idx < len(quanta):
                                quanta[idx]()
                            if idx >= LOOK:
                                emit_pv(idx - LOOK)
                        for idx in range(max(0, n - LOOK), n):
                            emit_pv(idx)
                        if fin is not None:
                            fin()

                        # normalization tail, deferred into the next head's
                        # tile loop: sums -> 1/sums, then broadcast -> mult
                        def mktails(acc=acc, ps_pv=ps_pv, oT_ic=oT_ic, h=h):
                            rc = rcp.tile([1, 512], bf16, tag="rc")

                            def t1():
                                ps_sums = psAux.tile([128, 512], f32,
                                                     tag="aux")
                                nc.tensor.matmul(ps_sums[0:1, :], ob_sb[:],
                                                 acc[:], start=True,
                                                 stop=True)
                                nc.vector.reciprocal(rc[:], ps_sums[0:1, :])

                            def t2():
                                rcb = rcp.tile([128, 512], bf16, tag="rcb")
                                pb = psAux.tile([128, 512], f32, tag="aux")
                                nc.tensor.matmul(pb[:], or_sb[:], rc[:],
                                                 start=True, stop=True)
                                nc.vector.tensor_copy(rcb[:], pb[:])
                                nc.vector.tensor_tensor(
                                    oT_ic[:, h, :], ps_pv[:], rcb[:], mult)
                            return t1, t2
                        flush("tail1")
                        flush("tail2")
                        pending["tail1"], pending["tail2"] = mktails()

                # interleave projection chunks with SDPA i-chunks so ACT's
                # exp stream never throttles the tensor engine
                for c in range(TC):
                    emit_proj_chunk(c)
                    emit_sdpa(c)

                # tail: output projection for the last i-chunk
                flush("tail1")
                flush("tail2")
                for h in range(ET):
                    quanta, fin = emit_outproj_tile(
                        4 * (TC - 1) + h, oT_c[TC - 1], eager_dma=True)
                    for q in quanta:
                        q()
                    fin()

    _split_waits(nc)
    return nc


_NC = None


def _get_nc():
    global _NC
    if _NC is None:
        _NC = _build_nc()
    return _NC


# ---------------------------------------------------------------------------
# Host-side prep + gather
# ---------------------------------------------------------------------------

def _rope_tables():
    j = np.arange(0, HD, 2, dtype=np.float32) / HD
    inv_freq = (1.0 / (ROPE_BASE ** j)).astype(np.float32)           # [64]
    t = np.arange(S, dtype=np.float32)
    freqs = np.outer(inv_freq, t)                                    # [64, S]
    cos = np.cos(freqs)
    sin = np.sin(freqs)
    cosF = np.empty((128, S), dtype=np.float32)
    sinF = np.empty((128, S), dtype=np.float32)
    cosF[0::2] = cos
    cosF[1::2] = cos
    sinF[0::2] = -sin
    sinF[1::2] = sin
    return cosF.astype(npbf16), sinF.astype(npbf16)


def _static_inputs():
    cosF, sinF = _rope_tables()
    k = np.arange(128)
    triL = (k[:, None] < k[None, :]).astype(np.float32)      # [k, j] k<j
    triU = np.where(k[:, None] >= k[None, :], -1e5, 0.0)     # [k, i] k>=i
    onesb = np.ones((128, 1), dtype=np.float32)
    onesr = np.ones((1, 128), dtype=np.float32)
    rperm = np.zeros((128, 128), dtype=np.float32)
    idx = np.arange(128)
    rperm[idx ^ 1, idx] = 1.0
    return {
        "cosF": cosF, "sinF": sinF,
        "triL": triL.astype(npbf16), "triU": triU.astype(npbf16),
        "onesb": onesb.astype(npbf16), "onesr": onesr.astype(npbf16),
        "rperm": rperm.astype(npbf16),
    }


def _core_inputs(x, wqk, wv, wo, static, b, g):
    xb = np.ascontiguousarray(x[b].T)                                # [D, S]
    xT = np.ascontiguousarray(
        xb.reshape(KT, 128, S).transpose(1, 0, 2)).astype(npbf16)

    wq_g = wqk[E * g:E * (g + 1), :]                                 # [E, D]
    wk_g = wqk[D + E * g:D + E * (g + 1), :]
    wv_g = wv[E * g:E * (g + 1), :]
    wqT = np.ascontiguousarray(
        wq_g.T.reshape(KT, 128, ET, 128).transpose(1, 2, 0, 3)).astype(npbf16)
    wkT = np.ascontiguousarray(
        wk_g.T.reshape(KT, 128, ET, 128).transpose(1, 2, 0, 3)).astype(npbf16)
    wvT = np.ascontiguousarray(
        wv_g.T.reshape(KT, 128, E).transpose(1, 0, 2)).astype(npbf16)
    woT = np.ascontiguousarray(
        wo[:, E * g:E * (g + 1)].T.reshape(ET, 128, D).transpose(1, 0, 2)
    ).astype(npbf16)

    m = dict(static)
    m.update({"xT": xT, "wqT": wqT, "wkT": wkT, "wvT": wvT, "woT": woT})
    return m


def kernel(x, wqk, wv, wo):
    x = np.asarray(x, dtype=np.float32)
    wqk = np.asarray(wqk, dtype=np.float32)
    wv = np.asarray(wv, dtype=np.float32)
    wo = np.asarray(wo, dtype=np.float32)

    nc = _get_nc()
    static = _static_inputs()
    in_maps = [
        _core_inputs(x, wqk, wv, wo, static, c // G, c % G) for c in range(8)
    ]
    res = run_bass_kernel_spmd(nc, in_maps, core_ids=list(range(8)))
    out = np.zeros((B, S, D), dtype=np.float32)
    for c in range(8):
        out[c // G] += res.results[c]["out"].astype(np.float32)
    return out


# revision 58
# speedup vs baseline: 1.2549x; 1.0183x over previous
"""Trainium2 Bass kernel for nn_Attention_43301860278871.

Full attention layer: fused QK projection + V projection, interleaved RoPE,
causal SDPA, output projection.  B=2, S=2048, D=2048, H=16, HD=128.

Sharding: 8 cores = 2 batches x 4 head-groups (tensor parallel over heads,
data parallel over batch).  Each core computes 4 heads for one batch and a
partial [S, D] output-projection contribution; the host sums the 4 partials
per batch, so no on-device collectives are needed.

v2 design (vs the f32r/DRAM-staging baseline):
  - All matmul operands in bf16 (host-converted); PSUM accumulation stays
    f32.  Q/K/V live in SBUF for the whole kernel - no DRAM round-trip and
    no phase barrier.
  - RoPE pair-swap via a bf16 permutation matmul (DVE lanes are
    partition-locked, so the swap cannot run there), then cos/sin
    multiply-add on DVE with bf16 2x modes where operands allow.
  - Causal masking: scores are computed per 128-wide j-tile with true
    (128-granular) causality; the boundary-diagonal 128x128 piece gets a
    second matmul accumulating -1e9 * max(0, j-i) into the scores PSUM
    (lower-tri x upper-tri constant operands), so exp() produces exact
    zeros and no mask multiply exists on the DVE critical path.
  - Softmax row sums: exp tiles (bf16) are accumulated over j-tiles on DVE
    (scalar_tensor_tensor, 4x mode) and reduced with ONE ones-column
    matmul per (head, i-chunk) instead of one per j-tile.
  - 1/sums broadcast via gpsimd partition_broadcast (Pool engine).
  - Output projection interleaved one i-chunk behind SDPA, one t-tile per
    head, in 4-matmul quanta between score tiles, so the Tensor engine
    stays fed while ACT streams exp().
"""
import sys
sys.path.insert(0, '/opt/trn_rl_repo')

import numpy as np
import ml_dtypes

import concourse.bass as bass
import concourse.mybir as mybir
from concourse.bass_utils import run_bass_kernel_spmd
from concourse.tile import TileContext

B, S, D, H = 2, 2048, 2048, 16
HD = D // H            # 128
G = 4                  # head-groups (cores per batch)
HPG = H // G           # heads per core = 4
E = HPG * HD           # per-core projection width = 512
ROPE_BASE = 10000.0
SCALE = float(HD) ** -0.5

f32 = mybir.dt.float32
bf16 = mybir.dt.bfloat16
npbf16 = ml_dtypes.bfloat16

KT = D // 128          # 16 contraction tiles
TT = S // 128          # 16 token tiles
TC = S // 512          # 4 token chunks
ET = E // 128          # 4 e-tiles (= heads per core)

Exp = mybir.ActivationFunctionType.Exp
mult = mybir.AluOpType.mult
add = mybir.AluOpType.add


# ---------------------------------------------------------------------------
# Workarounds for this walrus build: at most ONE sem wait per instruction.
# Tile's scheduler attaches several; hoist the excess onto NoOps injected on
# the same engine immediately before (sequencer executes waits in order, so
# semantics are identical).
# ---------------------------------------------------------------------------

def _patched_drain_and_barrier(self, tick_clock, wait_clock):
    from concourse.vector_clock import ScopedClock
    drain_inst = self.nc.sync.drain()
    wait_clock.add_sem_waits(
        drain_inst.ins, ScopedClock({None: tick_clock.global_clock})
    )
    si = drain_inst.ins.sync_info
    if si is not None and si.on_wait and len(si.on_wait) > 1:
        waits = list(si.on_wait)
        si.on_wait = waits[:1]
        for w in waits[1:]:
            extra = self.nc.sync.drain()
            esi = extra.ins.sync_info
            if esi is None:
                extra.ins.sync_info = mybir.SyncInfo(on_wait=[w], on_update=[])
            else:
                esi.on_wait = [w]

    self.nc.all_engine_barrier()
    assert self.sems is not None
    popped = self.nc._tile_sem_poison_stack.pop()
    assert popped is self._sem_poison
    self.nc.clear_and_free_semaphores(list(self.sems.allocated().values()))
    self.nc.all_engine_barrier()


def _install_tile_patch():
    import concourse.tile as tile_mod
    tile_mod.TileContext._drain_and_barrier = _patched_drain_and_barrier


def _split_waits(nc, max_waits: int = 1):
    for fn in nc.m.functions:
        for bb in fn.blocks:
            out = []
            changed = False
            for inst in list(bb.instructions):
                si = inst.sync_info
                if si is not None and si.on_wait and len(si.on_wait) > max_waits:
                    waits = list(si.on_wait)
                    for w in waits[:-max_waits]:
                        out.append(mybir.InstNoOp(
                            name=nc.get_next_instruction_name(),
                            engine=inst.engine,
                            sync_info=mybir.SyncInfo(on_wait=[w], on_update=[]),
                        ))
                    si.on_wait = waits[-max_waits:]
                    changed = True
                out.append(inst)
            if changed:
                bb.instructions = out


# ---------------------------------------------------------------------------
# Kernel build (one Bass module, SPMD across the 8 cores via input slices)
# ---------------------------------------------------------------------------

def _build_nc():
    _install_tile_patch()
    nc = bass.Bass()

    xT = nc.dram_tensor("xT", [128, KT, S], bf16, kind="ExternalInput")
    wqT = nc.dram_tensor("wqT", [128, ET, KT, 128], bf16, kind="ExternalInput")
    wkT = nc.dram_tensor("wkT", [128, ET, KT, 128], bf16, kind="ExternalInput")
    wvT = nc.dram_tensor("wvT", [128, KT, E], bf16, kind="ExternalInput")
    woT = nc.dram_tensor("woT", [128, ET, D], bf16, kind="ExternalInput")
    cosF = nc.dram_tensor("cosF", [128, S], bf16, kind="ExternalInput")
    sinF = nc.dram_tensor("sinF", [128, S], bf16, kind="ExternalInput")
    triL = nc.dram_tensor("triL", [128, 128], bf16, kind="ExternalInput")
    triU = nc.dram_tensor("triU", [128, 128], bf16, kind="ExternalInput")
    rperm = nc.dram_tensor("rperm", [128, 128], bf16, kind="ExternalInput")
    onesb = nc.dram_tensor("onesb", [128, 1], bf16, kind="ExternalInput")
    onesr = nc.dram_tensor("onesr", [1, 128], bf16, kind="ExternalInput")
    out = nc.dram_tensor("out", [S, D], bf16, kind="ExternalOutput")

    with TileContext(nc) as tc:
        with (
            nc.allow_low_precision(reason="bf16 datapath, fp32 accumulation"),
            tc.tile_pool(name="consts", bufs=1) as consts,
            tc.tile_pool(name="resid", bufs=1) as resid,
        ):
            lt_sb = consts.tile([128, 128], bf16, tag="lt")
            ut_sb = consts.tile([128, 128], bf16, tag="ut")
            ob_sb = consts.tile([128, 1], bf16, tag="ob")
            or_sb = consts.tile([1, 128], bf16, tag="or")
            rp_sb = consts.tile([128, 128], bf16, tag="rp")
            c_sb = consts.tile([128, S], bf16, tag="cos")
            s_sb = consts.tile([128, S], bf16, tag="sin")
            # consts go on the idle Pool queue so they don't block the
            # critical first-chunk loads on the SP queue
            nc.gpsimd.dma_start(rp_sb[:], rperm[:])
            nc.gpsimd.dma_start(lt_sb[:], triL[:])
            nc.gpsimd.dma_start(ut_sb[:], triU[:])
            nc.gpsimd.dma_start(ob_sb[:], onesb[:])
            nc.gpsimd.dma_start(or_sb[:], onesr[:])

            # SBUF-resident Q^T/K^T (e-major per head) and V (t-major)
            qres = resid.tile([128, ET, S], bf16, tag="qres")
            kres = resid.tile([128, ET, S], bf16, tag="kres")
            vres = resid.tile([128, TT, E], bf16, tag="vres")
            wo_sb = resid.tile([128, ET, D], bf16, tag="wo")

            with (
                tc.tile_pool(name="wpool", bufs=1) as wpool,
                tc.tile_pool(name="xpool", bufs=2) as xpool,
                tc.tile_pool(name="stage", bufs=4) as stage,
                tc.tile_pool(name="oTp", bufs=2) as oTp,
                tc.tile_pool(name="ptp", bufs=5) as ptp,
                tc.tile_pool(name="accp", bufs=2) as accp,
                tc.tile_pool(name="rcp", bufs=2) as rcp,
                tc.tile_pool(name="ostp", bufs=2) as ostp,
                # PSUM: proj-acc/rope-swap/outproj ring (3) + scores (2,
                # ACT-paced so isolated) + PV accumulators (2) + softmax
                # sums/broadcast alternating in one bank = 8 banks
                tc.tile_pool(name="psMM", bufs=3, space="PSUM") as psMM,
                tc.tile_pool(name="psC", bufs=2, space="PSUM") as psC,
                tc.tile_pool(name="psD", bufs=2, space="PSUM") as psD,
                tc.tile_pool(name="psAux", bufs=1, space="PSUM") as psAux,
            ):
                wq_sb = wpool.tile([128, ET, KT, 128], bf16, tag="wq")
                wk_sb = wpool.tile([128, ET, KT, 128], bf16, tag="wk")
                wv_sb = wpool.tile([128, KT, E], bf16, tag="wv")

                xc_t = {}
                oT_c = {}
                # deferred-emission slots: PE-blocking ops postponed until
                # the engine has other queued work covering their input
                # latency (ACT copy for the rope swap, DVE chain for the
                # softmax normalization)
                pending = {"swap": None, "tail1": None, "tail2": None}

                def flush(key):
                    fn = pending[key]
                    if fn is not None:
                        pending[key] = None
                        fn()

                def emit_chunk_loads(tcb):
                    ts = slice(tcb * 512, (tcb + 1) * 512)
                    xc = xpool.tile([128, KT, 512], bf16, tag="xc")
                    xc_t[tcb] = xc
                    if tcb == 0:
                        # single queue: issue order == transfer order on
                        # the (serialized) DMA path, so strict priority:
                        # first-group inputs, then just-in-time streaming
                        for k in range(4):
                            nc.sync.dma_start(xc[:, k:k + 1], xT[:, k:k + 1, ts])
                        nc.sync.dma_start(wq_sb[:, 0], wqT[:, 0])
                        for kc in range(1, 4):
                            ks = slice(kc * 4, (kc + 1) * 4)
                            nc.sync.dma_start(xc[:, ks], xT[:, ks, ts])
                        for et in range(1, ET):
                            nc.sync.dma_start(wq_sb[:, et], wqT[:, et])
                        nc.sync.dma_start(c_sb[:], cosF[:])
                        nc.sync.dma_start(s_sb[:], sinF[:])
                        for et in range(ET):
                            nc.sync.dma_start(wk_sb[:, et], wkT[:, et])
                        for kc in range(4):
                            ks = slice(kc * 4, (kc + 1) * 4)
                            nc.sync.dma_start(wv_sb[:, ks], wvT[:, ks])
                    else:
                        for kc in range(4):
                            ks = slice(kc * 4, (kc + 1) * 4)
                            nc.sync.dma_start(xc[:, ks], xT[:, ks, ts])
                    if tcb == 1:
                        for dcc in range(4):
                            dsl = slice(dcc * 512, (dcc + 1) * 512)
                            nc.sync.dma_start(wo_sb[:, :, dsl], woT[:, :, dsl])

                def proj_groups(tcb):
                    # 12 projection matmul groups for one x chunk, as
                    # closures so they can interleave into SDPA segments.
                    # The RoPE pair-swap matmul of each group is deferred
                    # behind the next group so the PE never waits on the
                    # ACT psum copy.
                    ts = slice(tcb * 512, (tcb + 1) * 512)
                    xc = xc_t[tcb]
                    groups = []

                    def mkqk(w_sb, dst, et):
                        def g():
                            pq = psMM.tile([128, 512], f32, tag="mm")
                            for k in range(KT):
                                nc.tensor.matmul(
                                    pq[:], w_sb[:, et, k, :], xc[:, k, :],
                                    start=(k == 0), stop=(k == KT - 1),
                                )
                            flush("swap")
                            qsb = stage.tile([128, 512], bf16, tag="qsb")
                            nc.scalar.copy(qsb[:], pq[:])

                            def f():
                                ps2 = psMM.tile([128, 512], f32, tag="mm")
                                nc.tensor.matmul(ps2[:], rp_sb[:], qsb[:],
                                                 start=True, stop=True)
                                t1 = stage.tile([128, 512], bf16, tag="t1")
                                t2 = stage.tile([128, 512], bf16, tag="t2")
                                nc.vector.tensor_tensor(
                                    t1[:], qsb[:], c_sb[:, ts], mult)
                                nc.vector.tensor_tensor(
                                    t2[:], ps2[:], s_sb[:, ts], mult)
                                nc.vector.tensor_tensor(
                                    dst[:, et, ts], t1[:], t2[:], add)
                            pending["swap"] = f
                        return g

                    def mkv(tt):
                        def g():
                            pv = psMM.tile([128, 512], f32, tag="mm")
                            for k in range(KT):
                                nc.tensor.matmul(
                                    pv[:], xc[:, k, tt * 128:(tt + 1) * 128],
                                    wv_sb[:, k, :],
                                    start=(k == 0), stop=(k == KT - 1),
                                )
                            flush("swap")
                            nc.scalar.copy(vres[:, 4 * tcb + tt, :], pv[:])
                        return g

                    for (w_sb, dst) in ((wq_sb, qres), (wk_sb, kres)):
                        for et in range(ET):
                            groups.append(mkqk(w_sb, dst, et))
                    for tt in range(4):
                        groups.append(mkv(tt))
                    return groups

                def emit_outproj_tile(tt, oT, eager_dma=False,
                                      copies_on_act=False):
                    # one t-tile of the output projection: 4 dc-quanta of 4
                    # matmuls each, returned as callables to interleave
                    tl = tt % 4
                    quanta = []
                    ost = ostp.tile([128, D], bf16, tag="ost")
                    rsl = slice(tt * 128, (tt + 1) * 128)

                    def mk(dc):
                        def q():
                            po = psMM.tile([128, 512], f32, tag="mm")
                            for eh in range(ET):
                                nc.tensor.matmul(
                                    po[:],
                                    oT[:, eh, tl * 128:(tl + 1) * 128],
                                    wo_sb[:, eh, dc * 512:(dc + 1) * 512],
                                    start=(eh == 0), stop=(eh == ET - 1),
                                )
                            dsl = slice(dc * 512, (dc + 1) * 512)
                            if dc < 3 and not copies_on_act:
                                nc.vector.tensor_copy(ost[:, dsl], po[:])
                            else:
                                nc.scalar.copy(ost[:, dsl], po[:])
                            if eager_dma:
                                nc.sync.dma_start(out[rsl, dsl], ost[:, dsl])
                        return q
                    for dc in range(4):
                        quanta.append(mk(dc))

                    def fin():
                        if not eager_dma:
                            nc.sync.dma_start(out[rsl, :], ost[:])
                    return quanta, fin

                def emit_sdpa(ic, gbh):
                    isl = slice(ic * 512, (ic + 1) * 512)
                    oT_ic = oTp.tile([128, ET, 512], bf16, tag="oT")
                    oT_c[ic] = oT_ic
                    for h in range(ET):
                        # deferred output projection work (one ic behind)
                        if ic > 0:
                            quanta, fin = emit_outproj_tile(
                                4 * (ic - 1) + h, oT_c[ic - 1],
                                copies_on_act=(ic == TC - 1 and h == ET - 1))
                        else:
                            quanta, fin = [], None
                        qtb = qres[:, h, isl]
                        ps_pv = psD.tile([128, 512], f32, tag="pv")
                        acc = accp.tile([128, 512], bf16, tag="acc")
                        # j-tiles: full below the diagonal block, then the
                        # 4 staircase tiles (i-extent shrinks by 128 each)
                        tiles = [(jt, 0) for jt in range(4 * ic)]
                        tiles += [(4 * ic + r, 128 * r) for r in range(4)]
                        n = len(tiles)
                        pts = [None] * n
                        LOOK = 3

                        def emit_pv(idx):
                            jt, ilo = tiles[idx]
                            nc.tensor.matmul(
                                ps_pv[:, ilo:512],
                                vres[:, jt, h * 128:(h + 1) * 128],
                                pts[idx][:, ilo:512],
                                start=(idx == 0), stop=(idx == n - 1),
                                skip_group_check=True,
                            )

                        for idx, (jt, ilo) in enumerate(tiles):
                            ps_sc = psC.tile([128, 512], f32, tag="sc")
                            nc.tensor.matmul(
                                ps_sc[:, ilo:512],
                                kres[:, h, jt * 128:(jt + 1) * 128],
                                qtb[:, ilo:512],
                                start=True, stop=True,
                            )
                            if jt >= 4 * ic:
                                # boundary-diagonal piece: accumulate
                                # -1e5*max(0, j-i) so exp gives exact zeros
                                nc.tensor.matmul(
                                    ps_sc[:, ilo:ilo + 128],
                                    lt_sb[:], ut_sb[:],
                                    start=False, stop=True,
                                    skip_group_check=True,
                                )
                            pt = ptp.tile([128, 512], bf16, tag="pt")
                            pts[idx] = pt
                            nc.scalar.activation(
                                pt[:, ilo:512], ps_sc[:, ilo:512], Exp,
                                scale=SCALE)
                            if idx == 0:
                                nc.vector.tensor_copy(acc[:], pt[:])
                            else:
                                nc.vector.scalar_tensor_tensor(
                                    acc[:, ilo:512], pt[:, ilo:512], 1.0,
                                    acc[:, ilo:512], mult, add)
                            if h == 0 and idx == 0:
                                # h0 quanta read every head slice of the
                                # previous oT - the pending tail must land
                                # before the first quantum
                                flush("tail1")
                                flush("tail2")
                            elif idx == 1:
                                flush("tail1")
                            elif idx == 2:
                                flush("swap")
                            elif idx == (4 if n > 4 else 3):
                                flush("tail2")
                            st = max(1, n // 4)
                            if quanta and idx % st == 0 and idx // st < 4:
                                quanta[idx // st]()
                            if idx >= LOOK:
                                emit_pv(idx - LOOK)
                        for idx in range(max(0, n - LOOK), n):
                            emit_pv(idx)
                        if fin is not None:
                            fin()

                        # normalization tail, deferred into the next head's
                        # tile loop: sums -> 1/sums, then broadcast -> mult
                        def mktails(acc=acc, ps_pv=ps_pv, oT_ic=oT_ic, h=h):
                            rc = rcp.tile([1, 512], bf16, tag="rc")

                            def t1():
                                ps_sums = psAux.tile([128, 512], f32,
                                                     tag="aux")
                                nc.tensor.matmul(ps_sums[0:1, :], ob_sb[:],
                                                 acc[:], start=True,
                                                 stop=True)
                                nc.vector.reciprocal(rc[:], ps_sums[0:1, :])

                            def t2():
                                rcb = rcp.tile([128, 512], bf16, tag="rcb")
                                pb = psAux.tile([128, 512], f32, tag="aux")
                                nc.tensor.matmul(pb[:], or_sb[:], rc[:],
                                                 start=True, stop=True)
                                nc.vector.tensor_copy(rcb[:], pb[:])
                                nc.vector.tensor_tensor(
                                    oT_ic[:, h, :], ps_pv[:], rcb[:], mult)
                            return t1, t2
                        flush("tail1")
                        flush("tail2")
                        pending["tail1"], pending["tail2"] = mktails()

                        # next chunk's projection groups: pure PE work
                        # that fills the exp-paced bubbles, with the
                        # normalization tail flushed in between
                        for gi, g in enumerate(gbh[h]):
                            g()
                            if gi == 0:
                                flush("tail1")
                            elif gi == 1:
                                flush("tail2")

                # interleave projection chunks with SDPA i-chunks so ACT's
                # exp stream never throttles the tensor engine.  chunk 3's
                # Q(h)/K(h) groups pipeline INTO sdpa(3) between heads
                # (each head only needs its own q/k slices), keeping the
                # final - otherwise exp-bound - segment fed with PE work.
                emit_chunk_loads(0)
                for g in proj_groups(0):
                    g()
                emit_chunk_loads(1)
                g1 = proj_groups(1)
                emit_sdpa(0, [g1[0:3], g1[3:6], g1[6:9], g1[9:12]])
                emit_chunk_loads(2)
                g2 = proj_groups(2)
                emit_sdpa(1, [g2[0:3], g2[3:6], g2[6:9], g2[9:12]])
                emit_chunk_loads(3)
                g3 = proj_groups(3)   # Q0-3 = g3[0:4], K0-3 = g3[4:8], V0-3 = g3[8:12]
                emit_sdpa(2, [[g3[8], g3[9]], [g3[10], g3[11]],
                              [g3[0]], [g3[4]]])
                emit_sdpa(3, [[g3[1], g3[5]], [g3[2], g3[6]],
                              [g3[3], g3[7]], []])

                # tail: output projection for the last i-chunk
                flush("tail1")
                flush("tail2")
                for h in range(ET):
                    quanta, fin = emit_outproj_tile(
                        4 * (TC - 1) + h, oT_c[TC - 1], eager_dma=True)
                    for q in quanta:
                        q()
                    fin()

    _split_waits(nc)
    return nc


_NC = None


def _get_nc():
    global _NC
    if _NC is None:
        _NC = _build_nc()
    return _NC


# ---------------------------------------------------------------------------
# Host-side prep + gather
# ---------------------------------------------------------------------------

def _rope_tables():
    j = np.arange(0, HD, 2, dtype=np.float32) / HD
    inv_freq = (1.0 / (ROPE_BASE ** j)).astype(np.float32)           # [64]
    t = np.arange(S, dtype=np.float32)
    freqs = np.outer(inv_freq, t)                                    # [64, S]
    cos = np.cos(freqs)
    sin = np.sin(freqs)
    cosF = np.empty((128, S), dtype=np.float32)
    sinF = np.empty((128, S), dtype=np.float32)
    cosF[0::2] = cos
    cosF[1::2] = cos
    sinF[0::2] = -sin
    sinF[1::2] = sin
    return cosF.astype(npbf16), sinF.astype(npbf16)


def _static_inputs():
    cosF, sinF = _rope_tables()
    k = np.arange(128)
    triL = (k[:, None] < k[None, :]).astype(np.float32)      # [k, j] k<j
    triU = np.where(k[:, None] >= k[None, :], -1e5, 0.0)     # [k, i] k>=i
    onesb = np.ones((128, 1), dtype=np.float32)
    onesr = np.ones((1, 128), dtype=np.float32)
    rperm = np.zeros((128, 128), dtype=np.float32)
    idx = np.arange(128)
    rperm[idx ^ 1, idx] = 1.0
    return {
        "cosF": cosF, "sinF": sinF,
        "triL": triL.astype(npbf16), "triU": triU.astype(npbf16),
        "onesb": onesb.astype(npbf16), "onesr": onesr.astype(npbf16),
        "rperm": rperm.astype(npbf16),
    }


def _core_inputs(x, wqk, wv, wo, static, b, g):
    xb = np.ascontiguousarray(x[b].T)                                # [D, S]
    xT = np.ascontiguousarray(
        xb.reshape(KT, 128, S).transpose(1, 0, 2)).astype(npbf16)

    wq_g = wqk[E * g:E * (g + 1), :]                                 # [E, D]
    wk_g = wqk[D + E * g:D + E * (g + 1), :]
    wv_g = wv[E * g:E * (g + 1), :]
    wqT = np.ascontiguousarray(
        wq_g.T.reshape(KT, 128, ET, 128).transpose(1, 2, 0, 3)).astype(npbf16)
    wkT = np.ascontiguousarray(
        wk_g.T.reshape(KT, 128, ET, 128).transpose(1, 2, 0, 3)).astype(npbf16)
    wvT = np.ascontiguousarray(
        wv_g.T.reshape(KT, 128, E).transpose(1, 0, 2)).astype(npbf16)
    woT = np.ascontiguousarray(
        wo[:, E * g:E * (g + 1)].T.reshape(ET, 128, D).transpose(1, 0, 2)
    ).astype(npbf16)

    m = dict(static)
    m.update({"xT": xT, "wqT": wqT, "wkT": wkT, "wvT": wvT, "woT": woT})
    return m


def kernel(x, wqk, wv, wo):
    x = np.asarray(x, dtype=np.float32)
    wqk = np.asarray(wqk, dtype=np.float32)
    wv = np.asarray(wv, dtype=np.float32)
    wo = np.asarray(wo, dtype=np.float32)

    nc = _get_nc()
    static = _static_inputs()
    in_maps = [
        _core_inputs(x, wqk, wv, wo, static, c // G, c % G) for c in range(8)
    ]
    res = run_bass_kernel_spmd(nc, in_maps, core_ids=list(range(8)))
    out = np.zeros((B, S, D), dtype=np.float32)
    for c in range(8):
        out[c // G] += res.results[c]["out"].astype(np.float32)
    return out


# revision 63
# speedup vs baseline: 1.2712x; 1.0129x over previous
"""Trainium2 Bass kernel for nn_Attention_43301860278871.

Full attention layer: fused QK projection + V projection, interleaved RoPE,
causal SDPA, output projection.  B=2, S=2048, D=2048, H=16, HD=128.

Sharding: 8 cores = 2 batches x 4 head-groups (tensor parallel over heads,
data parallel over batch).  Each core computes 4 heads for one batch and a
partial [S, D] output-projection contribution; the host sums the 4 partials
per batch, so no on-device collectives are needed.

v2 design (vs the f32r/DRAM-staging baseline):
  - All matmul operands in bf16 (host-converted); PSUM accumulation stays
    f32.  Q/K/V live in SBUF for the whole kernel - no DRAM round-trip and
    no phase barrier.
  - RoPE pair-swap via a bf16 permutation matmul (DVE lanes are
    partition-locked, so the swap cannot run there), then cos/sin
    multiply-add on DVE with bf16 2x modes where operands allow.
  - Causal masking: scores are computed per 128-wide j-tile with true
    (128-granular) causality; the boundary-diagonal 128x128 piece gets a
    second matmul accumulating -1e9 * max(0, j-i) into the scores PSUM
    (lower-tri x upper-tri constant operands), so exp() produces exact
    zeros and no mask multiply exists on the DVE critical path.
  - Softmax row sums: exp tiles (bf16) are accumulated over j-tiles on DVE
    (scalar_tensor_tensor, 4x mode) and reduced with ONE ones-column
    matmul per (head, i-chunk) instead of one per j-tile.
  - 1/sums broadcast via gpsimd partition_broadcast (Pool engine).
  - Output projection interleaved one i-chunk behind SDPA, one t-tile per
    head, in 4-matmul quanta between score tiles, so the Tensor engine
    stays fed while ACT streams exp().
"""
import sys
sys.path.insert(0, '/opt/trn_rl_repo')

import numpy as np
import ml_dtypes

import concourse.bass as bass
import concourse.mybir as mybir
from concourse.bass_utils import run_bass_kernel_spmd
from concourse.tile import TileContext

B, S, D, H = 2, 2048, 2048, 16
HD = D // H            # 128
G = 4                  # head-groups (cores per batch)
HPG = H // G           # heads per core = 4
E = HPG * HD           # per-core projection width = 512
ROPE_BASE = 10000.0
SCALE = float(HD) ** -0.5

f32 = mybir.dt.float32
bf16 = mybir.dt.bfloat16
npbf16 = ml_dtypes.bfloat16

KT = D // 128          # 16 contraction tiles
TT = S // 128          # 16 token tiles
TC = S // 512          # 4 token chunks
ET = E // 128          # 4 e-tiles (= heads per core)

Exp = mybir.ActivationFunctionType.Exp
mult = mybir.AluOpType.mult
add = mybir.AluOpType.add


# ---------------------------------------------------------------------------
# Workarounds for this walrus build: at most ONE sem wait per instruction.
# Tile's scheduler attaches several; hoist the excess onto NoOps injected on
# the same engine immediately before (sequencer executes waits in order, so
# semantics are identical).
# ---------------------------------------------------------------------------

def _patched_drain_and_barrier(self, tick_clock, wait_clock):
    from concourse.vector_clock import ScopedClock
    drain_inst = self.nc.sync.drain()
    wait_clock.add_sem_waits(
        drain_inst.ins, ScopedClock({None: tick_clock.global_clock})
    )
    si = drain_inst.ins.sync_info
    if si is not None and si.on_wait and len(si.on_wait) > 1:
        waits = list(si.on_wait)
        si.on_wait = waits[:1]
        for w in waits[1:]:
            extra = self.nc.sync.drain()
            esi = extra.ins.sync_info
            if esi is None:
                extra.ins.sync_info = mybir.SyncInfo(on_wait=[w], on_update=[])
            else:
                esi.on_wait = [w]

    self.nc.all_engine_barrier()
    assert self.sems is not None
    popped = self.nc._tile_sem_poison_stack.pop()
    assert popped is self._sem_poison
    self.nc.clear_and_free_semaphores(list(self.sems.allocated().values()))
    self.nc.all_engine_barrier()


def _install_tile_patch():
    import concourse.tile as tile_mod
    tile_mod.TileContext._drain_and_barrier = _patched_drain_and_barrier


def _split_waits(nc, max_waits: int = 1):
    for fn in nc.m.functions:
        for bb in fn.blocks:
            out = []
            changed = False
            for inst in list(bb.instructions):
                si = inst.sync_info
                if si is not None and si.on_wait and len(si.on_wait) > max_waits:
                    waits = list(si.on_wait)
                    for w in waits[:-max_waits]:
                        out.append(mybir.InstNoOp(
                            name=nc.get_next_instruction_name(),
                            engine=inst.engine,
                            sync_info=mybir.SyncInfo(on_wait=[w], on_update=[]),
                        ))
                    si.on_wait = waits[-max_waits:]
                    changed = True
                out.append(inst)
            if changed:
                bb.instructions = out


# ---------------------------------------------------------------------------
# Kernel build (one Bass module, SPMD across the 8 cores via input slices)
# ---------------------------------------------------------------------------

def _build_nc():
    _install_tile_patch()
    nc = bass.Bass()

    xT = nc.dram_tensor("xT", [128, KT, S], bf16, kind="ExternalInput")
    wqT = nc.dram_tensor("wqT", [128, ET, KT, 128], bf16, kind="ExternalInput")
    wkT = nc.dram_tensor("wkT", [128, ET, KT, 128], bf16, kind="ExternalInput")
    wvT = nc.dram_tensor("wvT", [128, KT, E], bf16, kind="ExternalInput")
    woT = nc.dram_tensor("woT", [128, ET, D], bf16, kind="ExternalInput")
    cosF = nc.dram_tensor("cosF", [128, S], bf16, kind="ExternalInput")
    sinF = nc.dram_tensor("sinF", [128, S], bf16, kind="ExternalInput")
    triL = nc.dram_tensor("triL", [128, 128], bf16, kind="ExternalInput")
    triU = nc.dram_tensor("triU", [128, 128], bf16, kind="ExternalInput")
    rperm = nc.dram_tensor("rperm", [128, 128], bf16, kind="ExternalInput")
    onesb = nc.dram_tensor("onesb", [128, 1], bf16, kind="ExternalInput")
    onesr = nc.dram_tensor("onesr", [1, 128], bf16, kind="ExternalInput")
    out = nc.dram_tensor("out", [S, D], bf16, kind="ExternalOutput")

    with TileContext(nc) as tc:
        with (
            nc.allow_low_precision(reason="bf16 datapath, fp32 accumulation"),
            tc.tile_pool(name="consts", bufs=1) as consts,
            tc.tile_pool(name="resid", bufs=1) as resid,
        ):
            lt_sb = consts.tile([128, 128], bf16, tag="lt")
            ut_sb = consts.tile([128, 128], bf16, tag="ut")
            ob_sb = consts.tile([128, 1], bf16, tag="ob")
            or_sb = consts.tile([1, 128], bf16, tag="or")
            rp_sb = consts.tile([128, 128], bf16, tag="rp")
            c_sb = consts.tile([128, S], bf16, tag="cos")
            s_sb = consts.tile([128, S], bf16, tag="sin")
            # consts go on the idle Pool queue so they don't block the
            # critical first-chunk loads on the SP queue
            nc.gpsimd.dma_start(rp_sb[:], rperm[:])
            nc.gpsimd.dma_start(lt_sb[:], triL[:])
            nc.gpsimd.dma_start(ut_sb[:], triU[:])
            nc.gpsimd.dma_start(ob_sb[:], onesb[:])
            nc.gpsimd.dma_start(or_sb[:], onesr[:])

            # SBUF-resident Q^T/K^T (e-major per head) and V (t-major)
            qres = resid.tile([128, ET, S], bf16, tag="qres")
            kres = resid.tile([128, ET, S], bf16, tag="kres")
            vres = resid.tile([128, TT, E], bf16, tag="vres")
            wo_sb = resid.tile([128, ET, D], bf16, tag="wo")

            with (
                tc.tile_pool(name="wpool", bufs=1) as wpool,
                tc.tile_pool(name="xpool", bufs=3) as xpool,
                tc.tile_pool(name="stage", bufs=4) as stage,
                tc.tile_pool(name="oTp", bufs=2) as oTp,
                tc.tile_pool(name="ptp", bufs=5) as ptp,
                tc.tile_pool(name="accp", bufs=2) as accp,
                tc.tile_pool(name="rcp", bufs=2) as rcp,
                tc.tile_pool(name="ostp", bufs=2) as ostp,
                # PSUM: proj-acc/rope-swap/outproj ring (3) + scores (2,
                # ACT-paced so isolated) + PV accumulators (2) + softmax
                # sums/broadcast alternating in one bank = 8 banks
                tc.tile_pool(name="psMM", bufs=3, space="PSUM") as psMM,
                tc.tile_pool(name="psC", bufs=2, space="PSUM") as psC,
                tc.tile_pool(name="psD", bufs=2, space="PSUM") as psD,
                tc.tile_pool(name="psAux", bufs=1, space="PSUM") as psAux,
            ):
                wq_sb = wpool.tile([128, ET, KT, 128], bf16, tag="wq")
                wk_sb = wpool.tile([128, ET, KT, 128], bf16, tag="wk")
                wv_sb = wpool.tile([128, KT, E], bf16, tag="wv")

                xc_t = {}
                oT_c = {}
                # deferred-emission slots: PE-blocking ops postponed until
                # the engine has other queued work covering their input
                # latency (ACT copy for the rope swap, DVE chain for the
                # softmax normalization)
                pending = {"swap": None, "tail1": None, "tail2": None}

                def flush(key):
                    fn = pending[key]
                    if fn is not None:
                        pending[key] = None
                        fn()

                def emit_chunk_loads(tcb):
                    ts = slice(tcb * 512, (tcb + 1) * 512)
                    xc = xpool.tile([128, KT, 512], bf16, tag="xc")
                    xc_t[tcb] = xc
                    if tcb == 0:
                        # single queue: issue order == transfer order on
                        # the (serialized) DMA path, so strict priority:
                        # first-group inputs, then just-in-time streaming
                        nc.sync.dma_start(wq_sb[:, 0], wqT[:, 0])
                        for k in range(4):
                            nc.sync.dma_start(xc[:, k:k + 1], xT[:, k:k + 1, ts])
                        for kc in range(1, 4):
                            ks = slice(kc * 4, (kc + 1) * 4)
                            nc.sync.dma_start(xc[:, ks], xT[:, ks, ts])
                        for et in range(1, ET):
                            nc.sync.dma_start(wq_sb[:, et], wqT[:, et])
                        nc.sync.dma_start(c_sb[:], cosF[:])
                        nc.sync.dma_start(s_sb[:], sinF[:])
                        for et in range(ET):
                            nc.sync.dma_start(wk_sb[:, et], wkT[:, et])
                        for kc in range(4):
                            ks = slice(kc * 4, (kc + 1) * 4)
                            nc.sync.dma_start(wv_sb[:, ks], wvT[:, ks])
                    else:
                        for kc in range(4):
                            ks = slice(kc * 4, (kc + 1) * 4)
                            nc.sync.dma_start(xc[:, ks], xT[:, ks, ts])
                    if tcb == 1:
                        for dcc in range(4):
                            dsl = slice(dcc * 512, (dcc + 1) * 512)
                            nc.sync.dma_start(wo_sb[:, :, dsl], woT[:, :, dsl])

                def proj_groups(tcb):
                    # 12 projection matmul groups for one x chunk, as
                    # closures so they can interleave into SDPA segments.
                    # The RoPE pair-swap matmul of each group is deferred
                    # behind the next group so the PE never waits on the
                    # ACT psum copy.
                    ts = slice(tcb * 512, (tcb + 1) * 512)
                    xc = xc_t[tcb]
                    groups = []

                    def mkqk(w_sb, dst, et):
                        def g():
                            pq = psMM.tile([128, 512], f32, tag="mm")
                            for k in range(KT):
                                nc.tensor.matmul(
                                    pq[:], w_sb[:, et, k, :], xc[:, k, :],
                                    start=(k == 0), stop=(k == KT - 1),
                                )
                            flush("swap")
                            qsb = stage.tile([128, 512], bf16, tag="qsb")
                            nc.scalar.copy(qsb[:], pq[:])

                            def f():
                                ps2 = psMM.tile([128, 512], f32, tag="mm")
                                nc.tensor.matmul(ps2[:], rp_sb[:], qsb[:],
                                                 start=True, stop=True)
                                t1 = stage.tile([128, 512], bf16, tag="t1")
                                t2 = stage.tile([128, 512], bf16, tag="t2")
                                nc.vector.tensor_tensor(
                                    t1[:], qsb[:], c_sb[:, ts], mult)
                                nc.vector.tensor_tensor(
                                    t2[:], ps2[:], s_sb[:, ts], mult)
                                nc.vector.tensor_tensor(
                                    dst[:, et, ts], t1[:], t2[:], add)
                            pending["swap"] = f
                        return g

                    def mkv(tt):
                        def g():
                            pv = psMM.tile([128, 512], f32, tag="mm")
                            for k in range(KT):
                                nc.tensor.matmul(
                                    pv[:], xc[:, k, tt * 128:(tt + 1) * 128],
                                    wv_sb[:, k, :],
                                    start=(k == 0), stop=(k == KT - 1),
                                )
                            flush("swap")
                            nc.scalar.copy(vres[:, 4 * tcb + tt, :], pv[:])
                        return g

                    for (w_sb, dst) in ((wq_sb, qres), (wk_sb, kres)):
                        for et in range(ET):
                            groups.append(mkqk(w_sb, dst, et))
                    for tt in range(4):
                        groups.append(mkv(tt))
                    return groups

                def emit_outproj_tile(tt, oT, eager_dma=False,
                                      copies_on_act=False):
                    # one t-tile of the output projection: 4 dc-quanta of 4
                    # matmuls each, returned as callables to interleave
                    tl = tt % 4
                    quanta = []
                    ost = ostp.tile([128, D], bf16, tag="ost")
                    rsl = slice(tt * 128, (tt + 1) * 128)

                    def mk(dc):
                        def q():
                            po = psMM.tile([128, 512], f32, tag="mm")
                            for eh in range(ET):
                                nc.tensor.matmul(
                                    po[:],
                                    oT[:, eh, tl * 128:(tl + 1) * 128],
                                    wo_sb[:, eh, dc * 512:(dc + 1) * 512],
                                    start=(eh == 0), stop=(eh == ET - 1),
                                )
                            dsl = slice(dc * 512, (dc + 1) * 512)
                            if dc < 3 and not copies_on_act:
                                nc.vector.tensor_copy(ost[:, dsl], po[:])
                            else:
                                nc.scalar.copy(ost[:, dsl], po[:])
                            if eager_dma:
                                nc.sync.dma_start(out[rsl, dsl], ost[:, dsl])
                        return q
                    for dc in range(4):
                        quanta.append(mk(dc))

                    def fin():
                        if not eager_dma:
                            nc.sync.dma_start(out[rsl, :], ost[:])
                    return quanta, fin

                def emit_sdpa(ic, gbh):
                    isl = slice(ic * 512, (ic + 1) * 512)
                    oT_ic = oTp.tile([128, ET, 512], bf16, tag="oT")
                    oT_c[ic] = oT_ic
                    for h in range(ET):
                        # deferred output projection work (one ic behind)
                        if ic > 0:
                            quanta, fin = emit_outproj_tile(
                                4 * (ic - 1) + h, oT_c[ic - 1],
                                copies_on_act=(ic == TC - 1 and h == ET - 1))
                        else:
                            quanta, fin = [], None
                        qtb = qres[:, h, isl]
                        ps_pv = psD.tile([128, 512], f32, tag="pv")
                        acc = accp.tile([128, 512], bf16, tag="acc")
                        # j-tiles: full below the diagonal block, then the
                        # 4 staircase tiles (i-extent shrinks by 128 each)
                        tiles = [(jt, 0) for jt in range(4 * ic)]
                        tiles += [(4 * ic + r, 128 * r) for r in range(4)]
                        n = len(tiles)
                        pts = [None] * n
                        LOOK = 3

                        def emit_pv(idx):
                            jt, ilo = tiles[idx]
                            nc.tensor.matmul(
                                ps_pv[:, ilo:512],
                                vres[:, jt, h * 128:(h + 1) * 128],
                                pts[idx][:, ilo:512],
                                start=(idx == 0), stop=(idx == n - 1),
                                skip_group_check=True,
                            )

                        for idx, (jt, ilo) in enumerate(tiles):
                            ps_sc = psC.tile([128, 512], f32, tag="sc")
                            nc.tensor.matmul(
                                ps_sc[:, ilo:512],
                                kres[:, h, jt * 128:(jt + 1) * 128],
                                qtb[:, ilo:512],
                                start=True, stop=True,
                            )
                            if jt >= 4 * ic:
                                # boundary-diagonal piece: accumulate
                                # -1e5*max(0, j-i) so exp gives exact zeros
                                nc.tensor.matmul(
                                    ps_sc[:, ilo:ilo + 128],
                                    lt_sb[:], ut_sb[:],
                                    start=False, stop=True,
                                    skip_group_check=True,
                                )
                            pt = ptp.tile([128, 512], bf16, tag="pt")
                            pts[idx] = pt
                            nc.scalar.activation(
                                pt[:, ilo:512], ps_sc[:, ilo:512], Exp,
                                scale=SCALE)
                            if idx == 0:
                                nc.vector.tensor_copy(acc[:], pt[:])
                            else:
                                nc.vector.scalar_tensor_tensor(
                                    acc[:, ilo:512], pt[:, ilo:512], 1.0,
                                    acc[:, ilo:512], mult, add)
                            if h == 0 and idx == 0:
                                # h0 quanta read every head slice of the
                                # previous oT - the pending tail must land
                                # before the first quantum
                                flush("tail1")
                                flush("tail2")
                            elif idx == 1:
                                flush("tail1")
                            elif idx == 2:
                                flush("swap")
                            elif idx == (4 if n > 4 else 3):
                                flush("tail2")
                            st = max(1, n // 4)
                            if quanta and idx % st == 0 and idx // st < 4:
                                quanta[idx // st]()
                            if idx >= LOOK:
                                emit_pv(idx - LOOK)
                        for idx in range(max(0, n - LOOK), n):
                            emit_pv(idx)
                        if fin is not None:
                            fin()

                        # normalization tail, deferred into the next head's
                        # tile loop: sums -> 1/sums, then broadcast -> mult
                        def mktails(acc=acc, ps_pv=ps_pv, oT_ic=oT_ic, h=h):
                            rc = rcp.tile([1, 512], bf16, tag="rc")

                            def t1():
                                ps_sums = psAux.tile([128, 512], f32,
                                                     tag="aux")
                                nc.tensor.matmul(ps_sums[0:1, :], ob_sb[:],
                                                 acc[:], start=True,
                                                 stop=True)
                                nc.vector.reciprocal(rc[:], ps_sums[0:1, :])

                            def t2():
                                rcb = rcp.tile([128, 512], bf16, tag="rcb")
                                pb = psAux.tile([128, 512], f32, tag="aux")
                                nc.tensor.matmul(pb[:], or_sb[:], rc[:],
                                                 start=True, stop=True)
                                nc.vector.tensor_copy(rcb[:], pb[:])
                                nc.vector.tensor_tensor(
                                    oT_ic[:, h, :], ps_pv[:], rcb[:], mult)
                            return t1, t2
                        flush("tail1")
                        flush("tail2")
                        pending["tail1"], pending["tail2"] = mktails()

                        # next chunk's projection groups: pure PE work
                        # that fills the exp-paced bubbles, with the
                        # normalization tail flushed in between
                        for gi, g in enumerate(gbh[h]):
                            g()
                            if gi == 0:
                                flush("tail1")
                            elif gi == 1:
                                flush("tail2")

                # interleave projection chunks with SDPA i-chunks so ACT's
                # exp stream never throttles the tensor engine.  chunk 3's
                # Q(h)/K(h) groups pipeline INTO sdpa(3) between heads
                # (each head only needs its own q/k slices), keeping the
                # final - otherwise exp-bound - segment fed with PE work.
                emit_chunk_loads(0)
                for g in proj_groups(0):
                    g()
                emit_chunk_loads(1)
                emit_chunk_loads(2)
                g1 = proj_groups(1)
                emit_sdpa(0, [g1[0:3], g1[3:6], g1[6:9], g1[9:12]])
                emit_chunk_loads(3)
                g2 = proj_groups(2)
                emit_sdpa(1, [g2[0:3], g2[3:6], g2[6:9], g2[9:12]])
                g3 = proj_groups(3)   # Q0-3 = g3[0:4], K0-3 = g3[4:8], V0-3 = g3[8:12]
                emit_sdpa(2, [[g3[8], g3[9]], [g3[10], g3[11]],
                              [g3[0]], [g3[4]]])
                emit_sdpa(3, [[g3[1], g3[5]], [g3[2], g3[6]],
                              [g3[3], g3[7]], []])

                # tail: output projection for the last i-chunk
                flush("tail1")
                flush("tail2")
                for h in range(ET):
                    quanta, fin = emit_outproj_tile(
                        4 * (TC - 1) + h, oT_c[TC - 1], eager_dma=True)
                    for q in quanta:
                        q()
                    fin()

    _split_waits(nc)
    return nc


_NC = None


def _get_nc():
    global _NC
    if _NC is None:
        _NC = _build_nc()
    return _NC


# ---------------------------------------------------------------------------
# Host-side prep + gather
# ---------------------------------------------------------------------------

def _rope_tables():
    j = np.arange(0, HD, 2, dtype=np.float32) / HD
    inv_freq = (1.0 / (ROPE_BASE ** j)).astype(np.float32)           # [64]
    t = np.arange(S, dtype=np.float32)
    freqs = np.outer(inv_freq, t)                                    # [64, S]
    cos = np.cos(freqs)
    sin = np.sin(freqs)
    cosF = np.empty((128, S), dtype=np.float32)
    sinF = np.empty((128, S), dtype=np.float32)
    cosF[0::2] = cos
    cosF[1::2] = cos
    sinF[0::2] = -sin
    sinF[1::2] = sin
    return cosF.astype(npbf16), sinF.astype(npbf16)


def _static_inputs():
    cosF, sinF = _rope_tables()
    k = np.arange(128)
    triL = (k[:, None] < k[None, :]).astype(np.float32)      # [k, j] k<j
    triU = np.where(k[:, None] >= k[None, :], -1e5, 0.0)     # [k, i] k>=i
    onesb = np.ones((128, 1), dtype=np.float32)
    onesr = np.ones((1, 128), dtype=np.float32)
    rperm = np.zeros((128, 128), dtype=np.float32)
    idx = np.arange(128)
    rperm[idx ^ 1, idx] = 1.0
    return {
        "cosF": cosF, "sinF": sinF,
        "triL": triL.astype(npbf16), "triU": triU.astype(npbf16),
        "onesb": onesb.astype(npbf16), "onesr": onesr.astype(npbf16),
        "rperm": rperm.astype(npbf16),
    }


def _core_inputs(x, wqk, wv, wo, static, b, g):
    xb = np.ascontiguousarray(x[b].T)                                # [D, S]
    xT = np.ascontiguousarray(
        xb.reshape(KT, 128, S).transpose(1, 0, 2)).astype(npbf16)

    wq_g = wqk[E * g:E * (g + 1), :]                                 # [E, D]
    wk_g = wqk[D + E * g:D + E * (g + 1), :]
    wv_g = wv[E * g:E * (g + 1), :]
    wqT = np.ascontiguousarray(
        wq_g.T.reshape(KT, 128, ET, 128).transpose(1, 2, 0, 3)).astype(npbf16)
    wkT = np.ascontiguousarray(
        wk_g.T.reshape(KT, 128, ET, 128).transpose(1, 2, 0, 3)).astype(npbf16)
    wvT = np.ascontiguousarray(
        wv_g.T.reshape(KT, 128, E).transpose(1, 0, 2)).astype(npbf16)
    woT = np.ascontiguousarray(
        wo[:, E * g:E * (g + 1)].T.reshape(ET, 128, D).transpose(1, 0, 2)
    ).astype(npbf16)

    m = dict(static)
    m.update({"xT": xT, "wqT": wqT, "wkT": wkT, "wvT": wvT, "woT": woT})
    return m


def kernel(x, wqk, wv, wo):
    x = np.asarray(x, dtype=np.float32)
    wqk = np.asarray(wqk, dtype=np.float32)
    wv = np.asarray(wv, dtype=np.float32)
    wo = np.asarray(wo, dtype=np.float32)

    nc = _get_nc()
    static = _static_inputs()
    in_maps = [
        _core_inputs(x, wqk, wv, wo, static, c // G, c % G) for c in range(8)
    ]
    res = run_bass_kernel_spmd(nc, in_maps, core_ids=list(range(8)))
    out = np.zeros((B, S, D), dtype=np.float32)
    for c in range(8):
        out[c // G] += res.results[c]["out"].astype(np.float32)
    return out


# revision 68
# speedup vs baseline: 1.2747x; 1.0027x over previous
"""Trainium2 Bass kernel for nn_Attention_43301860278871.

Full attention layer: fused QK projection + V projection, interleaved RoPE,
causal SDPA, output projection.  B=2, S=2048, D=2048, H=16, HD=128.

Sharding: 8 cores = 2 batches x 4 head-groups (tensor parallel over heads,
data parallel over batch).  Each core computes 4 heads for one batch and a
partial [S, D] output-projection contribution; the host sums the 4 partials
per batch, so no on-device collectives are needed.

v2 design (vs the f32r/DRAM-staging baseline):
  - All matmul operands in bf16 (host-converted); PSUM accumulation stays
    f32.  Q/K/V live in SBUF for the whole kernel - no DRAM round-trip and
    no phase barrier.
  - RoPE pair-swap via a bf16 permutation matmul (DVE lanes are
    partition-locked, so the swap cannot run there), then cos/sin
    multiply-add on DVE with bf16 2x modes where operands allow.
  - Causal masking: scores are computed per 128-wide j-tile with true
    (128-granular) causality; the boundary-diagonal 128x128 piece gets a
    second matmul accumulating -1e9 * max(0, j-i) into the scores PSUM
    (lower-tri x upper-tri constant operands), so exp() produces exact
    zeros and no mask multiply exists on the DVE critical path.
  - Softmax row sums: exp tiles (bf16) are accumulated over j-tiles on DVE
    (scalar_tensor_tensor, 4x mode) and reduced with ONE ones-column
    matmul per (head, i-chunk) instead of one per j-tile.
  - 1/sums broadcast via gpsimd partition_broadcast (Pool engine).
  - Output projection interleaved one i-chunk behind SDPA, one t-tile per
    head, in 4-matmul quanta between score tiles, so the Tensor engine
    stays fed while ACT streams exp().
"""
import sys
sys.path.insert(0, '/opt/trn_rl_repo')

import numpy as np
import ml_dtypes

import concourse.bass as bass
import concourse.mybir as mybir
from concourse.bass_utils import run_bass_kernel_spmd
from concourse.tile import TileContext

B, S, D, H = 2, 2048, 2048, 16
HD = D // H            # 128
G = 4                  # head-groups (cores per batch)
HPG = H // G           # heads per core = 4
E = HPG * HD           # per-core projection width = 512
ROPE_BASE = 10000.0
SCALE = float(HD) ** -0.5

f32 = mybir.dt.float32
bf16 = mybir.dt.bfloat16
npbf16 = ml_dtypes.bfloat16

KT = D // 128          # 16 contraction tiles
TT = S // 128          # 16 token tiles
TC = S // 512          # 4 token chunks
ET = E // 128          # 4 e-tiles (= heads per core)

Exp = mybir.ActivationFunctionType.Exp
mult = mybir.AluOpType.mult
add = mybir.AluOpType.add


# ---------------------------------------------------------------------------
# Workarounds for this walrus build: at most ONE sem wait per instruction.
# Tile's scheduler attaches several; hoist the excess onto NoOps injected on
# the same engine immediately before (sequencer executes waits in order, so
# semantics are identical).
# ---------------------------------------------------------------------------

def _patched_drain_and_barrier(self, tick_clock, wait_clock):
    from concourse.vector_clock import ScopedClock
    drain_inst = self.nc.sync.drain()
    wait_clock.add_sem_waits(
        drain_inst.ins, ScopedClock({None: tick_clock.global_clock})
    )
    si = drain_inst.ins.sync_info
    if si is not None and si.on_wait and len(si.on_wait) > 1:
        waits = list(si.on_wait)
        si.on_wait = waits[:1]
        for w in waits[1:]:
            extra = self.nc.sync.drain()
            esi = extra.ins.sync_info
            if esi is None:
                extra.ins.sync_info = mybir.SyncInfo(on_wait=[w], on_update=[])
            else:
                esi.on_wait = [w]

    self.nc.all_engine_barrier()
    assert self.sems is not None
    popped = self.nc._tile_sem_poison_stack.pop()
    assert popped is self._sem_poison
    self.nc.clear_and_free_semaphores(list(self.sems.allocated().values()))
    self.nc.all_engine_barrier()


def _install_tile_patch():
    import concourse.tile as tile_mod
    tile_mod.TileContext._drain_and_barrier = _patched_drain_and_barrier


def _split_waits(nc, max_waits: int = 1):
    for fn in nc.m.functions:
        for bb in fn.blocks:
            out = []
            changed = False
            for inst in list(bb.instructions):
                si = inst.sync_info
                if si is not None and si.on_wait and len(si.on_wait) > max_waits:
                    waits = list(si.on_wait)
                    for w in waits[:-max_waits]:
                        out.append(mybir.InstNoOp(
                            name=nc.get_next_instruction_name(),
                            engine=inst.engine,
                            sync_info=mybir.SyncInfo(on_wait=[w], on_update=[]),
                        ))
                    si.on_wait = waits[-max_waits:]
                    changed = True
                out.append(inst)
            if changed:
                bb.instructions = out


# ---------------------------------------------------------------------------
# Kernel build (one Bass module, SPMD across the 8 cores via input slices)
# ---------------------------------------------------------------------------

def _build_nc():
    _install_tile_patch()
    nc = bass.Bass()

    xT = nc.dram_tensor("xT", [128, KT, S], bf16, kind="ExternalInput")
    wqT = nc.dram_tensor("wqT", [128, ET, KT, 128], bf16, kind="ExternalInput")
    wkT = nc.dram_tensor("wkT", [128, ET, KT, 128], bf16, kind="ExternalInput")
    wvT = nc.dram_tensor("wvT", [128, KT, E], bf16, kind="ExternalInput")
    woT = nc.dram_tensor("woT", [128, ET, D], bf16, kind="ExternalInput")
    cosF = nc.dram_tensor("cosF", [128, S], bf16, kind="ExternalInput")
    sinF = nc.dram_tensor("sinF", [128, S], bf16, kind="ExternalInput")
    triL = nc.dram_tensor("triL", [128, 128], bf16, kind="ExternalInput")
    triU = nc.dram_tensor("triU", [128, 128], bf16, kind="ExternalInput")
    rperm = nc.dram_tensor("rperm", [128, 128], bf16, kind="ExternalInput")
    onesb = nc.dram_tensor("onesb", [128, 1], bf16, kind="ExternalInput")
    onesr = nc.dram_tensor("onesr", [1, 128], bf16, kind="ExternalInput")
    out = nc.dram_tensor("out", [S, D], bf16, kind="ExternalOutput")

    with TileContext(nc) as tc:
        with (
            nc.allow_low_precision(reason="bf16 datapath, fp32 accumulation"),
            tc.tile_pool(name="consts", bufs=1) as consts,
            tc.tile_pool(name="resid", bufs=1) as resid,
        ):
            lt_sb = consts.tile([128, 128], bf16, tag="lt")
            ut_sb = consts.tile([128, 128], bf16, tag="ut")
            ob_sb = consts.tile([128, 1], bf16, tag="ob")
            or_sb = consts.tile([1, 128], bf16, tag="or")
            rp_sb = consts.tile([128, 128], bf16, tag="rp")
            c_sb = consts.tile([128, S], bf16, tag="cos")
            s_sb = consts.tile([128, S], bf16, tag="sin")
            # consts go on the idle Pool queue so they don't block the
            # critical first-chunk loads on the SP queue
            nc.gpsimd.dma_start(rp_sb[:], rperm[:])
            nc.gpsimd.dma_start(lt_sb[:], triL[:])
            nc.gpsimd.dma_start(ut_sb[:], triU[:])
            nc.gpsimd.dma_start(ob_sb[:], onesb[:])
            nc.gpsimd.dma_start(or_sb[:], onesr[:])

            # SBUF-resident Q^T/K^T (e-major per head) and V (t-major)
            qres = resid.tile([128, ET, S], bf16, tag="qres")
            kres = resid.tile([128, ET, S], bf16, tag="kres")
            vres = resid.tile([128, TT, E], bf16, tag="vres")
            wo_sb = resid.tile([128, ET, D], bf16, tag="wo")

            with (
                tc.tile_pool(name="wpool", bufs=1) as wpool,
                tc.tile_pool(name="xpool", bufs=3) as xpool,
                tc.tile_pool(name="stage", bufs=4) as stage,
                tc.tile_pool(name="oTp", bufs=2) as oTp,
                tc.tile_pool(name="ptp", bufs=4) as ptp,
                tc.tile_pool(name="accp", bufs=2) as accp,
                tc.tile_pool(name="rcp", bufs=2) as rcp,
                tc.tile_pool(name="ostp", bufs=2) as ostp,
                # PSUM: proj-acc/rope-swap/outproj ring (3) + scores (2,
                # ACT-paced so isolated) + PV accumulators (2) + softmax
                # sums/broadcast alternating in one bank = 8 banks
                tc.tile_pool(name="psMM", bufs=3, space="PSUM") as psMM,
                tc.tile_pool(name="psC", bufs=2, space="PSUM") as psC,
                tc.tile_pool(name="psD", bufs=2, space="PSUM") as psD,
                tc.tile_pool(name="psAux", bufs=1, space="PSUM") as psAux,
            ):
                wq_sb = wpool.tile([128, ET, KT, 128], bf16, tag="wq")
                wk_sb = wpool.tile([128, ET, KT, 128], bf16, tag="wk")
                wv_sb = wpool.tile([128, KT, E], bf16, tag="wv")

                xc_t = {}
                oT_c = {}
                # deferred-emission slots: PE-blocking ops postponed until
                # the engine has other queued work covering their input
                # latency (ACT copy for the rope swap, DVE chain for the
                # softmax normalization)
                pending = {"swap": None, "tail1": None, "tail2": None}

                def flush(key):
                    fn = pending[key]
                    if fn is not None:
                        pending[key] = None
                        fn()

                def emit_chunk_loads(tcb):
                    ts = slice(tcb * 512, (tcb + 1) * 512)
                    xc = xpool.tile([128, KT, 512], bf16, tag="xc")
                    xc_t[tcb] = xc
                    if tcb == 0:
                        # single queue: issue order == transfer order on
                        # the (serialized) DMA path, so strict priority:
                        # first-group inputs, then just-in-time streaming
                        nc.sync.dma_start(wq_sb[:, 0], wqT[:, 0])
                        for k in range(4):
                            nc.sync.dma_start(xc[:, k:k + 1], xT[:, k:k + 1, ts])
                        nc.sync.dma_start(xc[:, 4:8], xT[:, 4:8, ts])
                        nc.sync.dma_start(wq_sb[:, 1], wqT[:, 1])
                        nc.sync.dma_start(xc[:, 8:12], xT[:, 8:12, ts])
                        nc.sync.dma_start(wq_sb[:, 2], wqT[:, 2])
                        nc.sync.dma_start(xc[:, 12:16], xT[:, 12:16, ts])
                        nc.sync.dma_start(wq_sb[:, 3], wqT[:, 3])
                        nc.sync.dma_start(c_sb[:], cosF[:])
                        nc.sync.dma_start(s_sb[:], sinF[:])
                        for et in range(ET):
                            nc.sync.dma_start(wk_sb[:, et], wkT[:, et])
                        for kc in range(4):
                            ks = slice(kc * 4, (kc + 1) * 4)
                            nc.sync.dma_start(wv_sb[:, ks], wvT[:, ks])
                    else:
                        for kc in range(4):
                            ks = slice(kc * 4, (kc + 1) * 4)
                            nc.sync.dma_start(xc[:, ks], xT[:, ks, ts])
                    if tcb == 1:
                        for dcc in range(4):
                            dsl = slice(dcc * 512, (dcc + 1) * 512)
                            nc.sync.dma_start(wo_sb[:, :, dsl], woT[:, :, dsl])

                def proj_groups(tcb):
                    # 12 projection matmul groups for one x chunk, as
                    # closures so they can interleave into SDPA segments.
                    # The RoPE pair-swap matmul of each group is deferred
                    # behind the next group so the PE never waits on the
                    # ACT psum copy.
                    ts = slice(tcb * 512, (tcb + 1) * 512)
                    xc = xc_t[tcb]
                    groups = []

                    def mkqk(w_sb, dst, et):
                        def g():
                            pq = psMM.tile([128, 512], f32, tag="mm")
                            for k in range(KT):
                                nc.tensor.matmul(
                                    pq[:], w_sb[:, et, k, :], xc[:, k, :],
                                    start=(k == 0), stop=(k == KT - 1),
                                )
                            flush("swap")
                            qsb = stage.tile([128, 512], bf16, tag="qsb")
                            nc.scalar.copy(qsb[:], pq[:])

                            def f():
                                ps2 = psMM.tile([128, 512], f32, tag="mm")
                                nc.tensor.matmul(ps2[:], rp_sb[:], qsb[:],
                                                 start=True, stop=True)
                                t1 = stage.tile([128, 512], bf16, tag="t1")
                                t2 = stage.tile([128, 512], bf16, tag="t2")
                                nc.vector.tensor_tensor(
                                    t1[:], qsb[:], c_sb[:, ts], mult)
                                nc.vector.tensor_tensor(
                                    t2[:], ps2[:], s_sb[:, ts], mult)
                                nc.vector.tensor_tensor(
                                    dst[:, et, ts], t1[:], t2[:], add)
                            pending["swap"] = f
                        return g

                    def mkv(tt):
                        def g():
                            pv = psMM.tile([128, 512], f32, tag="mm")
                            for k in range(KT):
                                nc.tensor.matmul(
                                    pv[:], xc[:, k, tt * 128:(tt + 1) * 128],
                                    wv_sb[:, k, :],
                                    start=(k == 0), stop=(k == KT - 1),
                                )
                            flush("swap")
                            nc.scalar.copy(vres[:, 4 * tcb + tt, :], pv[:])
                        return g

                    for (w_sb, dst) in ((wq_sb, qres), (wk_sb, kres)):
                        for et in range(ET):
                            groups.append(mkqk(w_sb, dst, et))
                    for tt in range(4):
                        groups.append(mkv(tt))
                    return groups

                def emit_outproj_tile(tt, oT, eager_dma=False,
                                      copies_on_act=False):
                    # one t-tile of the output projection: 4 dc-quanta of 4
                    # matmuls each, returned as callables to interleave
                    tl = tt % 4
                    quanta = []
                    ost = ostp.tile([128, D], bf16, tag="ost")
                    rsl = slice(tt * 128, (tt + 1) * 128)

                    def mk(dc):
                        def q():
                            po = psMM.tile([128, 512], f32, tag="mm")
                            for eh in range(ET):
                                nc.tensor.matmul(
                                    po[:],
                                    oT[:, eh, tl * 128:(tl + 1) * 128],
                                    wo_sb[:, eh, dc * 512:(dc + 1) * 512],
                                    start=(eh == 0), stop=(eh == ET - 1),
                                )
                            dsl = slice(dc * 512, (dc + 1) * 512)
                            if dc < 3 and not copies_on_act:
                                nc.vector.tensor_copy(ost[:, dsl], po[:])
                            else:
                                nc.scalar.copy(ost[:, dsl], po[:])
                            if eager_dma:
                                nc.sync.dma_start(out[rsl, dsl], ost[:, dsl])
                        return q
                    for dc in range(4):
                        quanta.append(mk(dc))

                    def fin():
                        if not eager_dma:
                            nc.sync.dma_start(out[rsl, :], ost[:])
                    return quanta, fin

                def emit_sdpa(ic, gbh):
                    isl = slice(ic * 512, (ic + 1) * 512)
                    oT_ic = oTp.tile([128, ET, 512], bf16, tag="oT")
                    oT_c[ic] = oT_ic
                    for h in range(ET):
                        # deferred output projection work (one ic behind)
                        if ic > 0:
                            quanta, fin = emit_outproj_tile(
                                4 * (ic - 1) + h, oT_c[ic - 1],
                                copies_on_act=(ic == TC - 1 and h == ET - 1))
                        else:
                            quanta, fin = [], None
                        qtb = qres[:, h, isl]
                        ps_pv = psD.tile([128, 512], f32, tag="pv")
                        acc = accp.tile([128, 512], bf16, tag="acc")
                        # j-tiles: full below the diagonal block, then the
                        # 4 staircase tiles (i-extent shrinks by 128 each)
                        tiles = [(jt, 0) for jt in range(4 * ic)]
                        tiles += [(4 * ic + r, 128 * r) for r in range(4)]
                        n = len(tiles)
                        pts = [None] * n
                        LOOK = 3

                        def emit_pv(idx):
                            jt, ilo = tiles[idx]
                            nc.tensor.matmul(
                                ps_pv[:, ilo:512],
                                vres[:, jt, h * 128:(h + 1) * 128],
                                pts[idx][:, ilo:512],
                                start=(idx == 0), stop=(idx == n - 1),
                                skip_group_check=True,
                            )

                        for idx, (jt, ilo) in enumerate(tiles):
                            ps_sc = psC.tile([128, 512], f32, tag="sc")
                            nc.tensor.matmul(
                                ps_sc[:, ilo:512],
                                kres[:, h, jt * 128:(jt + 1) * 128],
                                qtb[:, ilo:512],
                                start=True, stop=True,
                            )
                            if jt >= 4 * ic:
                                # boundary-diagonal piece: accumulate
                                # -1e5*max(0, j-i) so exp gives exact zeros
                                nc.tensor.matmul(
                                    ps_sc[:, ilo:ilo + 128],
                                    lt_sb[:], ut_sb[:],
                                    start=False, stop=True,
                                    skip_group_check=True,
                                )
                            pt = ptp.tile([128, 512], bf16, tag="pt")
                            pts[idx] = pt
                            nc.scalar.activation(
                                pt[:, ilo:512], ps_sc[:, ilo:512], Exp,
                                scale=SCALE)
                            if idx == 0:
                                nc.vector.tensor_copy(acc[:], pt[:])
                            else:
                                nc.vector.scalar_tensor_tensor(
                                    acc[:, ilo:512], pt[:, ilo:512], 1.0,
                                    acc[:, ilo:512], mult, add)
                            if h == 0 and idx == 0:
                                # h0 quanta read every head slice of the
                                # previous oT - the pending tail must land
                                # before the first quantum
                                flush("tail1")
                                flush("tail2")
                            elif idx == 1:
                                flush("tail1")
                            elif idx == 2:
                                flush("swap")
                            elif idx == (4 if n > 4 else 3):
                                flush("tail2")
                            st = max(1, n // 4)
                            if quanta and idx % st == 0 and idx // st < 4:
                                quanta[idx // st]()
                            if idx >= LOOK:
                                emit_pv(idx - LOOK)
                        for idx in range(max(0, n - LOOK), n):
                            emit_pv(idx)
                        if fin is not None:
                            fin()

                        # normalization tail, deferred into the next head's
                        # tile loop: sums -> 1/sums, then broadcast -> mult
                        last = (ic == TC - 1 and h == ET - 1)

                        def mktails(acc=acc, ps_pv=ps_pv, oT_ic=oT_ic, h=h,
                                    split=last):
                            rc = rcp.tile([1, 512], bf16, tag="rc")

                            def t1():
                                ps_sums = psAux.tile([128, 512], f32,
                                                     tag="aux")
                                nc.tensor.matmul(ps_sums[0:1, :], ob_sb[:],
                                                 acc[:], start=True,
                                                 stop=True)
                                nc.vector.reciprocal(rc[:], ps_sums[0:1, :])

                            def t2():
                                rcb = rcp.tile([128, 512], bf16, tag="rcb")
                                pb = psAux.tile([128, 512], f32, tag="aux")
                                nc.tensor.matmul(pb[:], or_sb[:], rc[:],
                                                 start=True, stop=True)
                                if split:
                                    # 128-col pieces so the dependent
                                    # output projection can start after
                                    # the first piece lands
                                    for tl in range(4):
                                        tls = slice(tl * 128, (tl + 1) * 128)
                                        nc.vector.tensor_copy(
                                            rcb[:, tls], pb[:, tls])
                                        nc.vector.tensor_tensor(
                                            oT_ic[:, h, tls], ps_pv[:, tls],
                                            rcb[:, tls], mult)
                                else:
                                    nc.vector.tensor_copy(rcb[:], pb[:])
                                    nc.vector.tensor_tensor(
                                        oT_ic[:, h, :], ps_pv[:], rcb[:],
                                        mult)
                            return t1, t2
                        flush("tail1")
                        flush("tail2")
                        pending["tail1"], pending["tail2"] = mktails()

                        # next chunk's projection groups: pure PE work
                        # that fills the exp-paced bubbles, with the
                        # normalization tail flushed in between
                        for gi, g in enumerate(gbh[h]):
                            g()
                            if gi == 0:
                                flush("tail1")
                            elif gi == 1:
                                flush("tail2")

                # interleave projection chunks with SDPA i-chunks so ACT's
                # exp stream never throttles the tensor engine.  chunk 3's
                # Q(h)/K(h) groups pipeline INTO sdpa(3) between heads
                # (each head only needs its own q/k slices), keeping the
                # final - otherwise exp-bound - segment fed with PE work.
                emit_chunk_loads(0)
                for g in proj_groups(0):
                    g()
                emit_chunk_loads(1)
                emit_chunk_loads(2)
                g1 = proj_groups(1)
                emit_sdpa(0, [g1[0:3], g1[3:6], g1[6:9], g1[9:12]])
                emit_chunk_loads(3)
                g2 = proj_groups(2)
                emit_sdpa(1, [g2[0:3], g2[3:6], g2[6:9], g2[9:12]])
                g3 = proj_groups(3)   # Q0-3 = g3[0:4], K0-3 = g3[4:8], V0-3 = g3[8:12]
                emit_sdpa(2, [[g3[8], g3[9]], [g3[10], g3[11]],
                              [g3[0]], [g3[4]]])
                emit_sdpa(3, [[g3[1], g3[5]], [g3[2], g3[6]],
                              [g3[3], g3[7]], []])

                # tail: output projection for the last i-chunk
                flush("tail1")
                flush("tail2")
                for h in range(ET):
                    quanta, fin = emit_outproj_tile(
                        4 * (TC - 1) + h, oT_c[TC - 1], eager_dma=True)
                    for q in quanta:
                        q()
                    fin()

    _split_waits(nc)
    return nc


_NC = None


def _get_nc():
    global _NC
    if _NC is None:
        _NC = _build_nc()
    return _NC


# ---------------------------------------------------------------------------
# Host-side prep + gather
# ---------------------------------------------------------------------------

def _rope_tables():
    j = np.arange(0, HD, 2, dtype=np.float32) / HD
    inv_freq = (1.0 / (ROPE_BASE ** j)).astype(np.float32)           # [64]
    t = np.arange(S, dtype=np.float32)
    freqs = np.outer(inv_freq, t)                                    # [64, S]
    cos = np.cos(freqs)
    sin = np.sin(freqs)
    cosF = np.empty((128, S), dtype=np.float32)
    sinF = np.empty((128, S), dtype=np.float32)
    cosF[0::2] = cos
    cosF[1::2] = cos
    sinF[0::2] = -sin
    sinF[1::2] = sin
    return cosF.astype(npbf16), sinF.astype(npbf16)


def _static_inputs():
    cosF, sinF = _rope_tables()
    k = np.arange(128)
    triL = (k[:, None] < k[None, :]).astype(np.float32)      # [k, j] k<j
    triU = np.where(k[:, None] >= k[None, :], -1e5, 0.0)     # [k, i] k>=i
    onesb = np.ones((128, 1), dtype=np.float32)
    onesr = np.ones((1, 128), dtype=np.float32)
    rperm = np.zeros((128, 128), dtype=np.float32)
    idx = np.arange(128)
    rperm[idx ^ 1, idx] = 1.0
    return {
        "cosF": cosF, "sinF": sinF,
        "triL": triL.astype(npbf16), "triU": triU.astype(npbf16),
        "onesb": onesb.astype(npbf16), "onesr": onesr.astype(npbf16),
        "rperm": rperm.astype(npbf16),
    }


def _core_inputs(x, wqk, wv, wo, static, b, g):
    xb = np.ascontiguousarray(x[b].T)                                # [D, S]
    xT = np.ascontiguousarray(
        xb.reshape(KT, 128, S).transpose(1, 0, 2)).astype(npbf16)

    wq_g = wqk[E * g:E * (g + 1), :]                                 # [E, D]
    wk_g = wqk[D + E * g:D + E * (g + 1), :]
    wv_g = wv[E * g:E * (g + 1), :]
    wqT = np.ascontiguousarray(
        wq_g.T.reshape(KT, 128, ET, 128).transpose(1, 2, 0, 3)).astype(npbf16)
    wkT = np.ascontiguousarray(
        wk_g.T.reshape(KT, 128, ET, 128).transpose(1, 2, 0, 3)).astype(npbf16)
    wvT = np.ascontiguousarray(
        wv_g.T.reshape(KT, 128, E).transpose(1, 0, 2)).astype(npbf16)
    woT = np.ascontiguousarray(
        wo[:, E * g:E * (g + 1)].T.reshape(ET, 128, D).transpose(1, 0, 2)
    ).astype(npbf16)

    m = dict(static)
    m.update({"xT": xT, "wqT": wqT, "wkT": wkT, "wvT": wvT, "woT": woT})
    return m


def kernel(x, wqk, wv, wo):
    x = np.asarray(x, dtype=np.float32)
    wqk = np.asarray(wqk, dtype=np.float32)
    wv = np.asarray(wv, dtype=np.float32)
    wo = np.asarray(wo, dtype=np.float32)

    nc = _get_nc()
    static = _static_inputs()
    in_maps = [
        _core_inputs(x, wqk, wv, wo, static, c // G, c % G) for c in range(8)
    ]
    res = run_bass_kernel_spmd(nc, in_maps, core_ids=list(range(8)))
    out = np.zeros((B, S, D), dtype=np.float32)
    for c in range(8):
        out[c // G] += res.results[c]["out"].astype(np.float32)
    return out


# revision 77
# speedup vs baseline: 1.2757x; 1.0008x over previous
"""Trainium2 Bass kernel for nn_Attention_43301860278871.

Full attention layer: fused QK projection + V projection, interleaved RoPE,
causal SDPA, output projection.  B=2, S=2048, D=2048, H=16, HD=128.

Sharding: 8 cores = 2 batches x 4 head-groups (tensor parallel over heads,
data parallel over batch).  Each core computes 4 heads for one batch and a
partial [S, D] output-projection contribution; the host sums the 4 partials
per batch, so no on-device collectives are needed.

Design (vs the f32r/DRAM-staging baseline at 402us):
  - All matmul operands in bf16 (host-converted; PSUM accumulation stays
    f32), halving DMA and letting Q/K/V stay SBUF-resident for the whole
    kernel - no DRAM round-trip, no phase barrier.  rel err ~8e-3 vs the
    2e-2 gate.
  - RoPE pair-swap via a bf16 permutation matmul (DVE lanes are
    partition-locked, so the swap cannot run there), deferred behind the
    next projection group so the PE never waits on the ACT psum copy.
  - Causal masking at true 128-granularity: per-j-tile scores matmuls
    with trimmed i-extents; the boundary-diagonal 128x128 piece gets a
    second matmul accumulating -1e5 * max(0, j-i) into the scores PSUM
    (lower-tri x upper-tri constant operands), so exp() produces exact
    zeros and no mask multiply exists anywhere on the critical path.
  - Softmax row sums: bf16 exp tiles accumulated over j-tiles on DVE
    (scalar_tensor_tensor, 4x mode) and reduced with ONE ones-column
    matmul per (head, i-chunk) instead of one per j-tile (the baseline
    spent 34us of PE streaming ones-matmuls).  1/sums broadcast via a
    ones-row matmul; normalization tails split in two stages and emitted
    inside the NEXT head's tile loop so their cross-engine latency hides
    behind queued PE work.
  - Global software pipeline: projection chunks interleave with SDPA
    i-chunks (12 proj groups spread over the preceding segment's heads;
    chunk 3's Q(h)/K(h) groups pipeline INTO sdpa(3) between heads), and
    the output projection runs one i-chunk behind SDPA in 4-matmul quanta
    spread across each head's score tiles - so the exp() stream on ACT
    (~87us) never throttles the tensor engine.
  - PSUM: 8 banks as proj/swap/outproj ring (3) + scores (2, the only
    ACT-paced pool, isolated) + PV accumulators (2) + sums/broadcast (1).
  - Single-queue DMA with strict priority order (the DMA path serializes
    at ~350GB/s, so issue order is everything): wq-et0 first, then x
    k-slices interleaved with remaining weight et-slices; chunk-0 Q
    matmuls emitted in DMA-arrival wavefront order.

TimelineSim: ~315.5us/core (PE 93% busy, 295us of a 290us matmul-stream
floor at 1 row/cycle); rel err 7.8e-3.
"""
import sys
sys.path.insert(0, '/opt/trn_rl_repo')

import numpy as np
import ml_dtypes

import concourse.bass as bass
import concourse.mybir as mybir
from concourse.bass_utils import run_bass_kernel_spmd
from concourse.tile import TileContext

B, S, D, H = 2, 2048, 2048, 16
HD = D // H            # 128
G = 4                  # head-groups (cores per batch)
HPG = H // G           # heads per core = 4
E = HPG * HD           # per-core projection width = 512
ROPE_BASE = 10000.0
SCALE = float(HD) ** -0.5

f32 = mybir.dt.float32
bf16 = mybir.dt.bfloat16
npbf16 = ml_dtypes.bfloat16

KT = D // 128          # 16 contraction tiles
TT = S // 128          # 16 token tiles
TC = S // 512          # 4 token chunks
ET = E // 128          # 4 e-tiles (= heads per core)

Exp = mybir.ActivationFunctionType.Exp
mult = mybir.AluOpType.mult
add = mybir.AluOpType.add


# ---------------------------------------------------------------------------
# Workarounds for this walrus build: at most ONE sem wait per instruction.
# Tile's scheduler attaches several; hoist the excess onto NoOps injected on
# the same engine immediately before (sequencer executes waits in order, so
# semantics are identical).
# ---------------------------------------------------------------------------

def _patched_drain_and_barrier(self, tick_clock, wait_clock):
    from concourse.vector_clock import ScopedClock
    drain_inst = self.nc.sync.drain()
    wait_clock.add_sem_waits(
        drain_inst.ins, ScopedClock({None: tick_clock.global_clock})
    )
    si = drain_inst.ins.sync_info
    if si is not None and si.on_wait and len(si.on_wait) > 1:
        waits = list(si.on_wait)
        si.on_wait = waits[:1]
        for w in waits[1:]:
            extra = self.nc.sync.drain()
            esi = extra.ins.sync_info
            if esi is None:
                extra.ins.sync_info = mybir.SyncInfo(on_wait=[w], on_update=[])
            else:
                esi.on_wait = [w]

    self.nc.all_engine_barrier()
    assert self.sems is not None
    popped = self.nc._tile_sem_poison_stack.pop()
    assert popped is self._sem_poison
    self.nc.clear_and_free_semaphores(list(self.sems.allocated().values()))
    self.nc.all_engine_barrier()


def _install_tile_patch():
    import concourse.tile as tile_mod
    tile_mod.TileContext._drain_and_barrier = _patched_drain_and_barrier


def _split_waits(nc, max_waits: int = 1):
    for fn in nc.m.functions:
        for bb in fn.blocks:
            out = []
            changed = False
            for inst in list(bb.instructions):
                si = inst.sync_info
                if si is not None and si.on_wait and len(si.on_wait) > max_waits:
                    waits = list(si.on_wait)
                    for w in waits[:-max_waits]:
                        out.append(mybir.InstNoOp(
                            name=nc.get_next_instruction_name(),
                            engine=inst.engine,
                            sync_info=mybir.SyncInfo(on_wait=[w], on_update=[]),
                        ))
                    si.on_wait = waits[-max_waits:]
                    changed = True
                out.append(inst)
            if changed:
                bb.instructions = out


# ---------------------------------------------------------------------------
# Kernel build (one Bass module, SPMD across the 8 cores via input slices)
# ---------------------------------------------------------------------------

def _build_nc():
    _install_tile_patch()
    nc = bass.Bass()

    xT = nc.dram_tensor("xT", [128, KT, S], bf16, kind="ExternalInput")
    wqT = nc.dram_tensor("wqT", [128, ET, KT, 128], bf16, kind="ExternalInput")
    wkT = nc.dram_tensor("wkT", [128, ET, KT, 128], bf16, kind="ExternalInput")
    wvT = nc.dram_tensor("wvT", [128, KT, E], bf16, kind="ExternalInput")
    woT = nc.dram_tensor("woT", [128, ET, D], bf16, kind="ExternalInput")
    cosF = nc.dram_tensor("cosF", [128, S], bf16, kind="ExternalInput")
    sinF = nc.dram_tensor("sinF", [128, S], bf16, kind="ExternalInput")
    triL = nc.dram_tensor("triL", [128, 128], bf16, kind="ExternalInput")
    triU = nc.dram_tensor("triU", [128, 128], bf16, kind="ExternalInput")
    rperm = nc.dram_tensor("rperm", [128, 128], bf16, kind="ExternalInput")
    onesb = nc.dram_tensor("onesb", [128, 1], bf16, kind="ExternalInput")
    onesr = nc.dram_tensor("onesr", [1, 128], bf16, kind="ExternalInput")
    out = nc.dram_tensor("out", [S, D], bf16, kind="ExternalOutput")

    with TileContext(nc) as tc:
        with (
            nc.allow_low_precision(reason="bf16 datapath, fp32 accumulation"),
            tc.tile_pool(name="consts", bufs=1) as consts,
            tc.tile_pool(name="resid", bufs=1) as resid,
        ):
            lt_sb = consts.tile([128, 128], bf16, tag="lt")
            ut_sb = consts.tile([128, 128], bf16, tag="ut")
            ob_sb = consts.tile([128, 1], bf16, tag="ob")
            or_sb = consts.tile([1, 128], bf16, tag="or")
            rp_sb = consts.tile([128, 128], bf16, tag="rp")
            c_sb = consts.tile([128, S], bf16, tag="cos")
            s_sb = consts.tile([128, S], bf16, tag="sin")
            # consts go on the idle Pool queue so they don't block the
            # critical first-chunk loads on the SP queue
            nc.gpsimd.dma_start(rp_sb[:], rperm[:])
            nc.gpsimd.dma_start(lt_sb[:], triL[:])
            nc.gpsimd.dma_start(ut_sb[:], triU[:])
            nc.gpsimd.dma_start(ob_sb[:], onesb[:])
            nc.gpsimd.dma_start(or_sb[:], onesr[:])

            # SBUF-resident Q^T/K^T (e-major per head) and V (t-major)
            qres = resid.tile([128, ET, S], bf16, tag="qres")
            kres = resid.tile([128, ET, S], bf16, tag="kres")
            vres = resid.tile([128, TT, E], bf16, tag="vres")
            wo_sb = resid.tile([128, ET, D], bf16, tag="wo")

            with (
                tc.tile_pool(name="wpool", bufs=1) as wpool,
                tc.tile_pool(name="xpool", bufs=3) as xpool,
                tc.tile_pool(name="stage", bufs=4) as stage,
                tc.tile_pool(name="oTp", bufs=2) as oTp,
                tc.tile_pool(name="ptp", bufs=4) as ptp,
                tc.tile_pool(name="accp", bufs=2) as accp,
                tc.tile_pool(name="rcp", bufs=2) as rcp,
                tc.tile_pool(name="ostp", bufs=2) as ostp,
                # PSUM: proj-acc/rope-swap/outproj ring (3) + scores (2,
                # ACT-paced so isolated) + PV accumulators (2) + softmax
                # sums/broadcast alternating in one bank = 8 banks
                tc.tile_pool(name="psMM", bufs=3, space="PSUM") as psMM,
                tc.tile_pool(name="psC", bufs=2, space="PSUM") as psC,
                tc.tile_pool(name="psD", bufs=2, space="PSUM") as psD,
                tc.tile_pool(name="psAux", bufs=1, space="PSUM") as psAux,
            ):
                wq_sb = wpool.tile([128, ET, KT, 128], bf16, tag="wq")
                wk_sb = wpool.tile([128, ET, KT, 128], bf16, tag="wk")
                wv_sb = wpool.tile([128, KT, E], bf16, tag="wv")

                xc_t = {}
                oT_c = {}
                # deferred-emission slots: PE-blocking ops postponed until
                # the engine has other queued work covering their input
                # latency (ACT copy for the rope swap, DVE chain for the
                # softmax normalization)
                pending = {"swap": None, "tail1": None, "tail2": None}

                def flush(key):
                    fn = pending[key]
                    if fn is not None:
                        pending[key] = None
                        fn()

                def emit_chunk_loads(tcb):
                    ts = slice(tcb * 512, (tcb + 1) * 512)
                    xc = xpool.tile([128, KT, 512], bf16, tag="xc")
                    xc_t[tcb] = xc
                    if tcb == 0:
                        # single queue: issue order == transfer order on
                        # the (serialized) DMA path, so strict priority:
                        # first-group inputs, then just-in-time streaming
                        nc.sync.dma_start(wq_sb[:, 0], wqT[:, 0])
                        for k in range(4):
                            nc.sync.dma_start(xc[:, k:k + 1], xT[:, k:k + 1, ts])
                        nc.sync.dma_start(xc[:, 4:8], xT[:, 4:8, ts])
                        nc.sync.dma_start(wq_sb[:, 1], wqT[:, 1])
                        nc.sync.dma_start(xc[:, 8:12], xT[:, 8:12, ts])
                        nc.sync.dma_start(wq_sb[:, 2], wqT[:, 2])
                        nc.sync.dma_start(xc[:, 12:16], xT[:, 12:16, ts])
                        nc.sync.dma_start(wq_sb[:, 3], wqT[:, 3])
                        nc.sync.dma_start(c_sb[:], cosF[:])
                        nc.sync.dma_start(s_sb[:], sinF[:])
                        for et in range(ET):
                            nc.sync.dma_start(wk_sb[:, et], wkT[:, et])
                        for kc in range(4):
                            ks = slice(kc * 4, (kc + 1) * 4)
                            nc.sync.dma_start(wv_sb[:, ks], wvT[:, ks])
                    else:
                        for kc in range(4):
                            ks = slice(kc * 4, (kc + 1) * 4)
                            nc.sync.dma_start(xc[:, ks], xT[:, ks, ts])
                    if tcb == 1:
                        for dcc in range(4):
                            dsl = slice(dcc * 512, (dcc + 1) * 512)
                            nc.sync.dma_start(wo_sb[:, :, dsl], woT[:, :, dsl])

                def proj_groups(tcb):
                    # 12 projection matmul groups for one x chunk, as
                    # closures so they can interleave into SDPA segments.
                    # The RoPE pair-swap matmul of each group is deferred
                    # behind the next group so the PE never waits on the
                    # ACT psum copy.
                    ts = slice(tcb * 512, (tcb + 1) * 512)
                    xc = xc_t[tcb]
                    groups = []

                    def mkqk(w_sb, dst, et):
                        def g():
                            pq = psMM.tile([128, 512], f32, tag="mm")
                            for k in range(KT):
                                nc.tensor.matmul(
                                    pq[:], w_sb[:, et, k, :], xc[:, k, :],
                                    start=(k == 0), stop=(k == KT - 1),
                                )
                            flush("swap")
                            qsb = stage.tile([128, 512], bf16, tag="qsb")
                            nc.scalar.copy(qsb[:], pq[:])

                            def f():
                                ps2 = psMM.tile([128, 512], f32, tag="mm")
                                nc.tensor.matmul(ps2[:], rp_sb[:], qsb[:],
                                                 start=True, stop=True)
                                t1 = stage.tile([128, 512], bf16, tag="t1")
                                t2 = stage.tile([128, 512], bf16, tag="t2")
                                nc.vector.tensor_tensor(
                                    t1[:], qsb[:], c_sb[:, ts], mult)
                                nc.vector.tensor_tensor(
                                    t2[:], ps2[:], s_sb[:, ts], mult)
                                nc.vector.tensor_tensor(
                                    dst[:, et, ts], t1[:], t2[:], add)
                            pending["swap"] = f
                        return g

                    def mkv(tt):
                        def g():
                            pv = psMM.tile([128, 512], f32, tag="mm")
                            for k in range(KT):
                                nc.tensor.matmul(
                                    pv[:], xc[:, k, tt * 128:(tt + 1) * 128],
                                    wv_sb[:, k, :],
                                    start=(k == 0), stop=(k == KT - 1),
                                )
                            flush("swap")
                            nc.scalar.copy(vres[:, 4 * tcb + tt, :], pv[:])
                        return g

                    for (w_sb, dst) in ((wq_sb, qres), (wk_sb, kres)):
                        for et in range(ET):
                            groups.append(mkqk(w_sb, dst, et))
                    for tt in range(4):
                        groups.append(mkv(tt))
                    return groups

                def emit_outproj_tile(tt, oT, eager_dma=False,
                                      copies_on_act=False):
                    # one t-tile of the output projection: 4 dc-quanta of 4
                    # matmuls each, returned as callables to interleave
                    tl = tt % 4
                    quanta = []
                    ost = ostp.tile([128, D], bf16, tag="ost")
                    rsl = slice(tt * 128, (tt + 1) * 128)

                    def mk(dc):
                        def q():
                            po = psMM.tile([128, 512], f32, tag="mm")
                            for eh in range(ET):
                                nc.tensor.matmul(
                                    po[:],
                                    oT[:, eh, tl * 128:(tl + 1) * 128],
                                    wo_sb[:, eh, dc * 512:(dc + 1) * 512],
                                    start=(eh == 0), stop=(eh == ET - 1),
                                )
                            dsl = slice(dc * 512, (dc + 1) * 512)
                            if dc < 3 and not copies_on_act:
                                nc.vector.tensor_copy(ost[:, dsl], po[:])
                            else:
                                nc.scalar.copy(ost[:, dsl], po[:])
                            if eager_dma:
                                nc.sync.dma_start(out[rsl, dsl], ost[:, dsl])
                        return q
                    for dc in range(4):
                        quanta.append(mk(dc))

                    def fin():
                        if not eager_dma:
                            nc.sync.dma_start(out[rsl, :], ost[:])
                    return quanta, fin

                def emit_sdpa(ic, gbh):
                    isl = slice(ic * 512, (ic + 1) * 512)
                    oT_ic = oTp.tile([128, ET, 512], bf16, tag="oT")
                    oT_c[ic] = oT_ic
                    for h in range(ET):
                        # deferred output projection work (one ic behind)
                        if ic > 0:
                            quanta, fin = emit_outproj_tile(
                                4 * (ic - 1) + h, oT_c[ic - 1],
                                copies_on_act=(ic == TC - 1 and h == ET - 1))
                        else:
                            quanta, fin = [], None
                        qtb = qres[:, h, isl]
                        ps_pv = psD.tile([128, 512], f32, tag="pv")
                        acc = accp.tile([128, 512], bf16, tag="acc")
                        # j-tiles: full below the diagonal block, then the
                        # 4 staircase tiles (i-extent shrinks by 128 each)
                        tiles = [(jt, 0) for jt in range(4 * ic)]
                        tiles += [(4 * ic + r, 128 * r) for r in range(4)]
                        n = len(tiles)
                        pts = [None] * n
                        LOOK = 3
                        quanta_done = [0]

                        def emit_pv(idx):
                            jt, ilo = tiles[idx]
                            nc.tensor.matmul(
                                ps_pv[:, ilo:512],
                                vres[:, jt, h * 128:(h + 1) * 128],
                                pts[idx][:, ilo:512],
                                start=(idx == 0), stop=(idx == n - 1),
                                skip_group_check=True,
                            )

                        for idx, (jt, ilo) in enumerate(tiles):
                            ps_sc = psC.tile([128, 512], f32, tag="sc")
                            nc.tensor.matmul(
                                ps_sc[:, ilo:512],
                                kres[:, h, jt * 128:(jt + 1) * 128],
                                qtb[:, ilo:512],
                                start=True, stop=True,
                            )
                            if jt >= 4 * ic:
                                # boundary-diagonal piece: accumulate
                                # -1e5*max(0, j-i) so exp gives exact zeros
                                nc.tensor.matmul(
                                    ps_sc[:, ilo:ilo + 128],
                                    lt_sb[:], ut_sb[:],
                                    start=False, stop=True,
                                    skip_group_check=True,
                                )
                            pt = ptp.tile([128, 512], bf16, tag="pt")
                            pts[idx] = pt
                            nc.scalar.activation(
                                pt[:, ilo:512], ps_sc[:, ilo:512], Exp,
                                scale=SCALE)
                            if idx == 0:
                                nc.vector.tensor_copy(acc[:], pt[:])
                            else:
                                nc.vector.scalar_tensor_tensor(
                                    acc[:, ilo:512], pt[:, ilo:512], 1.0,
                                    acc[:, ilo:512], mult, add)
                            if h == 0 and idx == 0:
                                # h0 quanta read every head slice of the
                                # previous oT - the pending tail must land
                                # before the first quantum
                                flush("tail1")
                                flush("tail2")
                            elif idx == 1:
                                flush("tail1")
                            elif idx == 2:
                                flush("swap")
                            elif idx == (4 if n > 4 else 3):
                                flush("tail2")
                            st = max(1, n // 4)
                            if quanta and idx % st == 0 and idx // st < 4:
                                quanta[idx // st]()
                                quanta_done[0] = idx // st + 1
                            if idx >= LOOK:
                                emit_pv(idx - LOOK)
                        for qi in range(quanta_done[0], len(quanta)):
                            quanta[qi]()
                        for idx in range(max(0, n - LOOK), n):
                            emit_pv(idx)
                        if fin is not None:
                            fin()

                        # normalization tail, deferred into the next head's
                        # tile loop: sums -> 1/sums, then broadcast -> mult
                        last = (ic == TC - 1 and h == ET - 1)

                        def mktails(acc=acc, ps_pv=ps_pv, oT_ic=oT_ic, h=h,
                                    split=last):
                            rc = rcp.tile([1, 512], bf16, tag="rc")

                            def t1():
                                ps_sums = psAux.tile([128, 512], f32,
                                                     tag="aux")
                                nc.tensor.matmul(ps_sums[0:1, :], ob_sb[:],
                                                 acc[:], start=True,
                                                 stop=True)
                                nc.vector.reciprocal(rc[:], ps_sums[0:1, :])

                            def t2():
                                rcb = rcp.tile([128, 512], bf16, tag="rcb")
                                pb = psAux.tile([128, 512], f32, tag="aux")
                                nc.tensor.matmul(pb[:], or_sb[:], rc[:],
                                                 start=True, stop=True)
                                if split:
                                    # 128-col pieces so the dependent
                                    # output projection can start after
                                    # the first piece lands
                                    for tl in range(4):
                                        tls = slice(tl * 128, (tl + 1) * 128)
                                        nc.vector.tensor_copy(
                                            rcb[:, tls], pb[:, tls])
                                        nc.vector.tensor_tensor(
                                            oT_ic[:, h, tls], ps_pv[:, tls],
                                            rcb[:, tls], mult)
                                else:
                                    nc.vector.tensor_copy(rcb[:], pb[:])
                                    nc.vector.tensor_tensor(
                                        oT_ic[:, h, :], ps_pv[:], rcb[:],
                                        mult)
                            return t1, t2
                        flush("tail1")
                        flush("tail2")
                        pending["tail1"], pending["tail2"] = mktails()

                        # next chunk's projection groups: pure PE work
                        # that fills the exp-paced bubbles, with the
                        # normalization tail flushed in between
                        for gi, g in enumerate(gbh[h]):
                            g()
                            if gi == 0:
                                flush("tail1")
                            elif gi == 1:
                                flush("tail2")

                # interleave projection chunks with SDPA i-chunks so ACT's
                # exp stream never throttles the tensor engine.  chunk 3's
                # Q(h)/K(h) groups pipeline INTO sdpa(3) between heads
                # (each head only needs its own q/k slices), keeping the
                # final - otherwise exp-bound - segment fed with PE work.
                emit_chunk_loads(0)
                # chunk-0 Q groups in DMA-arrival wavefront order: the
                # matmuls of et0-2 interleave by k-slice arrival so the PE
                # streams from the first weight landing instead of
                # serializing whole (DMA-gated) groups
                g0 = proj_groups(0)
                wave_pq = {}

                def wv_mm(et, k0, k1, close=False):
                    if et not in wave_pq:
                        wave_pq[et] = psMM.tile([128, 512], f32, tag="mm",
                                                name=f"wavepq{et}")
                    pq = wave_pq[et]
                    for k in range(k0, k1):
                        nc.tensor.matmul(
                            pq[:], wq_sb[:, et, k, :], xc_t[0][:, k, :],
                            start=(k == 0), stop=(close and k == k1 - 1),
                        )
                    if close:
                        qsb = stage.tile([128, 512], bf16, tag="qsb")
                        nc.scalar.copy(qsb[:], pq[:])

                        def f(qsb=qsb, et=et):
                            ps2 = psMM.tile([128, 512], f32, tag="mm")
                            nc.tensor.matmul(ps2[:], rp_sb[:], qsb[:],
                                             start=True, stop=True)
                            t1 = stage.tile([128, 512], bf16, tag="t1")
                            t2 = stage.tile([128, 512], bf16, tag="t2")
                            nc.vector.tensor_tensor(
                                t1[:], qsb[:], c_sb[:, 0:512], mult)
                            nc.vector.tensor_tensor(
                                t2[:], ps2[:], s_sb[:, 0:512], mult)
                            nc.vector.tensor_tensor(
                                qres[:, et, 0:512], t1[:], t2[:], add)
                        flush("swap")
                        pending["swap"] = f

                wv_mm(0, 0, 8)
                wv_mm(1, 0, 8)
                wv_mm(0, 8, 12)
                wv_mm(1, 8, 12)
                wv_mm(2, 0, 12)
                wv_mm(0, 12, 16, close=True)
                wv_mm(1, 12, 16, close=True)
                wv_mm(2, 12, 16, close=True)
                wv_mm(3, 0, 16, close=True)
                for g in g0[4:12]:
                    g()
                emit_chunk_loads(1)
                emit_chunk_loads(2)
                g1 = proj_groups(1)
                emit_sdpa(0, [g1[0:3], g1[3:6], g1[6:9], g1[9:12]])
                emit_chunk_loads(3)
                g2 = proj_groups(2)
                emit_sdpa(1, [g2[0:3], g2[3:6], g2[6:9], g2[9:12]])
                g3 = proj_groups(3)   # Q0-3 = g3[0:4], K0-3 = g3[4:8], V0-3 = g3[8:12]
                emit_sdpa(2, [[g3[8], g3[9]], [g3[10], g3[11]],
                              [g3[0]], [g3[4]]])
                emit_sdpa(3, [[g3[1], g3[5]], [g3[2], g3[6]],
                              [g3[3], g3[7]], []])

                # tail: output projection for the last i-chunk
                flush("tail1")
                flush("tail2")
                for h in range(ET):
                    quanta, fin = emit_outproj_tile(
                        4 * (TC - 1) + h, oT_c[TC - 1], eager_dma=True)
                    for q in quanta:
                        q()
                    fin()

    _split_waits(nc)
    return nc


_NC = None


def _get_nc():
    global _NC
    if _NC is None:
        _NC = _build_nc()
    return _NC


# ---------------------------------------------------------------------------
# Host-side prep + gather
# ---------------------------------------------------------------------------

def _rope_tables():
    j = np.arange(0, HD, 2, dtype=np.float32) / HD
    inv_freq = (1.0 / (ROPE_BASE ** j)).astype(np.float32)           # [64]
    t = np.arange(S, dtype=np.float32)
    freqs = np.outer(inv_freq, t)                                    # [64, S]
    cos = np.cos(freqs)
    sin = np.sin(freqs)
    cosF = np.empty((128, S), dtype=np.float32)
    sinF = np.empty((128, S), dtype=np.float32)
    cosF[0::2] = cos
    cosF[1::2] = cos
    sinF[0::2] = -sin
    sinF[1::2] = sin
    return cosF.astype(npbf16), sinF.astype(npbf16)


def _static_inputs():
    cosF, sinF = _rope_tables()
    k = np.arange(128)
    triL = (k[:, None] < k[None, :]).astype(np.float32)      # [k, j] k<j
    triU = np.where(k[:, None] >= k[None, :], -1e5, 0.0)     # [k, i] k>=i
    onesb = np.ones((128, 1), dtype=np.float32)
    onesr = np.ones((1, 128), dtype=np.float32)
    rperm = np.zeros((128, 128), dtype=np.float32)
    idx = np.arange(128)
    rperm[idx ^ 1, idx] = 1.0
    return {
        "cosF": cosF, "sinF": sinF,
        "triL": triL.astype(npbf16), "triU": triU.astype(npbf16),
        "onesb": onesb.astype(npbf16), "onesr": onesr.astype(npbf16),
        "rperm": rperm.astype(npbf16),
    }


def _core_inputs(x, wqk, wv, wo, static, b, g):
    xb = np.ascontiguousarray(x[b].T)                                # [D, S]
    xT = np.ascontiguousarray(
        xb.reshape(KT, 128, S).transpose(1, 0, 2)).astype(npbf16)

    wq_g = wqk[E * g:E * (g + 1), :]                                 # [E, D]
    wk_g = wqk[D + E * g:D + E * (g + 1), :]
    wv_g = wv[E * g:E * (g + 1), :]
    wqT = np.ascontiguousarray(
        wq_g.T.reshape(KT, 128, ET, 128).transpose(1, 2, 0, 3)).astype(npbf16)
    wkT = np.ascontiguousarray(
        wk_g.T.reshape(KT, 128, ET, 128).transpose(1, 2, 0, 3)).astype(npbf16)
    wvT = np.ascontiguousarray(
        wv_g.T.reshape(KT, 128, E).transpose(1, 0, 2)).astype(npbf16)
    woT = np.ascontiguousarray(
        wo[:, E * g:E * (g + 1)].T.reshape(ET, 128, D).transpose(1, 0, 2)
    ).astype(npbf16)

    m = dict(static)
    m.update({"xT": xT, "wqT": wqT, "wkT": wkT, "wvT": wvT, "woT": woT})
    return m


def kernel(x, wqk, wv, wo):
    x = np.asarray(x, dtype=np.float32)
    wqk = np.asarray(wqk, dtype=np.float32)
    wv = np.asarray(wv, dtype=np.float32)
    wo = np.asarray(wo, dtype=np.float32)

    nc = _get_nc()
    static = _static_inputs()
    in_maps = [
        _core_inputs(x, wqk, wv, wo, static, c // G, c % G) for c in range(8)
    ]
    res = run_bass_kernel_spmd(nc, in_maps, core_ids=list(range(8)))
    out = np.zeros((B, S, D), dtype=np.float32)
    for c in range(8):
        out[c // G] += res.results[c]["out"].astype(np.float32)
    return out


# revision 98
# speedup vs baseline: 1.2817x; 1.0047x over previous
"""Trainium2 Bass kernel for nn_Attention_43301860278871.

Full attention layer: fused QK projection + V projection, interleaved RoPE,
causal SDPA, output projection.  B=2, S=2048, D=2048, H=16, HD=128.

Sharding: 8 cores = 2 batches x 4 head-groups (tensor parallel over heads,
data parallel over batch).  Each core computes 4 heads for one batch and a
partial [S, D] output-projection contribution; the host sums the 4 partials
per batch, so no on-device collectives are needed.

Design (vs the f32r/DRAM-staging baseline at 402us):
  - All matmul operands in bf16 (host-converted; PSUM accumulation stays
    f32), halving DMA and letting Q/K/V stay SBUF-resident for the whole
    kernel - no DRAM round-trip, no phase barrier.  rel err ~8e-3 vs the
    2e-2 gate.
  - RoPE pair-swap via a bf16 permutation matmul (DVE lanes are
    partition-locked, so the swap cannot run there), deferred behind the
    next projection group so the PE never waits on the ACT psum copy.
  - Causal masking at true 128-granularity: per-j-tile scores matmuls
    with trimmed i-extents; the boundary-diagonal 128x128 piece gets a
    second matmul accumulating -1e5 * max(0, j-i) into the scores PSUM
    (lower-tri x upper-tri constant operands), so exp() produces exact
    zeros and no mask multiply exists anywhere on the critical path.
  - Softmax row sums: bf16 exp tiles accumulated over j-tiles on DVE
    (scalar_tensor_tensor, 4x mode) and reduced with ONE ones-column
    matmul per (head, i-chunk) instead of one per j-tile (the baseline
    spent 34us of PE streaming ones-matmuls).  1/sums broadcast via a
    ones-row matmul; normalization tails split in two stages and emitted
    inside the NEXT head's tile loop so their cross-engine latency hides
    behind queued PE work.
  - Global software pipeline: projection chunks interleave with SDPA
    i-chunks (12 proj groups spread over the preceding segment's heads;
    chunk 3's Q(h)/K(h) groups pipeline INTO sdpa(3) between heads), and
    the output projection runs one i-chunk behind SDPA in 4-matmul quanta
    spread across each head's score tiles - so the exp() stream on ACT
    (~87us) never throttles the tensor engine.
  - PSUM: 8 banks as proj/swap/outproj ring (3) + scores (2, the only
    ACT-paced pool, isolated) + PV accumulators (2) + sums/broadcast (1).
  - Single-queue DMA with strict priority order (the DMA path serializes
    at ~350GB/s, so issue order is everything): wq-et0 first, then x
    k-slices interleaved with remaining weight et-slices; chunk-0 Q
    matmuls emitted in DMA-arrival wavefront order.

TimelineSim: ~315.5us/core (PE 93% busy, 295us of a 290us matmul-stream
floor at 1 row/cycle); rel err 7.8e-3.
"""
import sys
sys.path.insert(0, '/opt/trn_rl_repo')

import numpy as np
import ml_dtypes

import concourse.bass as bass
import concourse.mybir as mybir
from concourse.bass_utils import run_bass_kernel_spmd
from concourse.tile import TileContext

B, S, D, H = 2, 2048, 2048, 16
HD = D // H            # 128
G = 4                  # head-groups (cores per batch)
HPG = H // G           # heads per core = 4
E = HPG * HD           # per-core projection width = 512
ROPE_BASE = 10000.0
SCALE = float(HD) ** -0.5

f32 = mybir.dt.float32
bf16 = mybir.dt.bfloat16
npbf16 = ml_dtypes.bfloat16

KT = D // 128          # 16 contraction tiles
TT = S // 128          # 16 token tiles
TC = S // 512          # 4 token chunks
ET = E // 128          # 4 e-tiles (= heads per core)

Exp = mybir.ActivationFunctionType.Exp
mult = mybir.AluOpType.mult
add = mybir.AluOpType.add


# ---------------------------------------------------------------------------
# Workarounds for this walrus build: at most ONE sem wait per instruction.
# Tile's scheduler attaches several; hoist the excess onto NoOps injected on
# the same engine immediately before (sequencer executes waits in order, so
# semantics are identical).
# ---------------------------------------------------------------------------

def _patched_drain_and_barrier(self, tick_clock, wait_clock):
    from concourse.vector_clock import ScopedClock
    drain_inst = self.nc.sync.drain()
    wait_clock.add_sem_waits(
        drain_inst.ins, ScopedClock({None: tick_clock.global_clock})
    )
    si = drain_inst.ins.sync_info
    if si is not None and si.on_wait and len(si.on_wait) > 1:
        waits = list(si.on_wait)
        si.on_wait = waits[:1]
        for w in waits[1:]:
            extra = self.nc.sync.drain()
            esi = extra.ins.sync_info
            if esi is None:
                extra.ins.sync_info = mybir.SyncInfo(on_wait=[w], on_update=[])
            else:
                esi.on_wait = [w]

    self.nc.all_engine_barrier()
    assert self.sems is not None
    popped = self.nc._tile_sem_poison_stack.pop()
    assert popped is self._sem_poison
    self.nc.clear_and_free_semaphores(list(self.sems.allocated().values()))
    self.nc.all_engine_barrier()


def _install_tile_patch():
    import concourse.tile as tile_mod
    tile_mod.TileContext._drain_and_barrier = _patched_drain_and_barrier


def _split_waits(nc, max_waits: int = 1):
    for fn in nc.m.functions:
        for bb in fn.blocks:
            out = []
            changed = False
            for inst in list(bb.instructions):
                si = inst.sync_info
                if si is not None and si.on_wait and len(si.on_wait) > max_waits:
                    waits = list(si.on_wait)
                    for w in waits[:-max_waits]:
                        out.append(mybir.InstNoOp(
                            name=nc.get_next_instruction_name(),
                            engine=inst.engine,
                            sync_info=mybir.SyncInfo(on_wait=[w], on_update=[]),
                        ))
                    si.on_wait = waits[-max_waits:]
                    changed = True
                out.append(inst)
            if changed:
                bb.instructions = out


# ---------------------------------------------------------------------------
# Kernel build (one Bass module, SPMD across the 8 cores via input slices)
# ---------------------------------------------------------------------------

def _build_nc():
    _install_tile_patch()
    nc = bass.Bass()

    xT = nc.dram_tensor("xT", [128, KT, S], bf16, kind="ExternalInput")
    wqT = nc.dram_tensor("wqT", [128, ET, KT, 128], bf16, kind="ExternalInput")
    wkT = nc.dram_tensor("wkT", [128, ET, KT, 128], bf16, kind="ExternalInput")
    wvT = nc.dram_tensor("wvT", [128, KT, E], bf16, kind="ExternalInput")
    woT = nc.dram_tensor("woT", [128, ET, D], bf16, kind="ExternalInput")
    cosF = nc.dram_tensor("cosF", [128, S], bf16, kind="ExternalInput")
    sinF = nc.dram_tensor("sinF", [128, S], bf16, kind="ExternalInput")
    triL = nc.dram_tensor("triL", [128, 128], bf16, kind="ExternalInput")
    triU = nc.dram_tensor("triU", [128, 128], bf16, kind="ExternalInput")
    rperm = nc.dram_tensor("rperm", [128, 128], bf16, kind="ExternalInput")
    onesb = nc.dram_tensor("onesb", [128, 1], bf16, kind="ExternalInput")
    onesr = nc.dram_tensor("onesr", [1, 128], bf16, kind="ExternalInput")
    out = nc.dram_tensor("out", [S, D], bf16, kind="ExternalOutput")

    with TileContext(nc) as tc:
        with (
            nc.allow_low_precision(reason="bf16 datapath, fp32 accumulation"),
            tc.tile_pool(name="consts", bufs=1) as consts,
            tc.tile_pool(name="resid", bufs=1) as resid,
        ):
            lt_sb = consts.tile([128, 128], bf16, tag="lt")
            ut_sb = consts.tile([128, 128], bf16, tag="ut")
            ob_sb = consts.tile([128, 1], bf16, tag="ob")
            or_sb = consts.tile([1, 128], bf16, tag="or")
            rp_sb = consts.tile([128, 128], bf16, tag="rp")
            c_sb = consts.tile([128, S], bf16, tag="cos")
            s_sb = consts.tile([128, S], bf16, tag="sin")
            # consts go on the idle Pool queue so they don't block the
            # critical first-chunk loads on the SP queue
            nc.gpsimd.dma_start(rp_sb[:], rperm[:])
            nc.gpsimd.dma_start(lt_sb[:], triL[:])
            nc.gpsimd.dma_start(ut_sb[:], triU[:])
            nc.gpsimd.dma_start(ob_sb[:], onesb[:])
            nc.gpsimd.dma_start(or_sb[:], onesr[:])

            # SBUF-resident Q^T/K^T (e-major per head) and V (t-major)
            qres = resid.tile([128, ET, S], bf16, tag="qres")
            kres = resid.tile([128, ET, S], bf16, tag="kres")
            vres = resid.tile([128, TT, E], bf16, tag="vres")
            wo_sb = resid.tile([128, ET, D], bf16, tag="wo")

            with (
                tc.tile_pool(name="wpool", bufs=1) as wpool,
                tc.tile_pool(name="xpool", bufs=3) as xpool,
                tc.tile_pool(name="stage", bufs=4) as stage,
                tc.tile_pool(name="oTp", bufs=2) as oTp,
                tc.tile_pool(name="ptp", bufs=4) as ptp,
                tc.tile_pool(name="accp", bufs=2) as accp,
                tc.tile_pool(name="rcp", bufs=2) as rcp,
                tc.tile_pool(name="ostp", bufs=2) as ostp,
                # PSUM: proj-acc/rope-swap/outproj ring (3) + scores (2,
                # ACT-paced so isolated) + PV accumulators (2) + softmax
                # sums/broadcast alternating in one bank = 8 banks
                tc.tile_pool(name="psMM", bufs=3, space="PSUM") as psMM,
                tc.tile_pool(name="psC", bufs=2, space="PSUM") as psC,
                tc.tile_pool(name="psD", bufs=2, space="PSUM") as psD,
                tc.tile_pool(name="psAux", bufs=1, space="PSUM") as psAux,
            ):
                wq_sb = wpool.tile([128, ET, KT, 128], bf16, tag="wq")
                wk_sb = wpool.tile([128, ET, KT, 128], bf16, tag="wk")
                wv_sb = wpool.tile([128, KT, E], bf16, tag="wv")

                xc_t = {}
                oT_c = {}
                pre_po = []
                # deferred-emission slots: PE-blocking ops postponed until
                # the engine has other queued work covering their input
                # latency (ACT copy for the rope swap, DVE chain for the
                # softmax normalization)
                pending = {"swap": None, "tail1": None, "tail2": None}

                def flush(key):
                    fn = pending[key]
                    if fn is not None:
                        pending[key] = None
                        fn()

                def emit_chunk_loads(tcb):
                    ts = slice(tcb * 512, (tcb + 1) * 512)
                    xc = xpool.tile([128, KT, 512], bf16, tag="xc")
                    xc_t[tcb] = xc
                    if tcb == 0:
                        # single queue: issue order == transfer order on
                        # the (serialized) DMA path, so strict priority:
                        # first-group inputs, then just-in-time streaming
                        nc.sync.dma_start(wq_sb[:, 0], wqT[:, 0])
                        nc.sync.dma_start(xc[:, 0:1], xT[:, 0:1, ts])
                        nc.sync.dma_start(xc[:, 1:4], xT[:, 1:4, ts])
                        nc.sync.dma_start(xc[:, 4:8], xT[:, 4:8, ts])
                        nc.sync.dma_start(wq_sb[:, 1], wqT[:, 1])
                        nc.sync.dma_start(xc[:, 8:12], xT[:, 8:12, ts])
                        nc.sync.dma_start(wq_sb[:, 2], wqT[:, 2])
                        nc.sync.dma_start(xc[:, 12:16], xT[:, 12:16, ts])
                        nc.sync.dma_start(wq_sb[:, 3], wqT[:, 3])
                        nc.sync.dma_start(c_sb[:], cosF[:])
                        nc.sync.dma_start(s_sb[:], sinF[:])
                        for et in range(ET):
                            nc.sync.dma_start(wk_sb[:, et], wkT[:, et])
                        for kc in range(4):
                            ks = slice(kc * 4, (kc + 1) * 4)
                            nc.sync.dma_start(wv_sb[:, ks], wvT[:, ks])
                    else:
                        for kc in range(4):
                            ks = slice(kc * 4, (kc + 1) * 4)
                            nc.sync.dma_start(xc[:, ks], xT[:, ks, ts])
                    if tcb == 1:
                        for dcc in range(4):
                            dsl = slice(dcc * 512, (dcc + 1) * 512)
                            nc.sync.dma_start(wo_sb[:, :, dsl], woT[:, :, dsl])

                def proj_groups(tcb):
                    # 12 projection matmul groups for one x chunk, as
                    # closures so they can interleave into SDPA segments.
                    # The RoPE pair-swap matmul of each group is deferred
                    # behind the next group so the PE never waits on the
                    # ACT psum copy.
                    ts = slice(tcb * 512, (tcb + 1) * 512)
                    xc = xc_t[tcb]
                    groups = []

                    def mkqk(w_sb, dst, et):
                        def g():
                            pq = psMM.tile([128, 512], f32, tag="mm")
                            for k in range(KT):
                                nc.tensor.matmul(
                                    pq[:], w_sb[:, et, k, :], xc[:, k, :],
                                    start=(k == 0), stop=(k == KT - 1),
                                )
                                if k == 7:
                                    # mid-group: the previous swap's DVE
                                    # chain gets cover before anything
                                    # reads its qres/kres write, and the
                                    # qsb ACT copy has had time to land
                                    flush("swap")
                            qsb = stage.tile([128, 512], bf16, tag="qsb")
                            nc.scalar.copy(qsb[:], pq[:])

                            def f():
                                ps2 = psMM.tile([128, 512], f32, tag="mm")
                                nc.tensor.matmul(ps2[:], rp_sb[:], qsb[:],
                                                 start=True, stop=True)
                                t1 = stage.tile([128, 512], bf16, tag="t1")
                                t2 = stage.tile([128, 512], bf16, tag="t2")
                                nc.vector.tensor_tensor(
                                    t1[:], qsb[:], c_sb[:, ts], mult)
                                nc.vector.tensor_tensor(
                                    t2[:], ps2[:], s_sb[:, ts], mult)
                                nc.vector.tensor_tensor(
                                    dst[:, et, ts], t1[:], t2[:], add)
                            pending["swap"] = f
                        return g

                    def mkv(tt):
                        def g():
                            pv = psMM.tile([128, 512], f32, tag="mm")
                            for k in range(KT):
                                nc.tensor.matmul(
                                    pv[:], xc[:, k, tt * 128:(tt + 1) * 128],
                                    wv_sb[:, k, :],
                                    start=(k == 0), stop=(k == KT - 1),
                                )
                                if k == 7:
                                    flush("swap")
                            nc.scalar.copy(vres[:, 4 * tcb + tt, :], pv[:])
                        return g

                    for (w_sb, dst) in ((wq_sb, qres), (wk_sb, kres)):
                        for et in range(ET):
                            groups.append(mkqk(w_sb, dst, et))
                    for tt in range(4):
                        groups.append(mkv(tt))
                    return groups

                def emit_outproj_tile(tt, oT, eager_dma=False,
                                      copies_on_act=False, alt_queue=False):
                    # one t-tile of the output projection: 4 dc-quanta of 4
                    # matmuls each, returned as callables to interleave
                    tl = tt % 4
                    quanta = []
                    ost = ostp.tile([128, D], bf16, tag="ost")
                    rsl = slice(tt * 128, (tt + 1) * 128)

                    def mk(dc):
                        def q():
                            po = psMM.tile([128, 512], f32, tag="mm")
                            for eh in range(ET):
                                nc.tensor.matmul(
                                    po[:],
                                    oT[:, eh, tl * 128:(tl + 1) * 128],
                                    wo_sb[:, eh, dc * 512:(dc + 1) * 512],
                                    start=(eh == 0), stop=(eh == ET - 1),
                                )
                            dsl = slice(dc * 512, (dc + 1) * 512)
                            if dc < 3 and not copies_on_act:
                                nc.vector.tensor_copy(ost[:, dsl], po[:])
                            else:
                                nc.scalar.copy(ost[:, dsl], po[:])
                            if eager_dma:
                                nc.sync.dma_start(out[rsl, dsl], ost[:, dsl])
                        return q
                    for dc in range(4):
                        quanta.append(mk(dc))

                    def fin():
                        if not eager_dma:
                            nc.sync.dma_start(out[rsl, :], ost[:])
                    return quanta, fin

                def emit_sdpa(ic, gbh):
                    isl = slice(ic * 512, (ic + 1) * 512)
                    oT_ic = oTp.tile([128, ET, 512], bf16, tag="oT")
                    oT_c[ic] = oT_ic
                    for h in range(ET):
                        # deferred output projection work (one ic behind)
                        if ic > 0:
                            quanta, fin = emit_outproj_tile(
                                4 * (ic - 1) + h, oT_c[ic - 1],
                                copies_on_act=(ic == TC - 1 and h == ET - 1))
                        else:
                            quanta, fin = [], None
                        qtb = qres[:, h, isl]
                        ps_pv = psD.tile([128, 512], f32, tag="pv")
                        acc = accp.tile([128, 512], bf16, tag="acc")
                        # j-tiles: full below the diagonal block, then the
                        # 4 staircase tiles (i-extent shrinks by 128 each)
                        tiles = [(jt, 0) for jt in range(4 * ic)]
                        tiles += [(4 * ic + r, 128 * r) for r in range(4)]
                        n = len(tiles)
                        pts = [None] * n
                        LOOK = 3
                        quanta_done = [0]

                        def emit_pv(idx):
                            jt, ilo = tiles[idx]
                            nc.tensor.matmul(
                                ps_pv[:, ilo:512],
                                vres[:, jt, h * 128:(h + 1) * 128],
                                pts[idx][:, ilo:512],
                                start=(idx == 0), stop=(idx == n - 1),
                                skip_group_check=True,
                            )

                        for idx, (jt, ilo) in enumerate(tiles):
                            ps_sc = psC.tile([128, 512], f32, tag="sc")
                            nc.tensor.matmul(
                                ps_sc[:, ilo:512],
                                kres[:, h, jt * 128:(jt + 1) * 128],
                                qtb[:, ilo:512],
                                start=True, stop=True,
                            )
                            if jt >= 4 * ic:
                                # boundary-diagonal piece: accumulate
                                # -1e5*max(0, j-i) so exp gives exact zeros
                                nc.tensor.matmul(
                                    ps_sc[:, ilo:ilo + 128],
                                    lt_sb[:], ut_sb[:],
                                    start=False, stop=True,
                                    skip_group_check=True,
                                )
                            pt = ptp.tile([128, 512], bf16, tag="pt")
                            pts[idx] = pt
                            nc.scalar.activation(
                                pt[:, ilo:512], ps_sc[:, ilo:512], Exp,
                                scale=SCALE)
                            if idx == 0:
                                nc.vector.tensor_copy(acc[:], pt[:])
                            else:
                                nc.vector.scalar_tensor_tensor(
                                    acc[:, ilo:512], pt[:, ilo:512], 1.0,
                                    acc[:, ilo:512], mult, add)
                            if h == 0 and idx == 0:
                                # h0 quanta read every head slice of the
                                # previous oT - the pending tail must land
                                # before the first quantum
                                flush("tail1")
                                flush("tail2")
                            elif idx == 1:
                                flush("tail1")
                            elif idx == 2:
                                flush("swap")
                            elif idx == (4 if n > 4 else 3):
                                flush("tail2")
                            st = max(1, n // 4)
                            if quanta and idx % st == 0 and idx // st < 4:
                                quanta[idx // st]()
                                quanta_done[0] = idx // st + 1
                            if idx >= LOOK:
                                emit_pv(idx - LOOK)
                        for qi in range(quanta_done[0], len(quanta)):
                            quanta[qi]()
                        for idx in range(max(0, n - LOOK), n):
                            emit_pv(idx)
                        if fin is not None:
                            fin()

                        if ic == TC - 1 and h == ET - 1:
                            # pre-start the first two tail-outproj quanta
                            # with the three already-normalized heads; the
                            # eh3 finish lands right after the last norm
                            for dc in range(2):
                                po = psMM.tile([128, 512], f32, tag="mm",
                                               name=f"pre{dc}")
                                for eh in range(3):
                                    nc.tensor.matmul(
                                        po[:], oT_ic[:, eh, 0:128],
                                        wo_sb[:, eh, dc * 512:(dc + 1) * 512],
                                        start=(eh == 0), stop=False,
                                    )
                                pre_po.append(po)

                        # normalization tail, deferred into the next head's
                        # tile loop: sums -> 1/sums, then broadcast -> mult
                        last = (ic == TC - 1 and h == ET - 1)

                        def mktails(acc=acc, ps_pv=ps_pv, oT_ic=oT_ic, h=h,
                                    split=last):
                            rc = rcp.tile([1, 512], bf16, tag="rc")

                            def t1():
                                ps_sums = psAux.tile([128, 512], f32,
                                                     tag="aux")
                                nc.tensor.matmul(ps_sums[0:1, :], ob_sb[:],
                                                 acc[:], start=True,
                                                 stop=True)
                                nc.vector.reciprocal(rc[:], ps_sums[0:1, :])

                            def t2():
                                rcb = rcp.tile([128, 512], bf16, tag="rcb")
                                pb = psAux.tile([128, 512], f32, tag="aux")
                                nc.tensor.matmul(pb[:], or_sb[:], rc[:],
                                                 start=True, stop=True)
                                if split:
                                    # 128-col pieces so the dependent
                                    # output projection can start after
                                    # the first piece lands
                                    for tl in range(4):
                                        tls = slice(tl * 128, (tl + 1) * 128)
                                        nc.vector.tensor_copy(
                                            rcb[:, tls], pb[:, tls])
                                        nc.vector.tensor_tensor(
                                            oT_ic[:, h, tls], ps_pv[:, tls],
                                            rcb[:, tls], mult)
                                else:
                                    nc.vector.tensor_copy(rcb[:], pb[:])
                                    nc.vector.tensor_tensor(
                                        oT_ic[:, h, :], ps_pv[:], rcb[:],
                                        mult)
                            return t1, t2
                        flush("tail1")
                        flush("tail2")
                        pending["tail1"], pending["tail2"] = mktails()

                        # next chunk's projection groups: pure PE work
                        # that fills the exp-paced bubbles, with the
                        # normalization tail flushed in between
                        for gi, g in enumerate(gbh[h]):
                            g()
                            if gi == 0:
                                flush("tail1")
                            elif gi == 1:
                                flush("tail2")

                # interleave projection chunks with SDPA i-chunks so ACT's
                # exp stream never throttles the tensor engine.  chunk 3's
                # Q(h)/K(h) groups pipeline INTO sdpa(3) between heads
                # (each head only needs its own q/k slices), keeping the
                # final - otherwise exp-bound - segment fed with PE work.
                emit_chunk_loads(0)
                # chunk-0 Q groups in DMA-arrival wavefront order: the
                # matmuls of et0-2 interleave by k-slice arrival so the PE
                # streams from the first weight landing instead of
                # serializing whole (DMA-gated) groups
                g0 = proj_groups(0)
                wave_pq = {}

                def wv_mm(et, k0, k1, close=False):
                    if et not in wave_pq:
                        wave_pq[et] = psMM.tile([128, 512], f32, tag="mm",
                                                name=f"wavepq{et}")
                    pq = wave_pq[et]
                    for k in range(k0, k1):
                        nc.tensor.matmul(
                            pq[:], wq_sb[:, et, k, :], xc_t[0][:, k, :],
                            start=(k == 0), stop=(close and k == k1 - 1),
                        )
                    if close:
                        qsb = stage.tile([128, 512], bf16, tag="qsb")
                        nc.scalar.copy(qsb[:], pq[:])

                        def f(qsb=qsb, et=et):
                            ps2 = psMM.tile([128, 512], f32, tag="mm")
                            nc.tensor.matmul(ps2[:], rp_sb[:], qsb[:],
                                             start=True, stop=True)
                            t1 = stage.tile([128, 512], bf16, tag="t1")
                            t2 = stage.tile([128, 512], bf16, tag="t2")
                            nc.vector.tensor_tensor(
                                t1[:], qsb[:], c_sb[:, 0:512], mult)
                            nc.vector.tensor_tensor(
                                t2[:], ps2[:], s_sb[:, 0:512], mult)
                            nc.vector.tensor_tensor(
                                qres[:, et, 0:512], t1[:], t2[:], add)
                        flush("swap")
                        pending["swap"] = f

                wv_mm(0, 0, 8)
                wv_mm(1, 0, 8)
                wv_mm(0, 8, 12)
                wv_mm(1, 8, 12)
                wv_mm(2, 0, 12)
                wv_mm(0, 12, 16, close=True)
                wv_mm(1, 12, 16, close=True)
                wv_mm(2, 12, 16, close=True)
                wv_mm(3, 0, 16, close=True)
                for g in g0[4:12]:
                    g()
                emit_chunk_loads(1)
                emit_chunk_loads(2)
                g1 = proj_groups(1)
                emit_sdpa(0, [g1[0:3], g1[3:6], g1[6:9], g1[9:12]])
                emit_chunk_loads(3)
                g2 = proj_groups(2)
                emit_sdpa(1, [g2[0:3], g2[3:6], g2[6:9], g2[9:12]])
                g3 = proj_groups(3)   # Q0-3 = g3[0:4], K0-3 = g3[4:8], V0-3 = g3[8:12]
                emit_sdpa(2, [[g3[8], g3[9]], [g3[10], g3[11]],
                              [g3[0]], [g3[4]]])
                emit_sdpa(3, [[g3[1], g3[5]], [g3[2], g3[6]],
                              [g3[3], g3[7]], []])

                # tail: output projection for the last i-chunk
                flush("tail1")
                flush("tail2")
                oT3 = oT_c[TC - 1]
                tt0 = 4 * (TC - 1)
                ost0 = ostp.tile([128, D], bf16, tag="ost")
                rsl0 = slice(tt0 * 128, (tt0 + 1) * 128)
                for dc in range(4):
                    dsl = slice(dc * 512, (dc + 1) * 512)
                    if dc < 2:
                        po = pre_po[dc]
                        nc.tensor.matmul(po[:], oT3[:, 3, 0:128],
                                         wo_sb[:, 3, dsl],
                                         start=False, stop=True)
                    else:
                        po = psMM.tile([128, 512], f32, tag="mm",
                                       name=f"tl{dc}")
                        for eh in range(ET):
                            nc.tensor.matmul(
                                po[:], oT3[:, eh, 0:128], wo_sb[:, eh, dsl],
                                start=(eh == 0), stop=(eh == ET - 1),
                            )
                    if dc < 3:
                        nc.vector.tensor_copy(ost0[:, dsl], po[:])
                    else:
                        nc.scalar.copy(ost0[:, dsl], po[:])
                    nc.sync.dma_start(out[rsl0, dsl], ost0[:, dsl])
                for h in range(1, ET):
                    quanta, fin = emit_outproj_tile(
                        4 * (TC - 1) + h, oT_c[TC - 1], eager_dma=True,
                        alt_queue=(h == ET - 1))
                    for q in quanta:
                        q()
                    fin()

    _split_waits(nc)
    return nc


_NC = None


def _get_nc():
    global _NC
    if _NC is None:
        _NC = _build_nc()
    return _NC


# ---------------------------------------------------------------------------
# Host-side prep + gather
# ---------------------------------------------------------------------------

def _rope_tables():
    j = np.arange(0, HD, 2, dtype=np.float32) / HD
    inv_freq = (1.0 / (ROPE_BASE ** j)).astype(np.float32)           # [64]
    t = np.arange(S, dtype=np.float32)
    freqs = np.outer(inv_freq, t)                                    # [64, S]
    cos = np.cos(freqs)
    sin = np.sin(freqs)
    cosF = np.empty((128, S), dtype=np.float32)
    sinF = np.empty((128, S), dtype=np.float32)
    cosF[0::2] = cos
    cosF[1::2] = cos
    sinF[0::2] = -sin
    sinF[1::2] = sin
    return cosF.astype(npbf16), sinF.astype(npbf16)


def _static_inputs():
    cosF, sinF = _rope_tables()
    k = np.arange(128)
    triL = (k[:, None] < k[None, :]).astype(np.float32)      # [k, j] k<j
    triU = np.where(k[:, None] >= k[None, :], -1e5, 0.0)     # [k, i] k>=i
    onesb = np.ones((128, 1), dtype=np.float32)
    onesr = np.ones((1, 128), dtype=np.float32)
    rperm = np.zeros((128, 128), dtype=np.float32)
    idx = np.arange(128)
    rperm[idx ^ 1, idx] = 1.0
    return {
        "cosF": cosF, "sinF": sinF,
        "triL": triL.astype(npbf16), "triU": triU.astype(npbf16),
        "onesb": onesb.astype(npbf16), "onesr": onesr.astype(npbf16),
        "rperm": rperm.astype(npbf16),
    }


def _core_inputs(x, wqk, wv, wo, static, b, g):
    xb = np.ascontiguousarray(x[b].T)                                # [D, S]
    xT = np.ascontiguousarray(
        xb.reshape(KT, 128, S).transpose(1, 0, 2)).astype(npbf16)

    wq_g = wqk[E * g:E * (g + 1), :]                                 # [E, D]
    wk_g = wqk[D + E * g:D + E * (g + 1), :]
    wv_g = wv[E * g:E * (g + 1), :]
    wqT = np.ascontiguousarray(
        wq_g.T.reshape(KT, 128, ET, 128).transpose(1, 2, 0, 3)).astype(npbf16)
    wkT = np.ascontiguousarray(
        wk_g.T.reshape(KT, 128, ET, 128).transpose(1, 2, 0, 3)).astype(npbf16)
    wvT = np.ascontiguousarray(
        wv_g.T.reshape(KT, 128, E).transpose(1, 0, 2)).astype(npbf16)
    woT = np.ascontiguousarray(
        wo[:, E * g:E * (g + 1)].T.reshape(ET, 128, D).transpose(1, 0, 2)
    ).astype(npbf16)

    m = dict(static)
    m.update({"xT": xT, "wqT": wqT, "wkT": wkT, "wvT": wvT, "woT": woT})
    return m


def kernel(x, wqk, wv, wo):
    x = np.asarray(x, dtype=np.float32)
    wqk = np.asarray(wqk, dtype=np.float32)
    wv = np.asarray(wv, dtype=np.float32)
    wo = np.asarray(wo, dtype=np.float32)

    nc = _get_nc()
    static = _static_inputs()
    in_maps = [
        _core_inputs(x, wqk, wv, wo, static, c // G, c % G) for c in range(8)
    ]
    res = run_bass_kernel_spmd(nc, in_maps, core_ids=list(range(8)))
    out = np.zeros((B, S, D), dtype=np.float32)
    for c in range(8):
        out[c // G] += res.results[c]["out"].astype(np.float32)
    return out


# revision 104
# speedup vs baseline: 1.2860x; 1.0034x over previous
"""Trainium2 Bass kernel for nn_Attention_43301860278871.

Full attention layer: fused QK projection + V projection, interleaved RoPE,
causal SDPA, output projection.  B=2, S=2048, D=2048, H=16, HD=128.

Sharding: 8 cores = 2 batches x 4 head-groups (tensor parallel over heads,
data parallel over batch).  Each core computes 4 heads for one batch and a
partial [S, D] output-projection contribution; the host sums the 4 partials
per batch, so no on-device collectives are needed.

Design (vs the f32r/DRAM-staging baseline at 402us):
  - All matmul operands in bf16 (host-converted; PSUM accumulation stays
    f32), halving DMA and letting Q/K/V stay SBUF-resident for the whole
    kernel - no DRAM round-trip, no phase barrier.  rel err ~8e-3 vs the
    2e-2 gate.
  - RoPE pair-swap via a bf16 permutation matmul (DVE lanes are
    partition-locked, so the swap cannot run there), deferred behind the
    next projection group so the PE never waits on the ACT psum copy.
  - Causal masking at true 128-granularity: per-j-tile scores matmuls
    with trimmed i-extents; the boundary-diagonal 128x128 piece gets a
    second matmul accumulating -1e5 * max(0, j-i) into the scores PSUM
    (lower-tri x upper-tri constant operands), so exp() produces exact
    zeros and no mask multiply exists anywhere on the critical path.
  - Softmax row sums: bf16 exp tiles accumulated over j-tiles on DVE
    (scalar_tensor_tensor, 4x mode) and reduced with ONE ones-column
    matmul per (head, i-chunk) instead of one per j-tile (the baseline
    spent 34us of PE streaming ones-matmuls).  1/sums broadcast via a
    ones-row matmul; normalization tails split in two stages and emitted
    inside the NEXT head's tile loop so their cross-engine latency hides
    behind queued PE work.
  - Global software pipeline: projection chunks interleave with SDPA
    i-chunks (12 proj groups spread over the preceding segment's heads;
    chunk 3's Q(h)/K(h) groups pipeline INTO sdpa(3) between heads), and
    the output projection runs one i-chunk behind SDPA in 4-matmul quanta
    spread across each head's score tiles - so the exp() stream on ACT
    (~87us) never throttles the tensor engine.
  - PSUM: 8 banks as proj/swap/outproj ring (3) + scores (2, the only
    ACT-paced pool, isolated) + PV accumulators (2) + sums/broadcast (1).
  - Single-queue DMA with strict priority order (the DMA path serializes
    at ~350GB/s, so issue order is everything): wq-et0 first, then x
    k-slices interleaved with remaining weight et-slices; chunk-0 Q
    matmuls emitted in DMA-arrival wavefront order.

TimelineSim / HW exec: ~314us/core (PE ~94% busy, ~295us of a 290us
matmul-stream floor at 1 row/cycle); rel err 7.8e-3.
"""
import sys
sys.path.insert(0, '/opt/trn_rl_repo')

import numpy as np
import ml_dtypes

import concourse.bass as bass
import concourse.mybir as mybir
from concourse.bass_utils import run_bass_kernel_spmd
from concourse.tile import TileContext

B, S, D, H = 2, 2048, 2048, 16
HD = D // H            # 128
G = 4                  # head-groups (cores per batch)
HPG = H // G           # heads per core = 4
E = HPG * HD           # per-core projection width = 512
ROPE_BASE = 10000.0
SCALE = float(HD) ** -0.5

f32 = mybir.dt.float32
bf16 = mybir.dt.bfloat16
npbf16 = ml_dtypes.bfloat16

KT = D // 128          # 16 contraction tiles
TT = S // 128          # 16 token tiles
TC = S // 512          # 4 token chunks
ET = E // 128          # 4 e-tiles (= heads per core)

Exp = mybir.ActivationFunctionType.Exp
mult = mybir.AluOpType.mult
add = mybir.AluOpType.add


# ---------------------------------------------------------------------------
# Workarounds for this walrus build: at most ONE sem wait per instruction.
# Tile's scheduler attaches several; hoist the excess onto NoOps injected on
# the same engine immediately before (sequencer executes waits in order, so
# semantics are identical).
# ---------------------------------------------------------------------------

def _patched_drain_and_barrier(self, tick_clock, wait_clock):
    from concourse.vector_clock import ScopedClock
    drain_inst = self.nc.sync.drain()
    wait_clock.add_sem_waits(
        drain_inst.ins, ScopedClock({None: tick_clock.global_clock})
    )
    si = drain_inst.ins.sync_info
    if si is not None and si.on_wait and len(si.on_wait) > 1:
        waits = list(si.on_wait)
        si.on_wait = waits[:1]
        for w in waits[1:]:
            extra = self.nc.sync.drain()
            esi = extra.ins.sync_info
            if esi is None:
                extra.ins.sync_info = mybir.SyncInfo(on_wait=[w], on_update=[])
            else:
                esi.on_wait = [w]

    self.nc.all_engine_barrier()
    assert self.sems is not None
    popped = self.nc._tile_sem_poison_stack.pop()
    assert popped is self._sem_poison
    self.nc.clear_and_free_semaphores(list(self.sems.allocated().values()))
    self.nc.all_engine_barrier()


def _install_tile_patch():
    import concourse.tile as tile_mod
    tile_mod.TileContext._drain_and_barrier = _patched_drain_and_barrier


def _split_waits(nc, max_waits: int = 1):
    for fn in nc.m.functions:
        for bb in fn.blocks:
            out = []
            changed = False
            for inst in list(bb.instructions):
                si = inst.sync_info
                if si is not None and si.on_wait and len(si.on_wait) > max_waits:
                    waits = list(si.on_wait)
                    for w in waits[:-max_waits]:
                        out.append(mybir.InstNoOp(
                            name=nc.get_next_instruction_name(),
                            engine=inst.engine,
                            sync_info=mybir.SyncInfo(on_wait=[w], on_update=[]),
                        ))
                    si.on_wait = waits[-max_waits:]
                    changed = True
                out.append(inst)
            if changed:
                bb.instructions = out


# ---------------------------------------------------------------------------
# Kernel build (one Bass module, SPMD across the 8 cores via input slices)
# ---------------------------------------------------------------------------

def _build_nc():
    _install_tile_patch()
    nc = bass.Bass()

    xT = nc.dram_tensor("xT", [128, KT, S], bf16, kind="ExternalInput")
    wqT = nc.dram_tensor("wqT", [128, ET, KT, 128], bf16, kind="ExternalInput")
    wkT = nc.dram_tensor("wkT", [128, ET, KT, 128], bf16, kind="ExternalInput")
    wvT = nc.dram_tensor("wvT", [128, KT, E], bf16, kind="ExternalInput")
    woT = nc.dram_tensor("woT", [128, ET, D], bf16, kind="ExternalInput")
    cosF = nc.dram_tensor("cosF", [128, S], bf16, kind="ExternalInput")
    sinF = nc.dram_tensor("sinF", [128, S], bf16, kind="ExternalInput")
    triL = nc.dram_tensor("triL", [128, 128], bf16, kind="ExternalInput")
    triU = nc.dram_tensor("triU", [128, 128], bf16, kind="ExternalInput")
    rperm = nc.dram_tensor("rperm", [128, 128], bf16, kind="ExternalInput")
    onesb = nc.dram_tensor("onesb", [128, 1], bf16, kind="ExternalInput")
    onesr = nc.dram_tensor("onesr", [1, 128], bf16, kind="ExternalInput")
    out = nc.dram_tensor("out", [S, D], bf16, kind="ExternalOutput")

    with TileContext(nc) as tc:
        with (
            nc.allow_low_precision(reason="bf16 datapath, fp32 accumulation"),
            tc.tile_pool(name="consts", bufs=1) as consts,
            tc.tile_pool(name="resid", bufs=1) as resid,
        ):
            lt_sb = consts.tile([128, 128], bf16, tag="lt")
            ut_sb = consts.tile([128, 128], bf16, tag="ut")
            ob_sb = consts.tile([128, 1], bf16, tag="ob")
            or_sb = consts.tile([1, 128], bf16, tag="or")
            rp_sb = consts.tile([128, 128], bf16, tag="rp")
            c_sb = consts.tile([128, S], bf16, tag="cos")
            s_sb = consts.tile([128, S], bf16, tag="sin")
            # consts go on the idle Pool queue so they don't block the
            # critical first-chunk loads on the SP queue
            nc.gpsimd.dma_start(rp_sb[:], rperm[:])
            nc.gpsimd.dma_start(lt_sb[:], triL[:])
            nc.gpsimd.dma_start(ut_sb[:], triU[:])
            nc.gpsimd.dma_start(ob_sb[:], onesb[:])
            nc.gpsimd.dma_start(or_sb[:], onesr[:])

            # SBUF-resident Q^T/K^T (e-major per head) and V (t-major)
            qres = resid.tile([128, ET, S], bf16, tag="qres")
            kres = resid.tile([128, ET, S], bf16, tag="kres")
            vres = resid.tile([128, TT, E], bf16, tag="vres")
            wo_sb = resid.tile([128, ET, D], bf16, tag="wo")

            with (
                tc.tile_pool(name="wpool", bufs=1) as wpool,
                tc.tile_pool(name="xpool", bufs=3) as xpool,
                tc.tile_pool(name="stage", bufs=4) as stage,
                tc.tile_pool(name="oTp", bufs=2) as oTp,
                tc.tile_pool(name="ptp", bufs=4) as ptp,
                tc.tile_pool(name="accp", bufs=2) as accp,
                tc.tile_pool(name="rcp", bufs=2) as rcp,
                tc.tile_pool(name="ostp", bufs=2) as ostp,
                # PSUM: proj-acc/rope-swap/outproj ring (3) + scores (2,
                # ACT-paced so isolated) + PV accumulators (2) + softmax
                # sums/broadcast alternating in one bank = 8 banks
                tc.tile_pool(name="psMM", bufs=3, space="PSUM") as psMM,
                tc.tile_pool(name="psC", bufs=2, space="PSUM") as psC,
                tc.tile_pool(name="psD", bufs=2, space="PSUM") as psD,
                tc.tile_pool(name="psAux", bufs=1, space="PSUM") as psAux,
            ):
                wq_sb = wpool.tile([128, ET, KT, 128], bf16, tag="wq")
                wk_sb = wpool.tile([128, ET, KT, 128], bf16, tag="wk")
                wv_sb = wpool.tile([128, KT, E], bf16, tag="wv")

                xc_t = {}
                oT_c = {}
                pre_po = []
                # deferred-emission slots: PE-blocking ops postponed until
                # the engine has other queued work covering their input
                # latency (ACT copy for the rope swap, DVE chain for the
                # softmax normalization)
                pending = {"swap": None, "tail1": None, "tail2": None}

                def flush(key):
                    fn = pending[key]
                    if fn is not None:
                        pending[key] = None
                        fn()

                def emit_chunk_loads(tcb):
                    ts = slice(tcb * 512, (tcb + 1) * 512)
                    xc = xpool.tile([128, KT, 512], bf16, tag="xc")
                    xc_t[tcb] = xc
                    if tcb == 0:
                        # single queue: issue order == transfer order on
                        # the (serialized) DMA path, so strict priority:
                        # first-group inputs, then just-in-time streaming
                        nc.sync.dma_start(wq_sb[:, 0], wqT[:, 0])
                        nc.sync.dma_start(xc[:, 0:1], xT[:, 0:1, ts])
                        nc.sync.dma_start(xc[:, 1:4], xT[:, 1:4, ts])
                        nc.sync.dma_start(xc[:, 4:8], xT[:, 4:8, ts])
                        nc.sync.dma_start(wq_sb[:, 1], wqT[:, 1])
                        nc.sync.dma_start(xc[:, 8:12], xT[:, 8:12, ts])
                        nc.sync.dma_start(wq_sb[:, 2], wqT[:, 2])
                        nc.sync.dma_start(xc[:, 12:16], xT[:, 12:16, ts])
                        nc.sync.dma_start(wq_sb[:, 3], wqT[:, 3])
                        nc.sync.dma_start(c_sb[:], cosF[:])
                        nc.sync.dma_start(s_sb[:], sinF[:])
                        for et in range(ET):
                            nc.sync.dma_start(wk_sb[:, et], wkT[:, et])
                        for kc in range(4):
                            ks = slice(kc * 4, (kc + 1) * 4)
                            nc.sync.dma_start(wv_sb[:, ks], wvT[:, ks])
                    else:
                        for kc in range(4):
                            ks = slice(kc * 4, (kc + 1) * 4)
                            nc.sync.dma_start(xc[:, ks], xT[:, ks, ts])
                    if tcb == 1:
                        for dcc in range(4):
                            dsl = slice(dcc * 512, (dcc + 1) * 512)
                            nc.sync.dma_start(wo_sb[:, :, dsl], woT[:, :, dsl])

                def proj_groups(tcb):
                    # 12 projection matmul groups for one x chunk, as
                    # closures so they can interleave into SDPA segments.
                    # The RoPE pair-swap matmul of each group is deferred
                    # behind the next group so the PE never waits on the
                    # ACT psum copy.
                    ts = slice(tcb * 512, (tcb + 1) * 512)
                    xc = xc_t[tcb]
                    groups = []

                    def mkqk(w_sb, dst, et):
                        def g():
                            pq = psMM.tile([128, 512], f32, tag="mm")
                            for k in range(KT):
                                nc.tensor.matmul(
                                    pq[:], w_sb[:, et, k, :], xc[:, k, :],
                                    start=(k == 0), stop=(k == KT - 1),
                                )
                                if k == 7:
                                    # mid-group: the previous swap's DVE
                                    # chain gets cover before anything
                                    # reads its qres/kres write, and the
                                    # qsb ACT copy has had time to land
                                    flush("swap")
                            qsb = stage.tile([128, 512], bf16, tag="qsb")
                            nc.scalar.copy(qsb[:], pq[:])

                            def f():
                                ps2 = psMM.tile([128, 512], f32, tag="mm")
                                nc.tensor.matmul(ps2[:], rp_sb[:], qsb[:],
                                                 start=True, stop=True)
                                t1 = stage.tile([128, 512], bf16, tag="t1")
                                t2 = stage.tile([128, 512], bf16, tag="t2")
                                nc.vector.tensor_tensor(
                                    t1[:], qsb[:], c_sb[:, ts], mult)
                                nc.vector.tensor_tensor(
                                    t2[:], ps2[:], s_sb[:, ts], mult)
                                nc.vector.tensor_tensor(
                                    dst[:, et, ts], t1[:], t2[:], add)
                            pending["swap"] = f
                        return g

                    def mkv(tt):
                        def g():
                            pv = psMM.tile([128, 512], f32, tag="mm")
                            for k in range(KT):
                                nc.tensor.matmul(
                                    pv[:], xc[:, k, tt * 128:(tt + 1) * 128],
                                    wv_sb[:, k, :],
                                    start=(k == 0), stop=(k == KT - 1),
                                )
                                if k == 7:
                                    flush("swap")
                            nc.scalar.copy(vres[:, 4 * tcb + tt, :], pv[:])
                        return g

                    for (w_sb, dst) in ((wq_sb, qres), (wk_sb, kres)):
                        for et in range(ET):
                            groups.append(mkqk(w_sb, dst, et))
                    for tt in range(4):
                        groups.append(mkv(tt))
                    return groups

                def emit_outproj_tile(tt, oT, eager_dma=False,
                                      copies_on_act=False, alt_queue=False):
                    # one t-tile of the output projection: 4 dc-quanta of 4
                    # matmuls each, returned as callables to interleave
                    tl = tt % 4
                    quanta = []
                    ost = ostp.tile([128, D], bf16, tag="ost")
                    rsl = slice(tt * 128, (tt + 1) * 128)

                    def mk(dc):
                        def q():
                            po = psMM.tile([128, 512], f32, tag="mm")
                            for eh in range(ET):
                                nc.tensor.matmul(
                                    po[:],
                                    oT[:, eh, tl * 128:(tl + 1) * 128],
                                    wo_sb[:, eh, dc * 512:(dc + 1) * 512],
                                    start=(eh == 0), stop=(eh == ET - 1),
                                )
                            dsl = slice(dc * 512, (dc + 1) * 512)
                            if dc < 3 and not copies_on_act:
                                nc.vector.tensor_copy(ost[:, dsl], po[:])
                            else:
                                nc.scalar.copy(ost[:, dsl], po[:])
                            if eager_dma:
                                nc.sync.dma_start(out[rsl, dsl], ost[:, dsl])
                        return q
                    for dc in range(4):
                        quanta.append(mk(dc))

                    def fin():
                        if not eager_dma:
                            nc.sync.dma_start(out[rsl, :], ost[:])
                    return quanta, fin

                def emit_sdpa(ic, gbh):
                    isl = slice(ic * 512, (ic + 1) * 512)
                    oT_ic = oTp.tile([128, ET, 512], bf16, tag="oT")
                    oT_c[ic] = oT_ic
                    for h in range(ET):
                        # deferred output projection work (one ic behind)
                        if ic > 0:
                            quanta, fin = emit_outproj_tile(
                                4 * (ic - 1) + h, oT_c[ic - 1],
                                copies_on_act=(ic == TC - 1 and h == ET - 1))
                        else:
                            quanta, fin = [], None
                        qtb = qres[:, h, isl]
                        ps_pv = psD.tile([128, 512], f32, tag="pv")
                        acc = accp.tile([128, 512], bf16, tag="acc")
                        # j-tiles: full below the diagonal block, then the
                        # 4 staircase tiles (i-extent shrinks by 128 each)
                        tiles = [(jt, 0) for jt in range(4 * ic)]
                        tiles += [(4 * ic + r, 128 * r) for r in range(4)]
                        n = len(tiles)
                        pts = [None] * n
                        LOOK = 3
                        quanta_done = [0]

                        def emit_pv(idx):
                            jt, ilo = tiles[idx]
                            nc.tensor.matmul(
                                ps_pv[:, ilo:512],
                                vres[:, jt, h * 128:(h + 1) * 128],
                                pts[idx][:, ilo:512],
                                start=(idx == 0), stop=(idx == n - 1),
                                skip_group_check=True,
                            )

                        for idx, (jt, ilo) in enumerate(tiles):
                            ps_sc = psC.tile([128, 512], f32, tag="sc")
                            nc.tensor.matmul(
                                ps_sc[:, ilo:512],
                                kres[:, h, jt * 128:(jt + 1) * 128],
                                qtb[:, ilo:512],
                                start=True, stop=True,
                            )
                            if jt >= 4 * ic:
                                # boundary-diagonal piece: accumulate
                                # -1e5*max(0, j-i) so exp gives exact zeros
                                nc.tensor.matmul(
                                    ps_sc[:, ilo:ilo + 128],
                                    lt_sb[:], ut_sb[:],
                                    start=False, stop=True,
                                    skip_group_check=True,
                                )
                            pt = ptp.tile([128, 512], bf16, tag="pt")
                            pts[idx] = pt
                            nc.scalar.activation(
                                pt[:, ilo:512], ps_sc[:, ilo:512], Exp,
                                scale=SCALE)
                            if idx == 0:
                                nc.vector.tensor_copy(acc[:], pt[:])
                            else:
                                nc.vector.scalar_tensor_tensor(
                                    acc[:, ilo:512], pt[:, ilo:512], 1.0,
                                    acc[:, ilo:512], mult, add)
                            if h == 0 and idx == 0:
                                # h0 quanta read every head slice of the
                                # previous oT - the pending tail must land
                                # before the first quantum
                                flush("tail1")
                                flush("tail2")
                            elif idx == 1:
                                flush("tail1")
                            elif idx == 2:
                                flush("swap")
                            elif idx == (4 if n > 4 else 3):
                                flush("tail2")
                            st = max(1, n // 4)
                            if quanta and idx % st == 0 and idx // st < 4:
                                quanta[idx // st]()
                                quanta_done[0] = idx // st + 1
                            if idx >= LOOK:
                                emit_pv(idx - LOOK)
                        for qi in range(quanta_done[0], len(quanta)):
                            quanta[qi]()
                        for idx in range(max(0, n - LOOK), n):
                            emit_pv(idx)
                        if fin is not None:
                            fin()

                        if ic == TC - 1 and h == ET - 1:
                            # pre-start five tail-outproj quanta with the
                            # three already-normalized heads, borrowing
                            # the now-idle scores and PV psum banks; the
                            # eh3 finishes land right after the last norm.
                            # This fills the sums->recip->broadcast->mult
                            # chain window with real PE work.
                            specs = [(psMM, "mm"), (psMM, "mm"),
                                     (psC, "sc"), (psC, "sc"), (psD, "pv")]
                            for qn, (pool, tg) in enumerate(specs):
                                ptt = qn // 4          # 0 -> tt12, 1 -> tt13
                                tl = ptt               # tt % 4
                                dc = qn % 4
                                po = pool.tile([128, 512], f32, tag=tg,
                                               name=f"pre{qn}")
                                for eh in range(3):
                                    nc.tensor.matmul(
                                        po[:],
                                        oT_ic[:, eh, tl * 128:(tl + 1) * 128],
                                        wo_sb[:, eh, dc * 512:(dc + 1) * 512],
                                        start=(eh == 0), stop=False,
                                    )
                                pre_po.append(po)

                        # normalization tail, deferred into the next head's
                        # tile loop: sums -> 1/sums, then broadcast -> mult
                        last = (ic == TC - 1 and h == ET - 1)

                        def mktails(acc=acc, ps_pv=ps_pv, oT_ic=oT_ic, h=h,
                                    split=last):
                            rc = rcp.tile([1, 512], bf16, tag="rc")

                            def t1():
                                ps_sums = psAux.tile([128, 512], f32,
                                                     tag="aux")
                                nc.tensor.matmul(ps_sums[0:1, :], ob_sb[:],
                                                 acc[:], start=True,
                                                 stop=True)
                                nc.vector.reciprocal(rc[:], ps_sums[0:1, :])

                            def t2():
                                rcb = rcp.tile([128, 512], bf16, tag="rcb")
                                pb = psAux.tile([128, 512], f32, tag="aux")
                                nc.tensor.matmul(pb[:], or_sb[:], rc[:],
                                                 start=True, stop=True)
                                if split:
                                    # 128-col pieces so the dependent
                                    # output projection can start after
                                    # the first piece lands
                                    for tl in range(4):
                                        tls = slice(tl * 128, (tl + 1) * 128)
                                        nc.vector.tensor_copy(
                                            rcb[:, tls], pb[:, tls])
                                        nc.vector.tensor_tensor(
                                            oT_ic[:, h, tls], ps_pv[:, tls],
                                            rcb[:, tls], mult)
                                else:
                                    nc.vector.tensor_copy(rcb[:], pb[:])
                                    nc.vector.tensor_tensor(
                                        oT_ic[:, h, :], ps_pv[:], rcb[:],
                                        mult)
                            return t1, t2
                        flush("tail1")
                        flush("tail2")
                        pending["tail1"], pending["tail2"] = mktails()

                        # next chunk's projection groups: pure PE work
                        # that fills the exp-paced bubbles, with the
                        # normalization tail flushed in between
                        for gi, g in enumerate(gbh[h]):
                            g()
                            if gi == 0:
                                flush("tail1")
                            elif gi == 1:
                                flush("tail2")

                # interleave projection chunks with SDPA i-chunks so ACT's
                # exp stream never throttles the tensor engine.  chunk 3's
                # Q(h)/K(h) groups pipeline INTO sdpa(3) between heads
                # (each head only needs its own q/k slices), keeping the
                # final - otherwise exp-bound - segment fed with PE work.
                emit_chunk_loads(0)
                # chunk-0 Q groups in DMA-arrival wavefront order: the
                # matmuls of et0-2 interleave by k-slice arrival so the PE
                # streams from the first weight landing instead of
                # serializing whole (DMA-gated) groups
                g0 = proj_groups(0)
                wave_pq = {}

                def wv_mm(et, k0, k1, close=False):
                    if et not in wave_pq:
                        wave_pq[et] = psMM.tile([128, 512], f32, tag="mm",
                                                name=f"wavepq{et}")
                    pq = wave_pq[et]
                    for k in range(k0, k1):
                        nc.tensor.matmul(
                            pq[:], wq_sb[:, et, k, :], xc_t[0][:, k, :],
                            start=(k == 0), stop=(close and k == k1 - 1),
                        )
                    if close:
                        qsb = stage.tile([128, 512], bf16, tag="qsb")
                        nc.scalar.copy(qsb[:], pq[:])

                        def f(qsb=qsb, et=et):
                            ps2 = psMM.tile([128, 512], f32, tag="mm")
                            nc.tensor.matmul(ps2[:], rp_sb[:], qsb[:],
                                             start=True, stop=True)
                            t1 = stage.tile([128, 512], bf16, tag="t1")
                            t2 = stage.tile([128, 512], bf16, tag="t2")
                            nc.vector.tensor_tensor(
                                t1[:], qsb[:], c_sb[:, 0:512], mult)
                            nc.vector.tensor_tensor(
                                t2[:], ps2[:], s_sb[:, 0:512], mult)
                            nc.vector.tensor_tensor(
                                qres[:, et, 0:512], t1[:], t2[:], add)
                        flush("swap")
                        pending["swap"] = f

                wv_mm(0, 0, 8)
                wv_mm(1, 0, 8)
                wv_mm(0, 8, 12)
                wv_mm(1, 8, 12)
                wv_mm(2, 0, 12)
                wv_mm(0, 12, 16, close=True)
                wv_mm(1, 12, 16, close=True)
                wv_mm(2, 12, 16, close=True)
                wv_mm(3, 0, 16, close=True)
                for g in g0[4:12]:
                    g()
                emit_chunk_loads(1)
                emit_chunk_loads(2)
                g1 = proj_groups(1)
                emit_sdpa(0, [g1[0:3], g1[3:6], g1[6:9], g1[9:12]])
                emit_chunk_loads(3)
                g2 = proj_groups(2)
                emit_sdpa(1, [g2[0:3], g2[3:6], g2[6:9], g2[9:12]])
                g3 = proj_groups(3)   # Q0-3 = g3[0:4], K0-3 = g3[4:8], V0-3 = g3[8:12]
                emit_sdpa(2, [[g3[8], g3[9]], [g3[10], g3[11]],
                              [g3[0]], [g3[4]]])
                emit_sdpa(3, [[g3[1], g3[5]], [g3[2], g3[6]],
                              [g3[3], g3[7]], []])

                # tail: output projection for the last i-chunk; the first
                # five quanta were pre-started with eh0-2 and only need
                # the eh3 matmul here
                flush("tail1")
                flush("tail2")
                oT3 = oT_c[TC - 1]
                for h in range(ET):
                    tt = 4 * (TC - 1) + h
                    tl = tt % 4
                    ost = ostp.tile([128, D], bf16, tag="ost",
                                    name=f"osttail{h}")
                    rsl = slice(tt * 128, (tt + 1) * 128)
                    for dc in range(4):
                        qn = h * 4 + dc
                        dsl = slice(dc * 512, (dc + 1) * 512)
                        if qn < len(pre_po):
                            po = pre_po[qn]
                            nc.tensor.matmul(
                                po[:], oT3[:, 3, tl * 128:(tl + 1) * 128],
                                wo_sb[:, 3, dsl], start=False, stop=True)
                        else:
                            po = psMM.tile([128, 512], f32, tag="mm",
                                           name=f"tl{qn}")
                            for eh in range(ET):
                                nc.tensor.matmul(
                                    po[:],
                                    oT3[:, eh, tl * 128:(tl + 1) * 128],
                                    wo_sb[:, eh, dsl],
                                    start=(eh == 0), stop=(eh == ET - 1),
                                )
                        if dc < 3:
                            nc.vector.tensor_copy(ost[:, dsl], po[:])
                        else:
                            nc.scalar.copy(ost[:, dsl], po[:])
                        nc.sync.dma_start(out[rsl, dsl], ost[:, dsl])

    _split_waits(nc)
    return nc


_NC = None


def _get_nc():
    global _NC
    if _NC is None:
        _NC = _build_nc()
    return _NC


# ---------------------------------------------------------------------------
# Host-side prep + gather
# ---------------------------------------------------------------------------

def _rope_tables():
    j = np.arange(0, HD, 2, dtype=np.float32) / HD
    inv_freq = (1.0 / (ROPE_BASE ** j)).astype(np.float32)           # [64]
    t = np.arange(S, dtype=np.float32)
    freqs = np.outer(inv_freq, t)                                    # [64, S]
    cos = np.cos(freqs)
    sin = np.sin(freqs)
    cosF = np.empty((128, S), dtype=np.float32)
    sinF = np.empty((128, S), dtype=np.float32)
    cosF[0::2] = cos
    cosF[1::2] = cos
    sinF[0::2] = -sin
    sinF[1::2] = sin
    return cosF.astype(npbf16), sinF.astype(npbf16)


def _static_inputs():
    cosF, sinF = _rope_tables()
    k = np.arange(128)
    triL = (k[:, None] < k[None, :]).astype(np.float32)      # [k, j] k<j
    triU = np.where(k[:, None] >= k[None, :], -1e5, 0.0)     # [k, i] k>=i
    onesb = np.ones((128, 1), dtype=np.float32)
    onesr = np.ones((1, 128), dtype=np.float32)
    rperm = np.zeros((128, 128), dtype=np.float32)
    idx = np.arange(128)
    rperm[idx ^ 1, idx] = 1.0
    return {
        "cosF": cosF, "sinF": sinF,
        "triL": triL.astype(npbf16), "triU": triU.astype(npbf16),
        "onesb": onesb.astype(npbf16), "onesr": onesr.astype(npbf16),
        "rperm": rperm.astype(npbf16),
    }


def _core_inputs(x, wqk, wv, wo, static, b, g):
    xb = np.ascontiguousarray(x[b].T)                                # [D, S]
    xT = np.ascontiguousarray(
        xb.reshape(KT, 128, S).transpose(1, 0, 2)).astype(npbf16)

    wq_g = wqk[E * g:E * (g + 1), :]                                 # [E, D]
    wk_g = wqk[D + E * g:D + E * (g + 1), :]
    wv_g = wv[E * g:E * (g + 1), :]
    wqT = np.ascontiguousarray(
        wq_g.T.reshape(KT, 128, ET, 128).transpose(1, 2, 0, 3)).astype(npbf16)
    wkT = np.ascontiguousarray(
        wk_g.T.reshape(KT, 128, ET, 128).transpose(1, 2, 0, 3)).astype(npbf16)
    wvT = np.ascontiguousarray(
        wv_g.T.reshape(KT, 128, E).transpose(1, 0, 2)).astype(npbf16)
    woT = np.ascontiguousarray(
        wo[:, E * g:E * (g + 1)].T.reshape(ET, 128, D).transpose(1, 0, 2)
    ).astype(npbf16)

    m = dict(static)
    m.update({"xT": xT, "wqT": wqT, "wkT": wkT, "wvT": wvT, "woT": woT})
    return m


def kernel(x, wqk, wv, wo):
    x = np.asarray(x, dtype=np.float32)
    wqk = np.asarray(wqk, dtype=np.float32)
    wv = np.asarray(wv, dtype=np.float32)
    wo = np.asarray(wo, dtype=np.float32)

    nc = _get_nc()
    static = _static_inputs()
    in_maps = [
        _core_inputs(x, wqk, wv, wo, static, c // G, c % G) for c in range(8)
    ]
    res = run_bass_kernel_spmd(nc, in_maps, core_ids=list(range(8)))
    out = np.zeros((B, S, D), dtype=np.float32)
    for c in range(8):
        out[c // G] += res.results[c]["out"].astype(np.float32)
    return out


# revision 109
# speedup vs baseline: 1.2999x; 1.0108x over previous
"""Trainium2 Bass kernel for nn_Attention_43301860278871.

Full attention layer: fused QK projection + V projection, interleaved RoPE,
causal SDPA, output projection.  B=2, S=2048, D=2048, H=16, HD=128.

Sharding: 8 cores = 2 batches x 4 head-groups (tensor parallel over heads,
data parallel over batch).  Each core computes 4 heads for one batch and a
partial [S, D] output-projection contribution; the host sums the 4 partials
per batch, so no on-device collectives are needed.

Design (vs the f32r/DRAM-staging baseline at 402us):
  - All matmul operands in bf16 (host-converted; PSUM accumulation stays
    f32), halving DMA and letting Q/K/V stay SBUF-resident for the whole
    kernel - no DRAM round-trip, no phase barrier.  rel err ~8e-3 vs the
    2e-2 gate.
  - RoPE pair-swap via a bf16 permutation matmul (DVE lanes are
    partition-locked, so the swap cannot run there), deferred behind the
    next projection group so the PE never waits on the ACT psum copy.
  - Causal masking at true 128-granularity: per-j-tile scores matmuls
    with trimmed i-extents; the boundary-diagonal 128x128 piece gets a
    second matmul accumulating -1e5 * max(0, j-i) into the scores PSUM
    (lower-tri x upper-tri constant operands), so exp() produces exact
    zeros and no mask multiply exists anywhere on the critical path.
  - Softmax row sums: bf16 exp tiles accumulated over j-tiles on DVE
    (scalar_tensor_tensor, 4x mode) and reduced with ONE ones-column
    matmul per (head, i-chunk) instead of one per j-tile (the baseline
    spent 34us of PE streaming ones-matmuls).  1/sums broadcast via a
    ones-row matmul; normalization tails split in two stages and emitted
    inside the NEXT head's tile loop so their cross-engine latency hides
    behind queued PE work.
  - Global software pipeline: projection chunks interleave with SDPA
    i-chunks (12 proj groups spread over the preceding segment's heads;
    chunk 3's Q(h)/K(h) groups pipeline INTO sdpa(3) between heads), and
    the output projection runs one i-chunk behind SDPA in 4-matmul quanta
    spread across each head's score tiles - so the exp() stream on ACT
    (~87us) never throttles the tensor engine.
  - PSUM: 8 banks as proj/swap/outproj ring (3) + scores (2, the only
    ACT-paced pool, isolated) + PV accumulators (2) + sums/broadcast (1).
  - Single-queue DMA with strict priority order (the DMA path serializes
    at ~350GB/s, so issue order is everything): wq-et0 first, then x
    k-slices interleaved with remaining weight et-slices; chunk-0 Q
    matmuls emitted in DMA-arrival wavefront order.

TimelineSim / HW exec: ~313us/core (PE ~94% busy, ~295us of a 290us
matmul-stream floor at 1 row/cycle); rel err 7.8e-3.
"""
import sys
sys.path.insert(0, '/opt/trn_rl_repo')

import numpy as np
import ml_dtypes

import concourse.bass as bass
import concourse.mybir as mybir
from concourse.bass_utils import run_bass_kernel_spmd
from concourse.tile import TileContext

B, S, D, H = 2, 2048, 2048, 16
HD = D // H            # 128
G = 4                  # head-groups (cores per batch)
HPG = H // G           # heads per core = 4
E = HPG * HD           # per-core projection width = 512
ROPE_BASE = 10000.0
SCALE = float(HD) ** -0.5

f32 = mybir.dt.float32
bf16 = mybir.dt.bfloat16
npbf16 = ml_dtypes.bfloat16

KT = D // 128          # 16 contraction tiles
TT = S // 128          # 16 token tiles
TC = S // 512          # 4 token chunks
ET = E // 128          # 4 e-tiles (= heads per core)

Exp = mybir.ActivationFunctionType.Exp
mult = mybir.AluOpType.mult
add = mybir.AluOpType.add


# ---------------------------------------------------------------------------
# Workarounds for this walrus build: at most ONE sem wait per instruction.
# Tile's scheduler attaches several; hoist the excess onto NoOps injected on
# the same engine immediately before (sequencer executes waits in order, so
# semantics are identical).
# ---------------------------------------------------------------------------

def _patched_drain_and_barrier(self, tick_clock, wait_clock):
    from concourse.vector_clock import ScopedClock
    drain_inst = self.nc.sync.drain()
    wait_clock.add_sem_waits(
        drain_inst.ins, ScopedClock({None: tick_clock.global_clock})
    )
    si = drain_inst.ins.sync_info
    if si is not None and si.on_wait and len(si.on_wait) > 1:
        waits = list(si.on_wait)
        si.on_wait = waits[:1]
        for w in waits[1:]:
            extra = self.nc.sync.drain()
            esi = extra.ins.sync_info
            if esi is None:
                extra.ins.sync_info = mybir.SyncInfo(on_wait=[w], on_update=[])
            else:
                esi.on_wait = [w]

    self.nc.all_engine_barrier()
    assert self.sems is not None
    popped = self.nc._tile_sem_poison_stack.pop()
    assert popped is self._sem_poison
    self.nc.clear_and_free_semaphores(list(self.sems.allocated().values()))
    self.nc.all_engine_barrier()


def _install_tile_patch():
    import concourse.tile as tile_mod
    tile_mod.TileContext._drain_and_barrier = _patched_drain_and_barrier


def _split_waits(nc, max_waits: int = 1):
    for fn in nc.m.functions:
        for bb in fn.blocks:
            out = []
            changed = False
            for inst in list(bb.instructions):
                si = inst.sync_info
                if si is not None and si.on_wait and len(si.on_wait) > max_waits:
                    waits = list(si.on_wait)
                    for w in waits[:-max_waits]:
                        out.append(mybir.InstNoOp(
                            name=nc.get_next_instruction_name(),
                            engine=inst.engine,
                            sync_info=mybir.SyncInfo(on_wait=[w], on_update=[]),
                        ))
                    si.on_wait = waits[-max_waits:]
                    changed = True
                out.append(inst)
            if changed:
                bb.instructions = out


# ---------------------------------------------------------------------------
# Kernel build (one Bass module, SPMD across the 8 cores via input slices)
# ---------------------------------------------------------------------------

def _build_nc():
    _install_tile_patch()
    nc = bass.Bass()

    xT = nc.dram_tensor("xT", [128, KT, S], bf16, kind="ExternalInput")
    wqT = nc.dram_tensor("wqT", [128, ET, KT, 128], bf16, kind="ExternalInput")
    wkT = nc.dram_tensor("wkT", [128, ET, KT, 128], bf16, kind="ExternalInput")
    wvT = nc.dram_tensor("wvT", [128, KT, E], bf16, kind="ExternalInput")
    woT = nc.dram_tensor("woT", [128, ET, D], bf16, kind="ExternalInput")
    cosF = nc.dram_tensor("cosF", [128, S], bf16, kind="ExternalInput")
    sinF = nc.dram_tensor("sinF", [128, S], bf16, kind="ExternalInput")
    triL = nc.dram_tensor("triL", [128, 128], bf16, kind="ExternalInput")
    triU = nc.dram_tensor("triU", [128, 128], bf16, kind="ExternalInput")
    rperm = nc.dram_tensor("rperm", [128, 128], bf16, kind="ExternalInput")
    onesq = nc.dram_tensor("onesq", [128, 128], bf16, kind="ExternalInput")
    out = nc.dram_tensor("out", [S, D], bf16, kind="ExternalOutput")

    with TileContext(nc) as tc:
        with (
            nc.allow_low_precision(reason="bf16 datapath, fp32 accumulation"),
            tc.tile_pool(name="consts", bufs=1) as consts,
            tc.tile_pool(name="resid", bufs=1) as resid,
        ):
            lt_sb = consts.tile([128, 128], bf16, tag="lt")
            ut_sb = consts.tile([128, 128], bf16, tag="ut")
            oq_sb = consts.tile([128, 128], bf16, tag="oq")
            rp_sb = consts.tile([128, 128], bf16, tag="rp")
            c_sb = consts.tile([128, S], bf16, tag="cos")
            s_sb = consts.tile([128, S], bf16, tag="sin")
            # consts go on the idle Pool queue so they don't block the
            # critical first-chunk loads on the SP queue
            nc.gpsimd.dma_start(rp_sb[:], rperm[:])
            nc.gpsimd.dma_start(lt_sb[:], triL[:])
            nc.gpsimd.dma_start(ut_sb[:], triU[:])
            nc.gpsimd.dma_start(oq_sb[:], onesq[:])

            # SBUF-resident Q^T/K^T (e-major per head) and V (t-major)
            qres = resid.tile([128, ET, S], bf16, tag="qres")
            kres = resid.tile([128, ET, S], bf16, tag="kres")
            vres = resid.tile([128, TT, E], bf16, tag="vres")
            wo_sb = resid.tile([128, ET, D], bf16, tag="wo")

            with (
                tc.tile_pool(name="wpool", bufs=1) as wpool,
                tc.tile_pool(name="xpool", bufs=3) as xpool,
                tc.tile_pool(name="stage", bufs=4) as stage,
                tc.tile_pool(name="oTp", bufs=2) as oTp,
                tc.tile_pool(name="ptp", bufs=4) as ptp,
                tc.tile_pool(name="accp", bufs=2) as accp,
                tc.tile_pool(name="rcp", bufs=2) as rcp,
                tc.tile_pool(name="ostp", bufs=2) as ostp,
                # PSUM: proj-acc/rope-swap/outproj ring (3) + scores (2,
                # ACT-paced so isolated) + PV accumulators (2) + softmax
                # sums/broadcast alternating in one bank = 8 banks
                tc.tile_pool(name="psMM", bufs=3, space="PSUM") as psMM,
                tc.tile_pool(name="psC", bufs=2, space="PSUM") as psC,
                tc.tile_pool(name="psD", bufs=2, space="PSUM") as psD,
                tc.tile_pool(name="psAux", bufs=1, space="PSUM") as psAux,
            ):
                wq_sb = wpool.tile([128, ET, KT, 128], bf16, tag="wq")
                wk_sb = wpool.tile([128, ET, KT, 128], bf16, tag="wk")
                wv_sb = wpool.tile([128, KT, E], bf16, tag="wv")

                xc_t = {}
                oT_c = {}
                pre_po = []
                # deferred-emission slots: PE-blocking ops postponed until
                # the engine has other queued work covering their input
                # latency (ACT copy for the rope swap, DVE chain for the
                # softmax normalization)
                pending = {"swap": None, "tail1": None, "tail2": None}

                def flush(key):
                    fn = pending[key]
                    if fn is not None:
                        pending[key] = None
                        fn()

                def emit_chunk_loads(tcb):
                    ts = slice(tcb * 512, (tcb + 1) * 512)
                    xc = xpool.tile([128, KT, 512], bf16, tag="xc")
                    xc_t[tcb] = xc
                    if tcb == 0:
                        # single queue: issue order == transfer order on
                        # the (serialized) DMA path, so strict priority:
                        # first-group inputs, then just-in-time streaming
                        nc.sync.dma_start(wq_sb[:, 0], wqT[:, 0])
                        nc.sync.dma_start(xc[:, 0:1], xT[:, 0:1, ts])
                        nc.sync.dma_start(xc[:, 1:4], xT[:, 1:4, ts])
                        nc.sync.dma_start(xc[:, 4:8], xT[:, 4:8, ts])
                        nc.sync.dma_start(wq_sb[:, 1], wqT[:, 1])
                        nc.sync.dma_start(xc[:, 8:12], xT[:, 8:12, ts])
                        nc.sync.dma_start(wq_sb[:, 2], wqT[:, 2])
                        nc.sync.dma_start(xc[:, 12:16], xT[:, 12:16, ts])
                        nc.sync.dma_start(wq_sb[:, 3], wqT[:, 3])
                        nc.sync.dma_start(c_sb[:], cosF[:])
                        nc.sync.dma_start(s_sb[:], sinF[:])
                        for et in range(ET):
                            nc.sync.dma_start(wk_sb[:, et], wkT[:, et])
                        for kc in range(4):
                            ks = slice(kc * 4, (kc + 1) * 4)
                            nc.sync.dma_start(wv_sb[:, ks], wvT[:, ks])
                    else:
                        for kc in range(4):
                            ks = slice(kc * 4, (kc + 1) * 4)
                            nc.sync.dma_start(xc[:, ks], xT[:, ks, ts])
                    if tcb == 1:
                        for dcc in range(4):
                            dsl = slice(dcc * 512, (dcc + 1) * 512)
                            nc.sync.dma_start(wo_sb[:, :, dsl], woT[:, :, dsl])

                def proj_groups(tcb):
                    # 12 projection matmul groups for one x chunk, as
                    # closures so they can interleave into SDPA segments.
                    # The RoPE pair-swap matmul of each group is deferred
                    # behind the next group so the PE never waits on the
                    # ACT psum copy.
                    ts = slice(tcb * 512, (tcb + 1) * 512)
                    xc = xc_t[tcb]
                    groups = []

                    def mkqk(w_sb, dst, et):
                        def g():
                            pq = psMM.tile([128, 512], f32, tag="mm")
                            for k in range(KT):
                                nc.tensor.matmul(
                                    pq[:], w_sb[:, et, k, :], xc[:, k, :],
                                    start=(k == 0), stop=(k == KT - 1),
                                )
                                if k == 7:
                                    # mid-group: the previous swap's DVE
                                    # chain gets cover before anything
                                    # reads its qres/kres write, and the
                                    # qsb ACT copy has had time to land
                                    flush("swap")
                            qsb = stage.tile([128, 512], bf16, tag="qsb")
                            nc.scalar.copy(qsb[:], pq[:])

                            def f():
                                ps2 = psMM.tile([128, 512], f32, tag="mm")
                                nc.tensor.matmul(ps2[:], rp_sb[:], qsb[:],
                                                 start=True, stop=True)
                                t1 = stage.tile([128, 512], bf16, tag="t1")
                                t2 = stage.tile([128, 512], bf16, tag="t2")
                                nc.vector.tensor_tensor(
                                    t1[:], qsb[:], c_sb[:, ts], mult)
                                nc.vector.tensor_tensor(
                                    t2[:], ps2[:], s_sb[:, ts], mult)
                                nc.vector.tensor_tensor(
                                    dst[:, et, ts], t1[:], t2[:], add)
                            pending["swap"] = f
                        return g

                    def mkv(tt):
                        def g():
                            pv = psMM.tile([128, 512], f32, tag="mm")
                            for k in range(KT):
                                nc.tensor.matmul(
                                    pv[:], xc[:, k, tt * 128:(tt + 1) * 128],
                                    wv_sb[:, k, :],
                                    start=(k == 0), stop=(k == KT - 1),
                                )
                                if k == 7:
                                    flush("swap")
                            nc.scalar.copy(vres[:, 4 * tcb + tt, :], pv[:])
                        return g

                    for (w_sb, dst) in ((wq_sb, qres), (wk_sb, kres)):
                        for et in range(ET):
                            groups.append(mkqk(w_sb, dst, et))
                    for tt in range(4):
                        groups.append(mkv(tt))
                    return groups

                def emit_outproj_tile(tt, oT, eager_dma=False,
                                      copies_on_act=False, alt_queue=False):
                    # one t-tile of the output projection: 4 dc-quanta of 4
                    # matmuls each, returned as callables to interleave
                    tl = tt % 4
                    quanta = []
                    ost = ostp.tile([128, D], bf16, tag="ost")
                    rsl = slice(tt * 128, (tt + 1) * 128)

                    def mk(dc):
                        def q():
                            po = psMM.tile([128, 512], f32, tag="mm")
                            for eh in range(ET):
                                nc.tensor.matmul(
                                    po[:],
                                    oT[:, eh, tl * 128:(tl + 1) * 128],
                                    wo_sb[:, eh, dc * 512:(dc + 1) * 512],
                                    start=(eh == 0), stop=(eh == ET - 1),
                                )
                            dsl = slice(dc * 512, (dc + 1) * 512)
                            if dc < 3 and not copies_on_act:
                                nc.vector.tensor_copy(ost[:, dsl], po[:])
                            else:
                                nc.scalar.copy(ost[:, dsl], po[:])
                            if eager_dma:
                                nc.sync.dma_start(out[rsl, dsl], ost[:, dsl])
                        return q
                    for dc in range(4):
                        quanta.append(mk(dc))

                    def fin():
                        if not eager_dma:
                            nc.sync.dma_start(out[rsl, :], ost[:])
                    return quanta, fin

                def emit_sdpa(ic, gbh):
                    isl = slice(ic * 512, (ic + 1) * 512)
                    oT_ic = oTp.tile([128, ET, 512], bf16, tag="oT")
                    oT_c[ic] = oT_ic
                    for h in range(ET):
                        # deferred output projection work (one ic behind)
                        if ic > 0:
                            quanta, fin = emit_outproj_tile(
                                4 * (ic - 1) + h, oT_c[ic - 1],
                                copies_on_act=(ic == TC - 1 and h == ET - 1))
                        else:
                            quanta, fin = [], None
                        qtb = qres[:, h, isl]
                        ps_pv = psD.tile([128, 512], f32, tag="pv")
                        acc = accp.tile([128, 512], bf16, tag="acc")
                        # j-tiles: full below the diagonal block, then the
                        # 4 staircase tiles (i-extent shrinks by 128 each)
                        tiles = [(jt, 0) for jt in range(4 * ic)]
                        tiles += [(4 * ic + r, 128 * r) for r in range(4)]
                        n = len(tiles)
                        pts = [None] * n
                        LOOK = 3
                        quanta_done = [0]

                        def emit_pv(idx):
                            jt, ilo = tiles[idx]
                            nc.tensor.matmul(
                                ps_pv[:, ilo:512],
                                vres[:, jt, h * 128:(h + 1) * 128],
                                pts[idx][:, ilo:512],
                                start=(idx == 0), stop=(idx == n - 1),
                                skip_group_check=True,
                            )

                        for idx, (jt, ilo) in enumerate(tiles):
                            ps_sc = psC.tile([128, 512], f32, tag="sc")
                            nc.tensor.matmul(
                                ps_sc[:, ilo:512],
                                kres[:, h, jt * 128:(jt + 1) * 128],
                                qtb[:, ilo:512],
                                start=True, stop=True,
                            )
                            if jt >= 4 * ic:
                                # boundary-diagonal piece: accumulate
                                # -1e5*max(0, j-i) so exp gives exact zeros
                                nc.tensor.matmul(
                                    ps_sc[:, ilo:ilo + 128],
                                    lt_sb[:], ut_sb[:],
                                    start=False, stop=True,
                                    skip_group_check=True,
                                )
                            pt = ptp.tile([128, 512], bf16, tag="pt")
                            pts[idx] = pt
                            nc.scalar.activation(
                                pt[:, ilo:512], ps_sc[:, ilo:512], Exp,
                                scale=SCALE)
                            if idx == 0:
                                nc.vector.tensor_copy(acc[:], pt[:])
                            else:
                                nc.vector.scalar_tensor_tensor(
                                    acc[:, ilo:512], pt[:, ilo:512], 1.0,
                                    acc[:, ilo:512], mult, add)
                            if h == 0 and idx == 0:
                                # h0 quanta read every head slice of the
                                # previous oT - the pending tail must land
                                # before the first quantum
                                flush("tail1")
                                flush("tail2")
                            elif idx == 1:
                                flush("tail1")
                            elif idx == 2:
                                flush("swap")
                            elif idx == (4 if n > 4 else 3):
                                flush("tail2")
                            st = max(1, n // 4)
                            if quanta and idx % st == 0 and idx // st < 4:
                                quanta[idx // st]()
                                quanta_done[0] = idx // st + 1
                            if idx >= LOOK:
                                emit_pv(idx - LOOK)
                        for qi in range(quanta_done[0], len(quanta)):
                            quanta[qi]()
                        for idx in range(max(0, n - LOOK), n):
                            emit_pv(idx)
                        if fin is not None:
                            fin()

                        if ic == TC - 1 and h == ET - 1:
                            # pre-start five tail-outproj quanta with the
                            # three already-normalized heads, borrowing
                            # the now-idle scores and PV psum banks; the
                            # eh3 finishes land right after the last norm.
                            # This fills the sums->recip->broadcast->mult
                            # chain window with real PE work.
                            specs = [(psMM, "mm"), (psMM, "mm"),
                                     (psC, "sc"), (psC, "sc"), (psD, "pv")]
                            for qn, (pool, tg) in enumerate(specs):
                                ptt = qn // 4          # 0 -> tt12, 1 -> tt13
                                tl = ptt               # tt % 4
                                dc = qn % 4
                                po = pool.tile([128, 512], f32, tag=tg,
                                               name=f"pre{qn}")
                                for eh in range(3):
                                    nc.tensor.matmul(
                                        po[:],
                                        oT_ic[:, eh, tl * 128:(tl + 1) * 128],
                                        wo_sb[:, eh, dc * 512:(dc + 1) * 512],
                                        start=(eh == 0), stop=False,
                                    )
                                pre_po.append(po)

                        # normalization tail, deferred into the next head's
                        # tile loop: sums -> 1/sums, then broadcast -> mult
                        last = (ic == TC - 1 and h == ET - 1)

                        def mktails(acc=acc, ps_pv=ps_pv, oT_ic=oT_ic, h=h,
                                    split=last):
                            rcb = rcp.tile([128, 512], bf16, tag="rcb")

                            def t1():
                                # all-ones [128,128] stationary: the sums
                                # reduction lands already broadcast across
                                # every partition (same streamed-row cost)
                                ps_sums = psAux.tile([128, 512], f32,
                                                     tag="aux")
                                nc.tensor.matmul(ps_sums[:], oq_sb[:],
                                                 acc[:], start=True,
                                                 stop=True)
                                if split:
                                    for tl in range(4):
                                        tls = slice(tl * 128, (tl + 1) * 128)
                                        nc.vector.reciprocal(
                                            rcb[:, tls], ps_sums[:, tls])
                                else:
                                    nc.vector.reciprocal(rcb[:], ps_sums[:])

                            def t2():
                                if split:
                                    # 128-col pieces so the dependent
                                    # output projection can start after
                                    # the first piece lands
                                    for tl in range(4):
                                        tls = slice(tl * 128, (tl + 1) * 128)
                                        nc.vector.tensor_tensor(
                                            oT_ic[:, h, tls], ps_pv[:, tls],
                                            rcb[:, tls], mult)
                                else:
                                    nc.vector.tensor_tensor(
                                        oT_ic[:, h, :], ps_pv[:], rcb[:],
                                        mult)
                            return t1, t2
                        flush("tail1")
                        flush("tail2")
                        pending["tail1"], pending["tail2"] = mktails()

                        # next chunk's projection groups: pure PE work
                        # that fills the exp-paced bubbles, with the
                        # normalization tail flushed in between
                        for gi, g in enumerate(gbh[h]):
                            g()
                            if gi == 0:
                                flush("tail1")
                            elif gi == 1:
                                flush("tail2")

                # interleave projection chunks with SDPA i-chunks so ACT's
                # exp stream never throttles the tensor engine.  chunk 3's
                # Q(h)/K(h) groups pipeline INTO sdpa(3) between heads
                # (each head only needs its own q/k slices), keeping the
                # final - otherwise exp-bound - segment fed with PE work.
                emit_chunk_loads(0)
                # chunk-0 Q groups in DMA-arrival wavefront order: the
                # matmuls of et0-2 interleave by k-slice arrival so the PE
                # streams from the first weight landing instead of
                # serializing whole (DMA-gated) groups
                g0 = proj_groups(0)
                wave_pq = {}

                def wv_mm(et, k0, k1, close=False):
                    if et not in wave_pq:
                        wave_pq[et] = psMM.tile([128, 512], f32, tag="mm",
                                                name=f"wavepq{et}")
                    pq = wave_pq[et]
                    for k in range(k0, k1):
                        nc.tensor.matmul(
                            pq[:], wq_sb[:, et, k, :], xc_t[0][:, k, :],
                            start=(k == 0), stop=(close and k == k1 - 1),
                        )
                    if close:
                        qsb = stage.tile([128, 512], bf16, tag="qsb")
                        nc.scalar.copy(qsb[:], pq[:])

                        def f(qsb=qsb, et=et):
                            ps2 = psMM.tile([128, 512], f32, tag="mm")
                            nc.tensor.matmul(ps2[:], rp_sb[:], qsb[:],
                                             start=True, stop=True)
                            t1 = stage.tile([128, 512], bf16, tag="t1")
                            t2 = stage.tile([128, 512], bf16, tag="t2")
                            nc.vector.tensor_tensor(
                                t1[:], qsb[:], c_sb[:, 0:512], mult)
                            nc.vector.tensor_tensor(
                                t2[:], ps2[:], s_sb[:, 0:512], mult)
                            nc.vector.tensor_tensor(
                                qres[:, et, 0:512], t1[:], t2[:], add)
                        flush("swap")
                        pending["swap"] = f

                wv_mm(0, 0, 8)
                wv_mm(1, 0, 8)
                wv_mm(0, 8, 12)
                wv_mm(1, 8, 12)
                wv_mm(2, 0, 12)
                wv_mm(0, 12, 16, close=True)
                wv_mm(1, 12, 16, close=True)
                wv_mm(2, 12, 16, close=True)
                wv_mm(3, 0, 16, close=True)
                for g in g0[4:12]:
                    g()
                emit_chunk_loads(1)
                emit_chunk_loads(2)
                g1 = proj_groups(1)
                emit_sdpa(0, [g1[0:3], g1[3:6], g1[6:9], g1[9:12]])
                emit_chunk_loads(3)
                g2 = proj_groups(2)
                emit_sdpa(1, [g2[0:3], g2[3:6], g2[6:9], g2[9:12]])
                g3 = proj_groups(3)   # Q0-3 = g3[0:4], K0-3 = g3[4:8], V0-3 = g3[8:12]
                emit_sdpa(2, [[g3[8], g3[9]], [g3[10], g3[11]],
                              [g3[0]], [g3[4]]])
                emit_sdpa(3, [[g3[1], g3[5]], [g3[2], g3[6]],
                              [g3[3], g3[7]], []])

                # tail: output projection for the last i-chunk; the first
                # five quanta were pre-started with eh0-2 and only need
                # the eh3 matmul here
                flush("tail1")
                flush("tail2")
                oT3 = oT_c[TC - 1]
                for h in range(ET):
                    tt = 4 * (TC - 1) + h
                    tl = tt % 4
                    ost = ostp.tile([128, D], bf16, tag="ost",
                                    name=f"osttail{h}")
                    rsl = slice(tt * 128, (tt + 1) * 128)
                    for dc in range(4):
                        qn = h * 4 + dc
                        dsl = slice(dc * 512, (dc + 1) * 512)
                        if qn < len(pre_po):
                            po = pre_po[qn]
                            nc.tensor.matmul(
                                po[:], oT3[:, 3, tl * 128:(tl + 1) * 128],
                                wo_sb[:, 3, dsl], start=False, stop=True)
                        else:
                            po = psMM.tile([128, 512], f32, tag="mm",
                                           name=f"tl{qn}")
                            for eh in range(ET):
                                nc.tensor.matmul(
                                    po[:],
                                    oT3[:, eh, tl * 128:(tl + 1) * 128],
                                    wo_sb[:, eh, dsl],
                                    start=(eh == 0), stop=(eh == ET - 1),
                                )
                        if dc < 3:
                            nc.vector.tensor_copy(ost[:, dsl], po[:])
                        else:
                            nc.scalar.copy(ost[:, dsl], po[:])
                        nc.sync.dma_start(out[rsl, dsl], ost[:, dsl])

    _split_waits(nc)
    return nc


_NC = None


def _get_nc():
    global _NC
    if _NC is None:
        _NC = _build_nc()
    return _NC


# ---------------------------------------------------------------------------
# Host-side prep + gather
# ---------------------------------------------------------------------------

def _rope_tables():
    j = np.arange(0, HD, 2, dtype=np.float32) / HD
    inv_freq = (1.0 / (ROPE_BASE ** j)).astype(np.float32)           # [64]
    t = np.arange(S, dtype=np.float32)
    freqs = np.outer(inv_freq, t)                                    # [64, S]
    cos = np.cos(freqs)
    sin = np.sin(freqs)
    cosF = np.empty((128, S), dtype=np.float32)
    sinF = np.empty((128, S), dtype=np.float32)
    cosF[0::2] = cos
    cosF[1::2] = cos
    sinF[0::2] = -sin
    sinF[1::2] = sin
    return cosF.astype(npbf16), sinF.astype(npbf16)


def _static_inputs():
    cosF, sinF = _rope_tables()
    k = np.arange(128)
    triL = (k[:, None] < k[None, :]).astype(np.float32)      # [k, j] k<j
    triU = np.where(k[:, None] >= k[None, :], -1e5, 0.0)     # [k, i] k>=i
    onesq = np.ones((128, 128), dtype=np.float32)
    rperm = np.zeros((128, 128), dtype=np.float32)
    idx = np.arange(128)
    rperm[idx ^ 1, idx] = 1.0
    return {
        "cosF": cosF, "sinF": sinF,
        "triL": triL.astype(npbf16), "triU": triU.astype(npbf16),
        "onesq": onesq.astype(npbf16), "rperm": rperm.astype(npbf16),
    }


def _core_inputs(x, wqk, wv, wo, static, b, g):
    xb = np.ascontiguousarray(x[b].T)                                # [D, S]
    xT = np.ascontiguousarray(
        xb.reshape(KT, 128, S).transpose(1, 0, 2)).astype(npbf16)

    wq_g = wqk[E * g:E * (g + 1), :]                                 # [E, D]
    wk_g = wqk[D + E * g:D + E * (g + 1), :]
    wv_g = wv[E * g:E * (g + 1), :]
    wqT = np.ascontiguousarray(
        wq_g.T.reshape(KT, 128, ET, 128).transpose(1, 2, 0, 3)).astype(npbf16)
    wkT = np.ascontiguousarray(
        wk_g.T.reshape(KT, 128, ET, 128).transpose(1, 2, 0, 3)).astype(npbf16)
    wvT = np.ascontiguousarray(
        wv_g.T.reshape(KT, 128, E).transpose(1, 0, 2)).astype(npbf16)
    woT = np.ascontiguousarray(
        wo[:, E * g:E * (g + 1)].T.reshape(ET, 128, D).transpose(1, 0, 2)
    ).astype(npbf16)

    m = dict(static)
    m.update({"xT": xT, "wqT": wqT, "wkT": wkT, "wvT": wvT, "woT": woT})
    return m


def kernel(x, wqk, wv, wo):
    x = np.asarray(x, dtype=np.float32)
    wqk = np.asarray(wqk, dtype=np.float32)
    wv = np.asarray(wv, dtype=np.float32)
    wo = np.asarray(wo, dtype=np.float32)

    nc = _get_nc()
    static = _static_inputs()
    in_maps = [
        _core_inputs(x, wqk, wv, wo, static, c // G, c % G) for c in range(8)
    ]
    res = run_bass_kernel_spmd(nc, in_maps, core_ids=list(range(8)))
    out = np.zeros((B, S, D), dtype=np.float32)
    for c in range(8):
        out[c // G] += res.results[c]["out"].astype(np.float32)
    return out


# revision 118
# speedup vs baseline: 1.3037x; 1.0029x over previous
"""Trainium2 Bass kernel for nn_Attention_43301860278871.

Full attention layer: fused QK projection + V projection, interleaved RoPE,
causal SDPA, output projection.  B=2, S=2048, D=2048, H=16, HD=128.

Sharding: 8 cores = 2 batches x 4 head-groups (tensor parallel over heads,
data parallel over batch).  Each core computes 4 heads for one batch and a
partial [S, D] output-projection contribution; the host sums the 4 partials
per batch, so no on-device collectives are needed.

Design (vs the f32r/DRAM-staging baseline at 402us):
  - All matmul operands in bf16 (host-converted; PSUM accumulation stays
    f32), halving DMA and letting Q/K/V stay SBUF-resident for the whole
    kernel - no DRAM round-trip, no phase barrier.  rel err ~8e-3 vs the
    2e-2 gate.
  - RoPE pair-swap via a bf16 permutation matmul (DVE lanes are
    partition-locked, so the swap cannot run there), deferred behind the
    next projection group so the PE never waits on the ACT psum copy.
  - Causal masking at true 128-granularity: per-j-tile scores matmuls
    with trimmed i-extents; the boundary-diagonal 128x128 piece gets a
    second matmul accumulating -1e5 * max(0, j-i) into the scores PSUM
    (lower-tri x upper-tri constant operands), so exp() produces exact
    zeros and no mask multiply exists anywhere on the critical path.
  - Softmax row sums: bf16 exp tiles accumulated over j-tiles on DVE
    (scalar_tensor_tensor, 4x mode) and reduced with ONE ones-column
    matmul per (head, i-chunk) instead of one per j-tile (the baseline
    spent 34us of PE streaming ones-matmuls).  The sums matmul uses an
    all-ones [128,128] stationary so the reduction lands already
    broadcast across every partition (matmul cost depends only on
    streamed rows) - no separate broadcast matmul or copy; normalization
    tails split in two stages and emitted inside the NEXT head's tile
    loop so their cross-engine latency hides behind queued PE work.
  - Global software pipeline: projection chunks interleave with SDPA
    i-chunks (12 proj groups spread over the preceding segment's heads;
    chunk 3's Q(h)/K(h) groups pipeline INTO sdpa(3) between heads), and
    the output projection runs one i-chunk behind SDPA in 4-matmul quanta
    spread across each head's score tiles - so the exp() stream on ACT
    (~87us) never throttles the tensor engine.
  - PSUM: 8 banks as proj/swap/outproj ring (3) + scores (2, the only
    ACT-paced pool, isolated) + PV accumulators (2) + sums/broadcast (1).
  - Single-queue DMA with strict priority order (the DMA path serializes
    at ~350GB/s, so issue order is everything): wq-et0 first, then x
    k-slices interleaved with remaining weight et-slices; chunk-0 Q
    matmuls emitted in DMA-arrival wavefront order.

TimelineSim / HW exec: ~309.5us/core (PE ~94% busy against a ~287us
matmul-stream floor at 1 row/cycle); rel err 7.8e-3.
"""
import sys
sys.path.insert(0, '/opt/trn_rl_repo')

import numpy as np
import ml_dtypes

import concourse.bass as bass
import concourse.mybir as mybir
from concourse.bass_utils import run_bass_kernel_spmd
from concourse.tile import TileContext

B, S, D, H = 2, 2048, 2048, 16
HD = D // H            # 128
G = 4                  # head-groups (cores per batch)
HPG = H // G           # heads per core = 4
E = HPG * HD           # per-core projection width = 512
ROPE_BASE = 10000.0
SCALE = float(HD) ** -0.5

f32 = mybir.dt.float32
bf16 = mybir.dt.bfloat16
npbf16 = ml_dtypes.bfloat16

KT = D // 128          # 16 contraction tiles
TT = S // 128          # 16 token tiles
TC = S // 512          # 4 token chunks
ET = E // 128          # 4 e-tiles (= heads per core)

Exp = mybir.ActivationFunctionType.Exp
mult = mybir.AluOpType.mult
add = mybir.AluOpType.add


# ---------------------------------------------------------------------------
# Workarounds for this walrus build: at most ONE sem wait per instruction.
# Tile's scheduler attaches several; hoist the excess onto NoOps injected on
# the same engine immediately before (sequencer executes waits in order, so
# semantics are identical).
# ---------------------------------------------------------------------------

def _patched_drain_and_barrier(self, tick_clock, wait_clock):
    from concourse.vector_clock import ScopedClock
    drain_inst = self.nc.sync.drain()
    wait_clock.add_sem_waits(
        drain_inst.ins, ScopedClock({None: tick_clock.global_clock})
    )
    si = drain_inst.ins.sync_info
    if si is not None and si.on_wait and len(si.on_wait) > 1:
        waits = list(si.on_wait)
        si.on_wait = waits[:1]
        for w in waits[1:]:
            extra = self.nc.sync.drain()
            esi = extra.ins.sync_info
            if esi is None:
                extra.ins.sync_info = mybir.SyncInfo(on_wait=[w], on_update=[])
            else:
                esi.on_wait = [w]

    self.nc.all_engine_barrier()
    assert self.sems is not None
    popped = self.nc._tile_sem_poison_stack.pop()
    assert popped is self._sem_poison
    self.nc.clear_and_free_semaphores(list(self.sems.allocated().values()))
    self.nc.all_engine_barrier()


def _install_tile_patch():
    import concourse.tile as tile_mod
    tile_mod.TileContext._drain_and_barrier = _patched_drain_and_barrier


def _split_waits(nc, max_waits: int = 1):
    for fn in nc.m.functions:
        for bb in fn.blocks:
            out = []
            changed = False
            for inst in list(bb.instructions):
                si = inst.sync_info
                if si is not None and si.on_wait and len(si.on_wait) > max_waits:
                    waits = list(si.on_wait)
                    for w in waits[:-max_waits]:
                        out.append(mybir.InstNoOp(
                            name=nc.get_next_instruction_name(),
                            engine=inst.engine,
                            sync_info=mybir.SyncInfo(on_wait=[w], on_update=[]),
                        ))
                    si.on_wait = waits[-max_waits:]
                    changed = True
                out.append(inst)
            if changed:
                bb.instructions = out


# ---------------------------------------------------------------------------
# Kernel build (one Bass module, SPMD across the 8 cores via input slices)
# ---------------------------------------------------------------------------

def _build_nc():
    _install_tile_patch()
    nc = bass.Bass()

    xT = nc.dram_tensor("xT", [128, KT, S], bf16, kind="ExternalInput")
    wqT = nc.dram_tensor("wqT", [128, ET, KT, 128], bf16, kind="ExternalInput")
    wkT = nc.dram_tensor("wkT", [128, ET, KT, 128], bf16, kind="ExternalInput")
    wvT = nc.dram_tensor("wvT", [128, KT, E], bf16, kind="ExternalInput")
    woT = nc.dram_tensor("woT", [128, ET, D], bf16, kind="ExternalInput")
    cosF = nc.dram_tensor("cosF", [128, S], bf16, kind="ExternalInput")
    sinF = nc.dram_tensor("sinF", [128, S], bf16, kind="ExternalInput")
    triL = nc.dram_tensor("triL", [128, 128], bf16, kind="ExternalInput")
    triU = nc.dram_tensor("triU", [128, 128], bf16, kind="ExternalInput")
    rperm = nc.dram_tensor("rperm", [128, 128], bf16, kind="ExternalInput")
    onesq = nc.dram_tensor("onesq", [128, 128], bf16, kind="ExternalInput")
    out = nc.dram_tensor("out", [S, D], bf16, kind="ExternalOutput")

    with TileContext(nc) as tc:
        with (
            nc.allow_low_precision(reason="bf16 datapath, fp32 accumulation"),
            tc.tile_pool(name="consts", bufs=1) as consts,
            tc.tile_pool(name="resid", bufs=1) as resid,
        ):
            lt_sb = consts.tile([128, 128], bf16, tag="lt")
            ut_sb = consts.tile([128, 128], bf16, tag="ut")
            oq_sb = consts.tile([128, 128], bf16, tag="oq")
            rp_sb = consts.tile([128, 128], bf16, tag="rp")
            c_sb = consts.tile([128, S], bf16, tag="cos")
            s_sb = consts.tile([128, S], bf16, tag="sin")
            # consts go on the idle Pool queue so they don't block the
            # critical first-chunk loads on the SP queue
            nc.gpsimd.dma_start(rp_sb[:], rperm[:])
            nc.gpsimd.dma_start(lt_sb[:], triL[:])
            nc.gpsimd.dma_start(ut_sb[:], triU[:])
            nc.gpsimd.dma_start(oq_sb[:], onesq[:])

            # SBUF-resident Q^T/K^T (e-major per head) and V (t-major)
            qres = resid.tile([128, ET, S], bf16, tag="qres")
            kres = resid.tile([128, ET, S], bf16, tag="kres")
            vres = resid.tile([128, TT, E], bf16, tag="vres")
            wo_sb = resid.tile([128, ET, D], bf16, tag="wo")

            with (
                tc.tile_pool(name="wpool", bufs=1) as wpool,
                tc.tile_pool(name="xpool", bufs=3) as xpool,
                tc.tile_pool(name="stage", bufs=4) as stage,
                tc.tile_pool(name="oTp", bufs=2) as oTp,
                tc.tile_pool(name="ptp", bufs=4) as ptp,
                tc.tile_pool(name="accp", bufs=2) as accp,
                tc.tile_pool(name="rcp", bufs=2) as rcp,
                tc.tile_pool(name="ostp", bufs=2) as ostp,
                # PSUM: proj-acc/rope-swap/outproj ring (3) + scores (2,
                # ACT-paced so isolated) + PV accumulators (2) + softmax
                # sums/broadcast alternating in one bank = 8 banks
                tc.tile_pool(name="psMM", bufs=3, space="PSUM") as psMM,
                tc.tile_pool(name="psC", bufs=2, space="PSUM") as psC,
                tc.tile_pool(name="psD", bufs=2, space="PSUM") as psD,
                tc.tile_pool(name="psAux", bufs=1, space="PSUM") as psAux,
            ):
                wq_sb = wpool.tile([128, ET, KT, 128], bf16, tag="wq")
                wk_sb = wpool.tile([128, ET, KT, 128], bf16, tag="wk")
                wv_sb = wpool.tile([128, KT, E], bf16, tag="wv")

                xc_t = {}
                oT_c = {}
                pre_po = []
                # deferred-emission slots: PE-blocking ops postponed until
                # the engine has other queued work covering their input
                # latency (ACT copy for the rope swap, DVE chain for the
                # softmax normalization)
                pending = {"swap": None, "tail1": None, "tail2": None}

                def flush(key):
                    fn = pending[key]
                    if fn is not None:
                        pending[key] = None
                        fn()

                def emit_chunk_loads(tcb):
                    ts = slice(tcb * 512, (tcb + 1) * 512)
                    xc = xpool.tile([128, KT, 512], bf16, tag="xc")
                    xc_t[tcb] = xc
                    if tcb == 0:
                        # single queue: issue order == transfer order on
                        # the (serialized) DMA path, so strict priority:
                        # first-group inputs, then just-in-time streaming
                        nc.sync.dma_start(wq_sb[:, 0], wqT[:, 0])
                        nc.sync.dma_start(xc[:, 0:1], xT[:, 0:1, ts])
                        nc.sync.dma_start(xc[:, 1:4], xT[:, 1:4, ts])
                        nc.sync.dma_start(xc[:, 4:8], xT[:, 4:8, ts])
                        nc.sync.dma_start(wq_sb[:, 1], wqT[:, 1])
                        nc.sync.dma_start(wq_sb[:, 2], wqT[:, 2])
                        nc.sync.dma_start(wq_sb[:, 3], wqT[:, 3])
                        nc.sync.dma_start(xc[:, 8:12], xT[:, 8:12, ts])
                        nc.sync.dma_start(xc[:, 12:16], xT[:, 12:16, ts])
                        nc.sync.dma_start(c_sb[:], cosF[:])
                        nc.sync.dma_start(s_sb[:], sinF[:])
                        for et in range(ET):
                            nc.sync.dma_start(wk_sb[:, et], wkT[:, et])
                        for kc in range(4):
                            ks = slice(kc * 4, (kc + 1) * 4)
                            nc.sync.dma_start(wv_sb[:, ks], wvT[:, ks])
                    else:
                        for kc in range(4):
                            ks = slice(kc * 4, (kc + 1) * 4)
                            nc.sync.dma_start(xc[:, ks], xT[:, ks, ts])
                    if tcb == 1:
                        for dcc in range(4):
                            dsl = slice(dcc * 512, (dcc + 1) * 512)
                            nc.sync.dma_start(wo_sb[:, :, dsl], woT[:, :, dsl])

                def proj_groups(tcb):
                    # 12 projection matmul groups for one x chunk, as
                    # closures so they can interleave into SDPA segments.
                    # The RoPE pair-swap matmul of each group is deferred
                    # behind the next group so the PE never waits on the
                    # ACT psum copy.
                    ts = slice(tcb * 512, (tcb + 1) * 512)
                    xc = xc_t[tcb]
                    groups = []

                    def mkqk(w_sb, dst, et):
                        def g():
                            pq = psMM.tile([128, 512], f32, tag="mm")
                            for k in range(KT):
                                nc.tensor.matmul(
                                    pq[:], w_sb[:, et, k, :], xc[:, k, :],
                                    start=(k == 0), stop=(k == KT - 1),
                                )
                                if k == 7:
                                    # mid-group: the previous swap's DVE
                                    # chain gets cover before anything
                                    # reads its qres/kres write, and the
                                    # qsb ACT copy has had time to land
                                    flush("swap")
                            qsb = stage.tile([128, 512], bf16, tag="qsb")
                            nc.vector.tensor_copy(qsb[:], pq[:])

                            def f():
                                ps2 = psMM.tile([128, 512], f32, tag="mm")
                                nc.tensor.matmul(ps2[:], rp_sb[:], qsb[:],
                                                 start=True, stop=True)
                                t1 = stage.tile([128, 512], bf16, tag="t1")
                                t2 = stage.tile([128, 512], bf16, tag="t2")
                                nc.vector.tensor_tensor(
                                    t1[:], qsb[:], c_sb[:, ts], mult)
                                nc.vector.tensor_tensor(
                                    t2[:], ps2[:], s_sb[:, ts], mult)
                                nc.vector.tensor_tensor(
                                    dst[:, et, ts], t1[:], t2[:], add)
                            pending["swap"] = f
                        return g

                    def mkv(tt):
                        def g():
                            pv = psMM.tile([128, 512], f32, tag="mm")
                            for k in range(KT):
                                nc.tensor.matmul(
                                    pv[:], xc[:, k, tt * 128:(tt + 1) * 128],
                                    wv_sb[:, k, :],
                                    start=(k == 0), stop=(k == KT - 1),
                                )
                                if k == 7:
                                    flush("swap")
                            nc.scalar.copy(vres[:, 4 * tcb + tt, :], pv[:])
                        return g

                    for (w_sb, dst) in ((wq_sb, qres), (wk_sb, kres)):
                        for et in range(ET):
                            groups.append(mkqk(w_sb, dst, et))
                    for tt in range(4):
                        groups.append(mkv(tt))
                    return groups

                def emit_outproj_tile(tt, oT, eager_dma=False,
                                      copies_on_act=False, alt_queue=False):
                    # one t-tile of the output projection: 4 dc-quanta of 4
                    # matmuls each, returned as callables to interleave
                    tl = tt % 4
                    quanta = []
                    ost = ostp.tile([128, D], bf16, tag="ost")
                    rsl = slice(tt * 128, (tt + 1) * 128)

                    def mk(dc):
                        def q():
                            po = psMM.tile([128, 512], f32, tag="mm")
                            for eh in range(ET):
                                nc.tensor.matmul(
                                    po[:],
                                    oT[:, eh, tl * 128:(tl + 1) * 128],
                                    wo_sb[:, eh, dc * 512:(dc + 1) * 512],
                                    start=(eh == 0), stop=(eh == ET - 1),
                                )
                            dsl = slice(dc * 512, (dc + 1) * 512)
                            if dc < 3 and not copies_on_act:
                                nc.vector.tensor_copy(ost[:, dsl], po[:])
                            else:
                                nc.scalar.copy(ost[:, dsl], po[:])
                            if eager_dma:
                                nc.sync.dma_start(out[rsl, dsl], ost[:, dsl])
                        return q
                    for dc in range(4):
                        quanta.append(mk(dc))

                    def fin():
                        if not eager_dma:
                            nc.sync.dma_start(out[rsl, :], ost[:])
                    return quanta, fin

                def emit_sdpa(ic, gbh):
                    isl = slice(ic * 512, (ic + 1) * 512)
                    oT_ic = oTp.tile([128, ET, 512], bf16, tag="oT")
                    oT_c[ic] = oT_ic
                    for h in range(ET):
                        # deferred output projection work (one ic behind)
                        if ic > 0:
                            quanta, fin = emit_outproj_tile(
                                4 * (ic - 1) + h, oT_c[ic - 1],
                                copies_on_act=(ic == TC - 1 and h == ET - 1))
                        else:
                            quanta, fin = [], None
                        qtb = qres[:, h, isl]
                        ps_pv = psD.tile([128, 512], f32, tag="pv")
                        acc = accp.tile([128, 512], bf16, tag="acc")
                        # j-tiles: full below the diagonal block, then the
                        # 4 staircase tiles (i-extent shrinks by 128 each)
                        tiles = [(jt, 0) for jt in range(4 * ic)]
                        tiles += [(4 * ic + r, 128 * r) for r in range(4)]
                        n = len(tiles)
                        pts = [None] * n
                        LOOK = 3
                        quanta_done = [0]

                        def emit_pv(idx):
                            jt, ilo = tiles[idx]
                            nc.tensor.matmul(
                                ps_pv[:, ilo:512],
                                vres[:, jt, h * 128:(h + 1) * 128],
                                pts[idx][:, ilo:512],
                                start=(idx == 0), stop=(idx == n - 1),
                                skip_group_check=True,
                            )

                        for idx, (jt, ilo) in enumerate(tiles):
                            ps_sc = psC.tile([128, 512], f32, tag="sc")
                            nc.tensor.matmul(
                                ps_sc[:, ilo:512],
                                kres[:, h, jt * 128:(jt + 1) * 128],
                                qtb[:, ilo:512],
                                start=True, stop=True,
                            )
                            if jt >= 4 * ic:
                                # boundary-diagonal piece: accumulate
                                # -1e5*max(0, j-i) so exp gives exact zeros
                                nc.tensor.matmul(
                                    ps_sc[:, ilo:ilo + 128],
                                    lt_sb[:], ut_sb[:],
                                    start=False, stop=True,
                                    skip_group_check=True,
                                )
                            pt = ptp.tile([128, 512], bf16, tag="pt")
                            pts[idx] = pt
                            nc.scalar.activation(
                                pt[:, ilo:512], ps_sc[:, ilo:512], Exp,
                                scale=SCALE)
                            if idx == 0:
                                nc.vector.tensor_copy(acc[:], pt[:])
                            else:
                                nc.vector.scalar_tensor_tensor(
                                    acc[:, ilo:512], pt[:, ilo:512], 1.0,
                                    acc[:, ilo:512], mult, add)
                            if h == 0 and idx == 0:
                                # h0 quanta read every head slice of the
                                # previous oT - the pending tail must land
                                # before the first quantum
                                flush("tail1")
                                flush("tail2")
                            elif idx == 1:
                                flush("tail1")
                            elif idx == 2:
                                flush("swap")
                            elif idx == (4 if n > 4 else 3):
                                flush("tail2")
                            st = max(1, n // 4)
                            if quanta and idx % st == 0 and idx // st < 4:
                                quanta[idx // st]()
                                quanta_done[0] = idx // st + 1
                            if idx >= LOOK:
                                emit_pv(idx - LOOK)
                        for qi in range(quanta_done[0], len(quanta)):
                            quanta[qi]()
                        for idx in range(max(0, n - LOOK), n):
                            emit_pv(idx)
                        if fin is not None:
                            fin()

                        if ic == TC - 1 and h == ET - 1:
                            # pre-start five tail-outproj quanta with the
                            # three already-normalized heads, borrowing
                            # the now-idle scores and PV psum banks; the
                            # eh3 finishes land right after the last norm.
                            # This fills the sums->recip->broadcast->mult
                            # chain window with real PE work.
                            specs = [(psMM, "mm"), (psMM, "mm"),
                                     (psC, "sc"), (psC, "sc"), (psD, "pv")]
                            for qn, (pool, tg) in enumerate(specs):
                                ptt = qn // 4          # 0 -> tt12, 1 -> tt13
                                tl = ptt               # tt % 4
                                dc = qn % 4
                                po = pool.tile([128, 512], f32, tag=tg,
                                               name=f"pre{qn}")
                                for eh in range(3):
                                    nc.tensor.matmul(
                                        po[:],
                                        oT_ic[:, eh, tl * 128:(tl + 1) * 128],
                                        wo_sb[:, eh, dc * 512:(dc + 1) * 512],
                                        start=(eh == 0), stop=False,
                                    )
                                pre_po.append(po)

                        # normalization tail, deferred into the next head's
                        # tile loop: sums -> 1/sums, then broadcast -> mult
                        last = (ic == TC - 1 and h == ET - 1)

                        def mktails(acc=acc, ps_pv=ps_pv, oT_ic=oT_ic, h=h,
                                    split=last):
                            rcb = rcp.tile([128, 512], bf16, tag="rcb")

                            def t1():
                                # all-ones [128,128] stationary: the sums
                                # reduction lands already broadcast across
                                # every partition (same streamed-row cost)
                                ps_sums = psAux.tile([128, 512], f32,
                                                     tag="aux")
                                nc.tensor.matmul(ps_sums[:], oq_sb[:],
                                                 acc[:], start=True,
                                                 stop=True)
                                nc.vector.reciprocal(rcb[:], ps_sums[:])

                            def t2():
                                if split:
                                    # 128-col pieces so the dependent
                                    # output projection can start after
                                    # the first piece lands
                                    for tl in range(4):
                                        tls = slice(tl * 128, (tl + 1) * 128)
                                        nc.vector.tensor_tensor(
                                            oT_ic[:, h, tls], ps_pv[:, tls],
                                            rcb[:, tls], mult)
                                else:
                                    nc.vector.tensor_tensor(
                                        oT_ic[:, h, :], ps_pv[:], rcb[:],
                                        mult)
                            return t1, t2
                        flush("tail1")
                        flush("tail2")
                        pending["tail1"], pending["tail2"] = mktails()

                        # next chunk's projection groups: pure PE work
                        # that fills the exp-paced bubbles, with the
                        # normalization tail flushed in between
                        for gi, g in enumerate(gbh[h]):
                            g()
                            if gi == 0:
                                flush("tail1")
                            elif gi == 1:
                                flush("tail2")

                # interleave projection chunks with SDPA i-chunks so ACT's
                # exp stream never throttles the tensor engine.  chunk 3's
                # Q(h)/K(h) groups pipeline INTO sdpa(3) between heads
                # (each head only needs its own q/k slices), keeping the
                # final - otherwise exp-bound - segment fed with PE work.
                emit_chunk_loads(0)
                # chunk-0 Q groups in DMA-arrival wavefront order: the
                # matmuls of et0-2 interleave by k-slice arrival so the PE
                # streams from the first weight landing instead of
                # serializing whole (DMA-gated) groups
                g0 = proj_groups(0)
                wave_pq = {}

                def wv_mm(et, k0, k1, close=False):
                    if et not in wave_pq:
                        # et2/et3 borrow the (still idle) scores banks so
                        # all four groups can be open concurrently
                        pool, tg = (psMM, "mm") if et < 2 else (psC, "sc")
                        wave_pq[et] = pool.tile([128, 512], f32, tag=tg,
                                                name=f"wavepq{et}")
                    pq = wave_pq[et]
                    for k in range(k0, k1):
                        nc.tensor.matmul(
                            pq[:], wq_sb[:, et, k, :], xc_t[0][:, k, :],
                            start=(k == 0), stop=(close and k == k1 - 1),
                        )
                    if close:
                        qsb = stage.tile([128, 512], bf16, tag="qsb")
                        nc.scalar.copy(qsb[:], pq[:])

                        def f(qsb=qsb, et=et):
                            ps2 = psMM.tile([128, 512], f32, tag="mm")
                            nc.tensor.matmul(ps2[:], rp_sb[:], qsb[:],
                                             start=True, stop=True)
                            t1 = stage.tile([128, 512], bf16, tag="t1")
                            t2 = stage.tile([128, 512], bf16, tag="t2")
                            nc.vector.tensor_tensor(
                                t1[:], qsb[:], c_sb[:, 0:512], mult)
                            nc.vector.tensor_tensor(
                                t2[:], ps2[:], s_sb[:, 0:512], mult)
                            nc.vector.tensor_tensor(
                                qres[:, et, 0:512], t1[:], t2[:], add)
                        flush("swap")
                        pending["swap"] = f

                wv_mm(0, 0, 4)
                wv_mm(0, 4, 8)
                wv_mm(1, 0, 8)
                wv_mm(2, 0, 8)
                wv_mm(3, 0, 8)
                wv_mm(0, 8, 12)
                wv_mm(1, 8, 12)
                wv_mm(2, 8, 12)
                wv_mm(3, 8, 12)
                wv_mm(0, 12, 16, close=True)
                wv_mm(1, 12, 16, close=True)
                wv_mm(2, 12, 16, close=True)
                wv_mm(3, 12, 16, close=True)
                for g in g0[4:12]:
                    g()
                emit_chunk_loads(1)
                emit_chunk_loads(2)
                g1 = proj_groups(1)
                emit_sdpa(0, [g1[0:3], g1[3:6], g1[6:9], g1[9:12]])
                emit_chunk_loads(3)
                g2 = proj_groups(2)
                emit_sdpa(1, [g2[0:3], g2[3:6], g2[6:9], g2[9:12]])
                g3 = proj_groups(3)   # Q0-3 = g3[0:4], K0-3 = g3[4:8], V0-3 = g3[8:12]
                emit_sdpa(2, [[g3[8], g3[9]], [g3[10], g3[11]],
                              [g3[0]], [g3[4]]])
                emit_sdpa(3, [[g3[1], g3[5]], [g3[2], g3[6]],
                              [g3[3], g3[7]], []])

                # tail: output projection for the last i-chunk; the first
                # five quanta were pre-started with eh0-2 and only need
                # the eh3 matmul here
                flush("tail1")
                flush("tail2")
                oT3 = oT_c[TC - 1]
                for h in range(ET):
                    tt = 4 * (TC - 1) + h
                    tl = tt % 4
                    ost = ostp.tile([128, D], bf16, tag="ost",
                                    name=f"osttail{h}")
                    rsl = slice(tt * 128, (tt + 1) * 128)
                    for dc in range(4):
                        qn = h * 4 + dc
                        dsl = slice(dc * 512, (dc + 1) * 512)
                        if qn < len(pre_po):
                            po = pre_po[qn]
                            nc.tensor.matmul(
                                po[:], oT3[:, 3, tl * 128:(tl + 1) * 128],
                                wo_sb[:, 3, dsl], start=False, stop=True)
                        else:
                            po = psMM.tile([128, 512], f32, tag="mm",
                                           name=f"tl{qn}")
                            for eh in range(ET):
                                nc.tensor.matmul(
                                    po[:],
                                    oT3[:, eh, tl * 128:(tl + 1) * 128],
                                    wo_sb[:, eh, dsl],
                                    start=(eh == 0), stop=(eh == ET - 1),
                                )
                        if dc % 2 == 0:
                            nc.vector.tensor_copy(ost[:, dsl], po[:])
                        else:
                            nc.scalar.copy(ost[:, dsl], po[:])
                        nc.sync.dma_start(out[rsl, dsl], ost[:, dsl])

    _split_waits(nc)
    return nc


_NC = None


def _get_nc():
    global _NC
    if _NC is None:
        _NC = _build_nc()
    return _NC


# ---------------------------------------------------------------------------
# Host-side prep + gather
# ---------------------------------------------------------------------------

def _rope_tables():
    j = np.arange(0, HD, 2, dtype=np.float32) / HD
    inv_freq = (1.0 / (ROPE_BASE ** j)).astype(np.float32)           # [64]
    t = np.arange(S, dtype=np.float32)
    freqs = np.outer(inv_freq, t)                                    # [64, S]
    cos = np.cos(freqs)
    sin = np.sin(freqs)
    cosF = np.empty((128, S), dtype=np.float32)
    sinF = np.empty((128, S), dtype=np.float32)
    cosF[0::2] = cos
    cosF[1::2] = cos
    sinF[0::2] = -sin
    sinF[1::2] = sin
    return cosF.astype(npbf16), sinF.astype(npbf16)


def _static_inputs():
    cosF, sinF = _rope_tables()
    k = np.arange(128)
    triL = (k[:, None] < k[None, :]).astype(np.float32)      # [k, j] k<j
    triU = np.where(k[:, None] >= k[None, :], -1e5, 0.0)     # [k, i] k>=i
    onesq = np.ones((128, 128), dtype=np.float32)
    rperm = np.zeros((128, 128), dtype=np.float32)
    idx = np.arange(128)
    rperm[idx ^ 1, idx] = 1.0
    return {
        "cosF": cosF, "sinF": sinF,
        "triL": triL.astype(npbf16), "triU": triU.astype(npbf16),
        "onesq": onesq.astype(npbf16), "rperm": rperm.astype(npbf16),
    }


def _core_inputs(x, wqk, wv, wo, static, b, g):
    xb = np.ascontiguousarray(x[b].T)                                # [D, S]
    xT = np.ascontiguousarray(
        xb.reshape(KT, 128, S).transpose(1, 0, 2)).astype(npbf16)

    wq_g = wqk[E * g:E * (g + 1), :]                                 # [E, D]
    wk_g = wqk[D + E * g:D + E * (g + 1), :]
    wv_g = wv[E * g:E * (g + 1), :]
    wqT = np.ascontiguousarray(
        wq_g.T.reshape(KT, 128, ET, 128).transpose(1, 2, 0, 3)).astype(npbf16)
    wkT = np.ascontiguousarray(
        wk_g.T.reshape(KT, 128, ET, 128).transpose(1, 2, 0, 3)).astype(npbf16)
    wvT = np.ascontiguousarray(
        wv_g.T.reshape(KT, 128, E).transpose(1, 0, 2)).astype(npbf16)
    woT = np.ascontiguousarray(
        wo[:, E * g:E * (g + 1)].T.reshape(ET, 128, D).transpose(1, 0, 2)
    ).astype(npbf16)

    m = dict(static)
    m.update({"xT": xT, "wqT": wqT, "wkT": wkT, "wvT": wvT, "woT": woT})
    return m


def kernel(x, wqk, wv, wo):
    x = np.asarray(x, dtype=np.float32)
    wqk = np.asarray(wqk, dtype=np.float32)
    wv = np.asarray(wv, dtype=np.float32)
    wo = np.asarray(wo, dtype=np.float32)

    nc = _get_nc()
    static = _static_inputs()
    in_maps = [
        _core_inputs(x, wqk, wv, wo, static, c // G, c % G) for c in range(8)
    ]
    res = run_bass_kernel_spmd(nc, in_maps, core_ids=list(range(8)))
    out = np.zeros((B, S, D), dtype=np.float32)
    for c in range(8):
        out[c // G] += res.results[c]["out"].astype(np.float32)
    return out


# revision 122
# speedup vs baseline: 1.3038x; 1.0001x over previous
"""Trainium2 Bass kernel for nn_Attention_43301860278871.

Full attention layer: fused QK projection + V projection, interleaved RoPE,
causal SDPA, output projection.  B=2, S=2048, D=2048, H=16, HD=128.

Sharding: 8 cores = 2 batches x 4 head-groups (tensor parallel over heads,
data parallel over batch).  Each core computes 4 heads for one batch and a
partial [S, D] output-projection contribution; the host sums the 4 partials
per batch, so no on-device collectives are needed.

Design (vs the f32r/DRAM-staging baseline at 402us):
  - All matmul operands in bf16 (host-converted; PSUM accumulation stays
    f32), halving DMA and letting Q/K/V stay SBUF-resident for the whole
    kernel - no DRAM round-trip, no phase barrier.  rel err ~8e-3 vs the
    2e-2 gate.
  - RoPE pair-swap via a bf16 permutation matmul (DVE lanes are
    partition-locked, so the swap cannot run there), deferred behind the
    next projection group so the PE never waits on the ACT psum copy.
  - Causal masking at true 128-granularity: per-j-tile scores matmuls
    with trimmed i-extents; the boundary-diagonal 128x128 piece gets a
    second matmul accumulating -1e5 * max(0, j-i) into the scores PSUM
    (lower-tri x upper-tri constant operands), so exp() produces exact
    zeros and no mask multiply exists anywhere on the critical path.
  - Softmax row sums: bf16 exp tiles accumulated over j-tiles on DVE
    (scalar_tensor_tensor, 4x mode) and reduced with ONE ones-column
    matmul per (head, i-chunk) instead of one per j-tile (the baseline
    spent 34us of PE streaming ones-matmuls).  The sums matmul uses an
    all-ones [128,128] stationary so the reduction lands already
    broadcast across every partition (matmul cost depends only on
    streamed rows) - no separate broadcast matmul or copy; normalization
    tails split in two stages and emitted inside the NEXT head's tile
    loop so their cross-engine latency hides behind queued PE work.
  - Global software pipeline: projection chunks interleave with SDPA
    i-chunks (12 proj groups spread over the preceding segment's heads;
    chunk 3's Q(h)/K(h) groups pipeline INTO sdpa(3) between heads), and
    the output projection runs one i-chunk behind SDPA in 4-matmul quanta
    spread across each head's score tiles - so the exp() stream on ACT
    (~87us) never throttles the tensor engine.
  - PSUM: 8 banks as proj/swap/outproj ring (3) + scores (2, the only
    ACT-paced pool, isolated) + PV accumulators (2) + sums/broadcast (1).
  - Single-queue DMA with strict priority order (the DMA path serializes
    at ~350GB/s, so issue order is everything): wq-et0 first, then x
    k-slices interleaved with remaining weight et-slices; chunk-0 Q
    matmuls emitted in DMA-arrival wavefront order.

TimelineSim / HW exec: ~308.7us/core (PE ~94% busy against a ~287us
matmul-stream floor at 1 row/cycle); rel err 7.8e-3.
"""
import sys
sys.path.insert(0, '/opt/trn_rl_repo')

import numpy as np
import ml_dtypes

import concourse.bass as bass
import concourse.mybir as mybir
from concourse.bass_utils import run_bass_kernel_spmd
from concourse.tile import TileContext

B, S, D, H = 2, 2048, 2048, 16
HD = D // H            # 128
G = 4                  # head-groups (cores per batch)
HPG = H // G           # heads per core = 4
E = HPG * HD           # per-core projection width = 512
ROPE_BASE = 10000.0
SCALE = float(HD) ** -0.5

f32 = mybir.dt.float32
bf16 = mybir.dt.bfloat16
npbf16 = ml_dtypes.bfloat16

KT = D // 128          # 16 contraction tiles
TT = S // 128          # 16 token tiles
TC = S // 512          # 4 token chunks
ET = E // 128          # 4 e-tiles (= heads per core)

Exp = mybir.ActivationFunctionType.Exp
mult = mybir.AluOpType.mult
add = mybir.AluOpType.add


# ---------------------------------------------------------------------------
# Workarounds for this walrus build: at most ONE sem wait per instruction.
# Tile's scheduler attaches several; hoist the excess onto NoOps injected on
# the same engine immediately before (sequencer executes waits in order, so
# semantics are identical).
# ---------------------------------------------------------------------------

def _patched_drain_and_barrier(self, tick_clock, wait_clock):
    from concourse.vector_clock import ScopedClock
    drain_inst = self.nc.sync.drain()
    wait_clock.add_sem_waits(
        drain_inst.ins, ScopedClock({None: tick_clock.global_clock})
    )
    si = drain_inst.ins.sync_info
    if si is not None and si.on_wait and len(si.on_wait) > 1:
        waits = list(si.on_wait)
        si.on_wait = waits[:1]
        for w in waits[1:]:
            extra = self.nc.sync.drain()
            esi = extra.ins.sync_info
            if esi is None:
                extra.ins.sync_info = mybir.SyncInfo(on_wait=[w], on_update=[])
            else:
                esi.on_wait = [w]

    self.nc.all_engine_barrier()
    assert self.sems is not None
    popped = self.nc._tile_sem_poison_stack.pop()
    assert popped is self._sem_poison
    self.nc.clear_and_free_semaphores(list(self.sems.allocated().values()))
    self.nc.all_engine_barrier()


def _install_tile_patch():
    import concourse.tile as tile_mod
    tile_mod.TileContext._drain_and_barrier = _patched_drain_and_barrier


def _split_waits(nc, max_waits: int = 1):
    for fn in nc.m.functions:
        for bb in fn.blocks:
            out = []
            changed = False
            for inst in list(bb.instructions):
                si = inst.sync_info
                if si is not None and si.on_wait and len(si.on_wait) > max_waits:
                    waits = list(si.on_wait)
                    for w in waits[:-max_waits]:
                        out.append(mybir.InstNoOp(
                            name=nc.get_next_instruction_name(),
                            engine=inst.engine,
                            sync_info=mybir.SyncInfo(on_wait=[w], on_update=[]),
                        ))
                    si.on_wait = waits[-max_waits:]
                    changed = True
                out.append(inst)
            if changed:
                bb.instructions = out


# ---------------------------------------------------------------------------
# Kernel build (one Bass module, SPMD across the 8 cores via input slices)
# ---------------------------------------------------------------------------

def _build_nc():
    _install_tile_patch()
    nc = bass.Bass()

    xT = nc.dram_tensor("xT", [128, KT, S], bf16, kind="ExternalInput")
    wqT = nc.dram_tensor("wqT", [128, ET, KT, 128], bf16, kind="ExternalInput")
    wkT = nc.dram_tensor("wkT", [128, ET, KT, 128], bf16, kind="ExternalInput")
    wvT = nc.dram_tensor("wvT", [128, KT, E], bf16, kind="ExternalInput")
    woT = nc.dram_tensor("woT", [128, ET, D], bf16, kind="ExternalInput")
    cosF = nc.dram_tensor("cosF", [128, S], bf16, kind="ExternalInput")
    sinF = nc.dram_tensor("sinF", [128, S], bf16, kind="ExternalInput")
    triL = nc.dram_tensor("triL", [128, 128], bf16, kind="ExternalInput")
    triU = nc.dram_tensor("triU", [128, 128], bf16, kind="ExternalInput")
    rperm = nc.dram_tensor("rperm", [128, 128], bf16, kind="ExternalInput")
    onesq = nc.dram_tensor("onesq", [128, 128], bf16, kind="ExternalInput")
    out = nc.dram_tensor("out", [S, D], bf16, kind="ExternalOutput")

    with TileContext(nc) as tc:
        with (
            nc.allow_low_precision(reason="bf16 datapath, fp32 accumulation"),
            tc.tile_pool(name="consts", bufs=1) as consts,
            tc.tile_pool(name="resid", bufs=1) as resid,
        ):
            lt_sb = consts.tile([128, 128], bf16, tag="lt")
            ut_sb = consts.tile([128, 128], bf16, tag="ut")
            oq_sb = consts.tile([128, 128], bf16, tag="oq")
            rp_sb = consts.tile([128, 128], bf16, tag="rp")
            c_sb = consts.tile([128, S], bf16, tag="cos")
            s_sb = consts.tile([128, S], bf16, tag="sin")
            # consts go on the idle Pool queue so they don't block the
            # critical first-chunk loads on the SP queue
            nc.gpsimd.dma_start(rp_sb[:], rperm[:])
            nc.gpsimd.dma_start(lt_sb[:], triL[:])
            nc.gpsimd.dma_start(ut_sb[:], triU[:])
            nc.gpsimd.dma_start(oq_sb[:], onesq[:])

            # SBUF-resident Q^T/K^T (e-major per head) and V (t-major)
            qres = resid.tile([128, ET, S], bf16, tag="qres")
            kres = resid.tile([128, ET, S], bf16, tag="kres")
            vres = resid.tile([128, TT, E], bf16, tag="vres")
            wo_sb = resid.tile([128, ET, D], bf16, tag="wo")

            with (
                tc.tile_pool(name="wpool", bufs=1) as wpool,
                tc.tile_pool(name="xpool", bufs=3) as xpool,
                tc.tile_pool(name="stage", bufs=4) as stage,
                tc.tile_pool(name="oTp", bufs=2) as oTp,
                tc.tile_pool(name="ptp", bufs=4) as ptp,
                tc.tile_pool(name="accp", bufs=2) as accp,
                tc.tile_pool(name="rcp", bufs=2) as rcp,
                tc.tile_pool(name="ostp", bufs=2) as ostp,
                # PSUM: proj-acc/rope-swap/outproj ring (3) + scores (2,
                # ACT-paced so isolated) + PV accumulators (2) + softmax
                # sums/broadcast alternating in one bank = 8 banks
                tc.tile_pool(name="psMM", bufs=3, space="PSUM") as psMM,
                tc.tile_pool(name="psC", bufs=2, space="PSUM") as psC,
                tc.tile_pool(name="psD", bufs=2, space="PSUM") as psD,
                tc.tile_pool(name="psAux", bufs=1, space="PSUM") as psAux,
            ):
                wq_sb = wpool.tile([128, ET, KT, 128], bf16, tag="wq")
                wk_sb = wpool.tile([128, ET, KT, 128], bf16, tag="wk")
                wv_sb = wpool.tile([128, KT, E], bf16, tag="wv")

                xc_t = {}
                oT_c = {}
                pre_po = []
                # deferred-emission slots: PE-blocking ops postponed until
                # the engine has other queued work covering their input
                # latency (ACT copy for the rope swap, DVE chain for the
                # softmax normalization)
                pending = {"swap": None, "tail1": None, "tail2": None}

                def flush(key):
                    fn = pending[key]
                    if fn is not None:
                        pending[key] = None
                        fn()

                def emit_chunk_loads(tcb):
                    ts = slice(tcb * 512, (tcb + 1) * 512)
                    xc = xpool.tile([128, KT, 512], bf16, tag="xc")
                    xc_t[tcb] = xc
                    if tcb == 0:
                        # single queue: issue order == transfer order on
                        # the (serialized) DMA path, so strict priority:
                        # first-group inputs, then just-in-time streaming
                        nc.sync.dma_start(wq_sb[:, 0, 0:8], wqT[:, 0, 0:8])
                        nc.sync.dma_start(xc[:, 0:1], xT[:, 0:1, ts])
                        nc.sync.dma_start(wq_sb[:, 0, 8:16], wqT[:, 0, 8:16])
                        nc.sync.dma_start(xc[:, 1:4], xT[:, 1:4, ts])
                        nc.sync.dma_start(xc[:, 4:8], xT[:, 4:8, ts])
                        nc.sync.dma_start(wq_sb[:, 1], wqT[:, 1])
                        nc.sync.dma_start(wq_sb[:, 2], wqT[:, 2])
                        nc.sync.dma_start(wq_sb[:, 3], wqT[:, 3])
                        nc.sync.dma_start(xc[:, 8:12], xT[:, 8:12, ts])
                        nc.sync.dma_start(xc[:, 12:16], xT[:, 12:16, ts])
                        nc.sync.dma_start(c_sb[:], cosF[:])
                        nc.sync.dma_start(s_sb[:], sinF[:])
                        for et in range(ET):
                            nc.sync.dma_start(wk_sb[:, et], wkT[:, et])
                        for kc in range(4):
                            ks = slice(kc * 4, (kc + 1) * 4)
                            nc.sync.dma_start(wv_sb[:, ks], wvT[:, ks])
                    else:
                        for kc in range(4):
                            ks = slice(kc * 4, (kc + 1) * 4)
                            nc.sync.dma_start(xc[:, ks], xT[:, ks, ts])
                    if tcb == 1:
                        for dcc in range(4):
                            dsl = slice(dcc * 512, (dcc + 1) * 512)
                            nc.sync.dma_start(wo_sb[:, :, dsl], woT[:, :, dsl])

                def proj_groups(tcb):
                    # 12 projection matmul groups for one x chunk, as
                    # closures so they can interleave into SDPA segments.
                    # The RoPE pair-swap matmul of each group is deferred
                    # behind the next group so the PE never waits on the
                    # ACT psum copy.
                    ts = slice(tcb * 512, (tcb + 1) * 512)
                    xc = xc_t[tcb]
                    groups = []

                    def mkqk(w_sb, dst, et):
                        def g():
                            pq = psMM.tile([128, 512], f32, tag="mm")
                            for k in range(KT):
                                nc.tensor.matmul(
                                    pq[:], w_sb[:, et, k, :], xc[:, k, :],
                                    start=(k == 0), stop=(k == KT - 1),
                                )
                                if k == 7:
                                    # mid-group: the previous swap's DVE
                                    # chain gets cover before anything
                                    # reads its qres/kres write, and the
                                    # qsb ACT copy has had time to land
                                    flush("swap")
                            qsb = stage.tile([128, 512], bf16, tag="qsb")
                            nc.vector.tensor_copy(qsb[:], pq[:])

                            def f():
                                ps2 = psMM.tile([128, 512], f32, tag="mm")
                                nc.tensor.matmul(ps2[:], rp_sb[:], qsb[:],
                                                 start=True, stop=True)
                                t1 = stage.tile([128, 512], bf16, tag="t1")
                                t2 = stage.tile([128, 512], bf16, tag="t2")
                                nc.vector.tensor_tensor(
                                    t1[:], qsb[:], c_sb[:, ts], mult)
                                nc.vector.tensor_tensor(
                                    t2[:], ps2[:], s_sb[:, ts], mult)
                                nc.vector.tensor_tensor(
                                    dst[:, et, ts], t1[:], t2[:], add)
                            pending["swap"] = f
                        return g

                    def mkv(tt):
                        def g():
                            pv = psMM.tile([128, 512], f32, tag="mm")
                            for k in range(KT):
                                nc.tensor.matmul(
                                    pv[:], xc[:, k, tt * 128:(tt + 1) * 128],
                                    wv_sb[:, k, :],
                                    start=(k == 0), stop=(k == KT - 1),
                                )
                                if k == 7:
                                    flush("swap")
                            nc.scalar.copy(vres[:, 4 * tcb + tt, :], pv[:])
                        return g

                    for (w_sb, dst) in ((wq_sb, qres), (wk_sb, kres)):
                        for et in range(ET):
                            groups.append(mkqk(w_sb, dst, et))
                    for tt in range(4):
                        groups.append(mkv(tt))
                    return groups

                def emit_outproj_tile(tt, oT, eager_dma=False,
                                      copies_on_act=False, alt_queue=False):
                    # one t-tile of the output projection: 4 dc-quanta of 4
                    # matmuls each, returned as callables to interleave
                    tl = tt % 4
                    quanta = []
                    ost = ostp.tile([128, D], bf16, tag="ost")
                    rsl = slice(tt * 128, (tt + 1) * 128)

                    def mk(dc):
                        def q():
                            po = psMM.tile([128, 512], f32, tag="mm")
                            for eh in range(ET):
                                nc.tensor.matmul(
                                    po[:],
                                    oT[:, eh, tl * 128:(tl + 1) * 128],
                                    wo_sb[:, eh, dc * 512:(dc + 1) * 512],
                                    start=(eh == 0), stop=(eh == ET - 1),
                                )
                            dsl = slice(dc * 512, (dc + 1) * 512)
                            if dc < 3 and not copies_on_act:
                                nc.vector.tensor_copy(ost[:, dsl], po[:])
                            else:
                                nc.scalar.copy(ost[:, dsl], po[:])
                            if eager_dma:
                                nc.sync.dma_start(out[rsl, dsl], ost[:, dsl])
                        return q
                    for dc in range(4):
                        quanta.append(mk(dc))

                    def fin():
                        if not eager_dma:
                            nc.sync.dma_start(out[rsl, :], ost[:])
                    return quanta, fin

                def emit_sdpa(ic, gbh):
                    isl = slice(ic * 512, (ic + 1) * 512)
                    oT_ic = oTp.tile([128, ET, 512], bf16, tag="oT")
                    oT_c[ic] = oT_ic
                    for h in range(ET):
                        # deferred output projection work (one ic behind)
                        if ic > 0:
                            quanta, fin = emit_outproj_tile(
                                4 * (ic - 1) + h, oT_c[ic - 1],
                                copies_on_act=(ic == TC - 1 and h == ET - 1))
                        else:
                            quanta, fin = [], None
                        qtb = qres[:, h, isl]
                        ps_pv = psD.tile([128, 512], f32, tag="pv")
                        acc = accp.tile([128, 512], bf16, tag="acc")
                        # j-tiles: full below the diagonal block, then the
                        # 4 staircase tiles (i-extent shrinks by 128 each)
                        tiles = [(jt, 0) for jt in range(4 * ic)]
                        tiles += [(4 * ic + r, 128 * r) for r in range(4)]
                        n = len(tiles)
                        pts = [None] * n
                        LOOK = 3
                        quanta_done = [0]

                        def emit_pv(idx):
                            jt, ilo = tiles[idx]
                            nc.tensor.matmul(
                                ps_pv[:, ilo:512],
                                vres[:, jt, h * 128:(h + 1) * 128],
                                pts[idx][:, ilo:512],
                                start=(idx == 0), stop=(idx == n - 1),
                                skip_group_check=True,
                            )

                        for idx, (jt, ilo) in enumerate(tiles):
                            ps_sc = psC.tile([128, 512], f32, tag="sc")
                            nc.tensor.matmul(
                                ps_sc[:, ilo:512],
                                kres[:, h, jt * 128:(jt + 1) * 128],
                                qtb[:, ilo:512],
                                start=True, stop=True,
                            )
                            if jt >= 4 * ic:
                                # boundary-diagonal piece: accumulate
                                # -1e5*max(0, j-i) so exp gives exact zeros
                                nc.tensor.matmul(
                                    ps_sc[:, ilo:ilo + 128],
                                    lt_sb[:], ut_sb[:],
                                    start=False, stop=True,
                                    skip_group_check=True,
                                )
                            pt = ptp.tile([128, 512], bf16, tag="pt")
                            pts[idx] = pt
                            nc.scalar.activation(
                                pt[:, ilo:512], ps_sc[:, ilo:512], Exp,
                                scale=SCALE)
                            if idx == 0:
                                nc.vector.tensor_copy(acc[:], pt[:])
                            else:
                                nc.vector.scalar_tensor_tensor(
                                    acc[:, ilo:512], pt[:, ilo:512], 1.0,
                                    acc[:, ilo:512], mult, add)
                            if h == 0 and idx == 0:
                                # h0 quanta read every head slice of the
                                # previous oT - the pending tail must land
                                # before the first quantum
                                flush("tail1")
                                flush("tail2")
                            elif idx == 1:
                                flush("tail1")
                            elif idx == 2:
                                flush("swap")
                            elif idx == (4 if n > 4 else 3):
                                flush("tail2")
                            st = max(1, n // 4)
                            if quanta and idx % st == 0 and idx // st < 4:
                                quanta[idx // st]()
                                quanta_done[0] = idx // st + 1
                            if idx >= LOOK:
                                emit_pv(idx - LOOK)
                        for qi in range(quanta_done[0], len(quanta)):
                            quanta[qi]()
                        for idx in range(max(0, n - LOOK), n):
                            emit_pv(idx)
                        if fin is not None:
                            fin()

                        if ic == TC - 1 and h == ET - 1:
                            # pre-start five tail-outproj quanta with the
                            # three already-normalized heads, borrowing
                            # the now-idle scores and PV psum banks; the
                            # eh3 finishes land right after the last norm.
                            # This fills the sums->recip->broadcast->mult
                            # chain window with real PE work.
                            specs = [(psMM, "mm"), (psMM, "mm"),
                                     (psC, "sc"), (psC, "sc"), (psD, "pv")]
                            for qn, (pool, tg) in enumerate(specs):
                                ptt = qn // 4          # 0 -> tt12, 1 -> tt13
                                tl = ptt               # tt % 4
                                dc = qn % 4
                                po = pool.tile([128, 512], f32, tag=tg,
                                               name=f"pre{qn}")
                                for eh in range(3):
                                    nc.tensor.matmul(
                                        po[:],
                                        oT_ic[:, eh, tl * 128:(tl + 1) * 128],
                                        wo_sb[:, eh, dc * 512:(dc + 1) * 512],
                                        start=(eh == 0), stop=False,
                                    )
                                pre_po.append(po)

                        # normalization tail, deferred into the next head's
                        # tile loop: sums -> 1/sums, then broadcast -> mult
                        last = (ic == TC - 1 and h == ET - 1)

                        def mktails(acc=acc, ps_pv=ps_pv, oT_ic=oT_ic, h=h,
                                    split=last):
                            rcb = rcp.tile([128, 512], bf16, tag="rcb")

                            def t1():
                                # all-ones [128,128] stationary: the sums
                                # reduction lands already broadcast across
                                # every partition (same streamed-row cost)
                                ps_sums = psAux.tile([128, 512], f32,
                                                     tag="aux")
                                nc.tensor.matmul(ps_sums[:], oq_sb[:],
                                                 acc[:], start=True,
                                                 stop=True)
                                nc.vector.reciprocal(rcb[:], ps_sums[:])

                            def t2():
                                if split:
                                    # 128-col pieces so the dependent
                                    # output projection can start after
                                    # the first piece lands
                                    for tl in range(4):
                                        tls = slice(tl * 128, (tl + 1) * 128)
                                        nc.vector.tensor_tensor(
                                            oT_ic[:, h, tls], ps_pv[:, tls],
                                            rcb[:, tls], mult)
                                else:
                                    nc.vector.tensor_tensor(
                                        oT_ic[:, h, :], ps_pv[:], rcb[:],
                                        mult)
                            return t1, t2
                        flush("tail1")
                        flush("tail2")
                        pending["tail1"], pending["tail2"] = mktails()

                        # next chunk's projection groups: pure PE work
                        # that fills the exp-paced bubbles, with the
                        # normalization tail flushed in between
                        for gi, g in enumerate(gbh[h]):
                            g()
                            if gi == 0:
                                flush("tail1")
                            elif gi == 1:
                                flush("tail2")

                # interleave projection chunks with SDPA i-chunks so ACT's
                # exp stream never throttles the tensor engine.  chunk 3's
                # Q(h)/K(h) groups pipeline INTO sdpa(3) between heads
                # (each head only needs its own q/k slices), keeping the
                # final - otherwise exp-bound - segment fed with PE work.
                emit_chunk_loads(0)
                # chunk-0 Q groups in DMA-arrival wavefront order: the
                # matmuls of et0-2 interleave by k-slice arrival so the PE
                # streams from the first weight landing instead of
                # serializing whole (DMA-gated) groups
                g0 = proj_groups(0)
                wave_pq = {}

                def wv_mm(et, k0, k1, close=False):
                    if et not in wave_pq:
                        # et2/et3 borrow the (still idle) scores banks so
                        # all four groups can be open concurrently
                        pool, tg = (psMM, "mm") if et < 2 else (psC, "sc")
                        wave_pq[et] = pool.tile([128, 512], f32, tag=tg,
                                                name=f"wavepq{et}")
                    pq = wave_pq[et]
                    for k in range(k0, k1):
                        nc.tensor.matmul(
                            pq[:], wq_sb[:, et, k, :], xc_t[0][:, k, :],
                            start=(k == 0), stop=(close and k == k1 - 1),
                        )
                    if close:
                        qsb = stage.tile([128, 512], bf16, tag="qsb")
                        nc.scalar.copy(qsb[:], pq[:])

                        def f(qsb=qsb, et=et):
                            ps2 = psMM.tile([128, 512], f32, tag="mm")
                            nc.tensor.matmul(ps2[:], rp_sb[:], qsb[:],
                                             start=True, stop=True)
                            t1 = stage.tile([128, 512], bf16, tag="t1")
                            t2 = stage.tile([128, 512], bf16, tag="t2")
                            nc.vector.tensor_tensor(
                                t1[:], qsb[:], c_sb[:, 0:512], mult)
                            nc.vector.tensor_tensor(
                                t2[:], ps2[:], s_sb[:, 0:512], mult)
                            nc.vector.tensor_tensor(
                                qres[:, et, 0:512], t1[:], t2[:], add)
                        flush("swap")
                        pending["swap"] = f

                wv_mm(0, 0, 4)
                wv_mm(0, 4, 8)
                wv_mm(1, 0, 8)
                wv_mm(2, 0, 8)
                wv_mm(3, 0, 8)
                wv_mm(0, 8, 12)
                wv_mm(1, 8, 12)
                wv_mm(2, 8, 12)
                wv_mm(3, 8, 12)
                wv_mm(0, 12, 16, close=True)
                wv_mm(1, 12, 16, close=True)
                wv_mm(2, 12, 16, close=True)
                wv_mm(3, 12, 16, close=True)
                for g in g0[4:12]:
                    g()
                emit_chunk_loads(1)
                emit_chunk_loads(2)
                g1 = proj_groups(1)
                emit_sdpa(0, [g1[0:3], g1[3:6], g1[6:9], g1[9:12]])
                emit_chunk_loads(3)
                g2 = proj_groups(2)
                emit_sdpa(1, [g2[0:3], g2[3:6], g2[6:9], g2[9:12]])
                g3 = proj_groups(3)   # Q0-3 = g3[0:4], K0-3 = g3[4:8], V0-3 = g3[8:12]
                emit_sdpa(2, [[g3[8], g3[9]], [g3[10], g3[11]],
                              [g3[0]], [g3[4]]])
                emit_sdpa(3, [[g3[1], g3[5]], [g3[2], g3[6]],
                              [g3[3], g3[7]], []])

                # tail: output projection for the last i-chunk; the first
                # five quanta were pre-started with eh0-2 and only need
                # the eh3 matmul here
                flush("tail1")
                flush("tail2")
                oT3 = oT_c[TC - 1]
                for h in range(ET):
                    tt = 4 * (TC - 1) + h
                    tl = tt % 4
                    ost = ostp.tile([128, D], bf16, tag="ost",
                                    name=f"osttail{h}")
                    rsl = slice(tt * 128, (tt + 1) * 128)
                    for dc in range(4):
                        qn = h * 4 + dc
                        dsl = slice(dc * 512, (dc + 1) * 512)
                        if qn < len(pre_po):
                            po = pre_po[qn]
                            nc.tensor.matmul(
                                po[:], oT3[:, 3, tl * 128:(tl + 1) * 128],
                                wo_sb[:, 3, dsl], start=False, stop=True)
                        else:
                            po = psMM.tile([128, 512], f32, tag="mm",
                                           name=f"tl{qn}")
                            for eh in range(ET):
                                nc.tensor.matmul(
                                    po[:],
                                    oT3[:, eh, tl * 128:(tl + 1) * 128],
                                    wo_sb[:, eh, dsl],
                                    start=(eh == 0), stop=(eh == ET - 1),
                                )
                        if dc % 2 == 0:
                            nc.vector.tensor_copy(ost[:, dsl], po[:])
                        else:
                            nc.scalar.copy(ost[:, dsl], po[:])
                        nc.sync.dma_start(out[rsl, dsl], ost[:, dsl])

    _split_waits(nc)
    return nc


_NC = None


def _get_nc():
    global _NC
    if _NC is None:
        _NC = _build_nc()
    return _NC


# ---------------------------------------------------------------------------
# Host-side prep + gather
# ---------------------------------------------------------------------------

def _rope_tables():
    j = np.arange(0, HD, 2, dtype=np.float32) / HD
    inv_freq = (1.0 / (ROPE_BASE ** j)).astype(np.float32)           # [64]
    t = np.arange(S, dtype=np.float32)
    freqs = np.outer(inv_freq, t)                                    # [64, S]
    cos = np.cos(freqs)
    sin = np.sin(freqs)
    cosF = np.empty((128, S), dtype=np.float32)
    sinF = np.empty((128, S), dtype=np.float32)
    cosF[0::2] = cos
    cosF[1::2] = cos
    sinF[0::2] = -sin
    sinF[1::2] = sin
    return cosF.astype(npbf16), sinF.astype(npbf16)


def _static_inputs():
    cosF, sinF = _rope_tables()
    k = np.arange(128)
    triL = (k[:, None] < k[None, :]).astype(np.float32)      # [k, j] k<j
    triU = np.where(k[:, None] >= k[None, :], -1e5, 0.0)     # [k, i] k>=i
    onesq = np.ones((128, 128), dtype=np.float32)
    rperm = np.zeros((128, 128), dtype=np.float32)
    idx = np.arange(128)
    rperm[idx ^ 1, idx] = 1.0
    return {
        "cosF": cosF, "sinF": sinF,
        "triL": triL.astype(npbf16), "triU": triU.astype(npbf16),
        "onesq": onesq.astype(npbf16), "rperm": rperm.astype(npbf16),
    }


def _core_inputs(x, wqk, wv, wo, static, b, g):
    xb = np.ascontiguousarray(x[b].T)                                # [D, S]
    xT = np.ascontiguousarray(
        xb.reshape(KT, 128, S).transpose(1, 0, 2)).astype(npbf16)

    wq_g = wqk[E * g:E * (g + 1), :]                                 # [E, D]
    wk_g = wqk[D + E * g:D + E * (g + 1), :]
    wv_g = wv[E * g:E * (g + 1), :]
    wqT = np.ascontiguousarray(
        wq_g.T.reshape(KT, 128, ET, 128).transpose(1, 2, 0, 3)).astype(npbf16)
    wkT = np.ascontiguousarray(
        wk_g.T.reshape(KT, 128, ET, 128).transpose(1, 2, 0, 3)).astype(npbf16)
    wvT = np.ascontiguousarray(
        wv_g.T.reshape(KT, 128, E).transpose(1, 0, 2)).astype(npbf16)
    woT = np.ascontiguousarray(
        wo[:, E * g:E * (g + 1)].T.reshape(ET, 128, D).transpose(1, 0, 2)
    ).astype(npbf16)

    m = dict(static)
    m.update({"xT": xT, "wqT": wqT, "wkT": wkT, "wvT": wvT, "woT": woT})
    return m


def kernel(x, wqk, wv, wo):
    x = np.asarray(x, dtype=np.float32)
    wqk = np.asarray(wqk, dtype=np.float32)
    wv = np.asarray(wv, dtype=np.float32)
    wo = np.asarray(wo, dtype=np.float32)

    nc = _get_nc()
    static = _static_inputs()
    in_maps = [
        _core_inputs(x, wqk, wv, wo, static, c // G, c % G) for c in range(8)
    ]
    res = run_bass_kernel_spmd(nc, in_maps, core_ids=list(range(8)))
    out = np.zeros((B, S, D), dtype=np.float32)
    for c in range(8):
        out[c // G] += res.results[c]["out"].astype(np.float32)
    return out
